# revision 1
# baseline (speedup 1.0000x reference)
"""Trainium2 Bass kernel for nn_ConvAttLIF (conv3x3 + temporal attention + LIF scan).

Sharding: data-parallel over batch B=16 across 8 NeuronCores (2 samples/core).

Layout: frames are host-padded to 34x34 (+2 guard cols) so every conv tap is a
contiguous SBUF window and every matmul output a contiguous PSUM window
(strided matmul APs are illegal on TRN2). The 9 taps run as K=64 matmuls
tile-position packed across the two PE row halves into two PSUM accumulators
(shared-PSUM cross-half accumulation crashes, separate tiles are exact).

Precision: matmuls run in float32r (fp32 rounded to 11 mantissa bits,
1 cycle/row vs 4 for fp32). Inputs/weights are split hi/lo on the host
(x_hi = trunc13(x)) and the conv computes x_hi*w_hi + x_hi*w_lo + x_lo*w_hi,
giving ~fp32 accuracy (needed: the output is binary spikes u >= 0.6) at
3 bf16-rate passes.

LIF scan: attention folded into the recurrence via v_t = u_t / att_t, so each
step is v = g*c_t + y (DVE fused), spm = Sign(v - thr_t) (ACT), g = v*[spm<0]
(DVE fused), spike = Relu(spm) (ACT).

kernel(**inputs) takes the FULL unsharded inputs, returns the FULL output.
"""
import sys

sys.path.insert(0, "/opt/trn_rl_repo")

import numpy as np
import concourse.bass as bass
import concourse.bacc as bacc
import concourse.tile as tile
import concourse.mybir as mybir
from concourse.bass_utils import run_bass_kernel_spmd

F32 = mybir.dt.float32
F32R = mybir.dt.float32r
AF = mybir.ActivationFunctionType
OP = mybir.AluOpType

B, T, CIN, H, W = 16, 20, 64, 32, 32
CH = 128
N_CORES = 8
BPC = B // N_CORES
ALPHA, VTH = 0.3, 0.6
HW = H * W                     # 1024
PW = H + 2                     # 34 padded width/height
FLAT = PW * PW                 # 1156
XCOL = FLAT + 2                # 1158 with guard cols
NY = 26                        # y-tile ring size

CONV_MODE = "f32r3"            # "f32" (native fp32) or "f32r3" (3-pass split)

TAPS = [(dy, dx) for dy in (-1, 0, 1) for dx in (-1, 0, 1)]
# output span: padded positions 34..1122 (rows 1..32, all 34 cols)
# equal ~363-col chunks: all >=256 so f32r streams at 1 cycle/row
# (fp32r matmul requires the moving-dim count to be a multiple of 4)
CH_N = [364, 364, 360]         # psum bank chunks (each <=512, bank-aligned)
CH_OFF = [PW, PW + 364, PW + 728]    # y-offset of each chunk


def _build_program():
    nc = bacc.Bacc("TRN2", target_bir_lowering=False, debug=False,
                   num_devices=N_CORES)

    f32r3 = CONV_MODE == "f32r3"
    mm_dt = F32R if f32r3 else F32
    xhi_d = nc.dram_tensor("xhi", [BPC, T, CIN, XCOL], F32,
                           kind="ExternalInput").ap()
    xlo_d = wlo_d = None
    if f32r3:
        xlo_d = nc.dram_tensor("xlo", [BPC, T, CIN, XCOL], F32,
                               kind="ExternalInput").ap()
        wlo_d = nc.dram_tensor("wcorr", [128, 9 * 128], F32,
                               kind="ExternalInput").ap()
    wtap_d = nc.dram_tensor("wtap", [128, 9 * 128], F32, kind="ExternalInput").ap()
    bias_d = nc.dram_tensor("bias", [128, 1], F32, kind="ExternalInput").ap()
    w1t_d = nc.dram_tensor("w1t", [T, 5], F32, kind="ExternalInput").ap()
    w2t_d = nc.dram_tensor("w2t", [5, T], F32, kind="ExternalInput").ap()
    ident_d = nc.dram_tensor("ident", [128, 128], F32, kind="ExternalInput").ap()
    spk = nc.dram_tensor("spk", [BPC, T, CH, H, W], F32, kind="ExternalOutput").ap()

    with tile.TileContext(nc) as tc:
        with tc.tile_pool(name="sb", bufs=1) as P1, \
             tc.tile_pool(name="scr", bufs=2) as P2, \
             tc.tile_pool(name="so", bufs=3) as P3, \
             tc.tile_pool(name="ps", bufs=1, space="PSUM") as PP:

            # ---- persistent tiles ----
            wt = P1.tile([128, 9 * 128], mm_dt, tag="wt", name="wt")
            nc.sync.dma_start(wt[:], wtap_d[:].bitcast(mm_dt))
            wt_lo = None
            if f32r3:
                wt_lo = P1.tile([128, 9 * 128], F32R, tag="wtlo", name="wtlo")
                nc.sync.dma_start(wt_lo[:], wlo_d[:].bitcast(F32R))
            bias_t = P1.tile([128, 1], F32, tag="bias", name="bias")
            nc.sync.dma_start(bias_t[:], bias_d[:])
            w1t_s = P1.tile([T, 5], F32, tag="w1t", name="w1t")
            nc.sync.dma_start(w1t_s[:], w1t_d[:])
            w2t_s = P1.tile([5, T], F32, tag="w2t", name="w2t")
            nc.sync.dma_start(w2t_s[:], w2t_d[:])
            ident = P1.tile([128, 128], F32, tag="ident", name="ident")
            nc.sync.dma_start(ident[:], ident_d[:])
            ones_t = P1.tile([1, 128], F32, tag="ones", name="ones")
            nc.vector.memset(ones_t[:], 1.0)

            ys = [P1.tile([128, FLAT], F32, tag=f"y{i}", name=f"y{i}")
                  for i in range(NY)]
            xhs = [P1.tile([128, XCOL], mm_dt, tag=f"xh{i}", name=f"xh{i}")
                   for i in range(3)]
            xls = [P1.tile([128, XCOL], F32R, tag=f"xl{i}", name=f"xl{i}")
                   for i in range(3)] if f32r3 else []
            g_t = P1.tile([128, HW], F32, tag="g", name="g")
            # per-frame stats: 3 chunk-sums, junkL, junkR, max
            s_st = [P1.tile([128, 6 * T], F32, tag=f"S{s}", name=f"S{s}")
                    for s in range(BPC)]
            bc = [P1.tile([128, 2 * T], F32, tag=f"bc{s}", name=f"bc{s}")
                  for s in range(BPC)]

            def yview(y):
                return y.rearrange("p (r c) -> p r c", c=PW)

            def conv_frame(s, t):
                f = s * T + t
                xh = xhs[f % 3]
                for h in range(2):
                    nc.sync.dma_start(xh[h * 64:(h + 1) * 64, :],
                                      xhi_d[s, t].bitcast(mm_dt))
                if f32r3:
                    xl = xls[f % 3]
                    nc.sync.dma_start(xl[0:64, :], xhi_d[s, t].bitcast(F32R))
                    nc.sync.dma_start(xl[64:128, :], xlo_d[s, t].bitcast(F32R))

                psA = PP.tile([128, 3 * 512], F32, tag="psA", name="psA")
                psB = PP.tile([128, 3 * 512], F32, tag="psB", name="psB")
                ps = [psA, psB]

                # units: (psum_idx, x_tile, w_tile, tap, chunk, full_k)
                # corr first (tiny terms accumulate losslessly), as single
                # K=128 stacked matmuls [x_hi; x_lo] . [w_lo; w_hi]; then the
                # main K=64 pass tile-position packed across the row halves.
                order = []
                if f32r3:
                    for j in range(9):
                        for c in range(3):
                            order.append(((j + c) % 2, xls[f % 3], wt_lo,
                                          j, c, True))
                halves = ([], [])
                for j in range(9):
                    for c in range(3):
                        halves[(j + c) % 2].append(
                            (xhs[f % 3], wt, j, c, False))
                for i in range(max(len(halves[0]), len(halves[1]))):
                    for h in range(2):
                        if i < len(halves[h]):
                            order.append((h,) + halves[h][i])
                n_units = {}
                for (h, x_t, w_t, j, c, fk) in order:
                    n_units[(h, c)] = n_units.get((h, c), 0) + 1
                cnt = {k: 0 for k in n_units}
                for (h, x_t, w_t, j, c, fk) in order:
                    dy, dx = TAPS[j]
                    n = CH_N[c]
                    base = 1 + CH_OFF[c] + dy * PW + dx
                    cnt[(h, c)] += 1
                    kw = dict(start=(cnt[(h, c)] == 1),
                              stop=(cnt[(h, c)] == n_units[(h, c)]))
                    if fk:
                        nc.tensor.matmul(
                            ps[h][:, c * 512:c * 512 + n],
                            w_t[0:128, j * 128:(j + 1) * 128],
                            x_t[0:128, base:base + n], **kw)
                    else:
                        nc.tensor.matmul(
                            ps[h][:, c * 512:c * 512 + n],
                            w_t[h * 64:(h + 1) * 64, j * 128:(j + 1) * 128],
                            x_t[h * 64:(h + 1) * 64, base:base + n],
                            tile_position=(h * 64, 0), **kw)

                yB = P2.tile([128, 3 * 512], F32, tag="yB", name="yB")
                y = ys[f % NY]
                for c in range(3):
                    n = CH_N[c]
                    nc.scalar.activation(yB[:, c * 512:c * 512 + n],
                                         ps[1][:, c * 512:c * 512 + n],
                                         AF.Identity, bias=bias_t[:, 0:1])
                    nc.vector.scalar_tensor_tensor(
                        y[:, CH_OFF[c]:CH_OFF[c] + n],
                        ps[0][:, c * 512:c * 512 + n], 0.0,
                        yB[:, c * 512:c * 512 + n],
                        op0=OP.add, op1=OP.add,
                        accum_out=s_st[s][:, c * T + t:c * T + t + 1])
                yv = yview(y)
                # junk column sums (pad cols 0 and 33 of rows 1..32)
                nc.vector.reduce_sum(s_st[s][:, 3 * T + t:3 * T + t + 1],
                                     yv[:, 1:33, 0:1],
                                     axis=mybir.AxisListType.XY)
                nc.vector.reduce_sum(s_st[s][:, 4 * T + t:4 * T + t + 1],
                                     yv[:, 1:33, 33:34],
                                     axis=mybir.AxisListType.XY)
                nc.vector.reduce_max(s_st[s][:, 5 * T + t:5 * T + t + 1],
                                     yv[:, 1:33, 1:33],
                                     axis=mybir.AxisListType.XY)

            def attention(s):
                S = s_st[s]
                stot = P2.tile([128, T], F32, tag="stot", name="stot")
                nc.vector.tensor_tensor(stot[:], S[:, 0:T], S[:, T:2 * T],
                                        op=OP.add)
                nc.vector.tensor_tensor(stot[:], stot[:], S[:, 2 * T:3 * T],
                                        op=OP.add)
                nc.vector.tensor_tensor(stot[:], stot[:], S[:, 3 * T:4 * T],
                                        op=OP.subtract)
                nc.vector.tensor_tensor(stot[:], stot[:], S[:, 4 * T:5 * T],
                                        op=OP.subtract)
                psTs = PP.tile([T, 128], F32, tag="psA", name="psTs")
                psTm = PP.tile([T, 128], F32, tag="psB", name="psTm")
                nc.tensor.transpose(psTs[:], stot[:], ident[:])
                nc.tensor.transpose(psTm[:], S[:, 5 * T:6 * T], ident[:])
                att_in = P2.tile([T, 2], F32, tag="att_in", name="att_in")
                tmp = P2.tile([T, 1], F32, tag="att_tmp", name="att_tmp")
                nc.vector.reduce_sum(tmp[:], psTs[:], axis=mybir.AxisListType.X)
                nc.vector.tensor_scalar_mul(att_in[:, 0:1], tmp[:],
                                            1.0 / (CH * HW))
                nc.vector.reduce_max(att_in[:, 1:2], psTm[:],
                                     axis=mybir.AxisListType.X)
                ps5 = PP.tile([5, 2], F32, tag="psA", name="ps5")
                nc.tensor.matmul(ps5[:], w1t_s[:], att_in[:], start=True,
                                 stop=True)
                h5 = P2.tile([5, 2], F32, tag="h5", name="h5")
                nc.scalar.activation(h5[:], ps5[:], AF.Relu)
                ps20 = PP.tile([T, 2], F32, tag="psB", name="ps20")
                nc.tensor.matmul(ps20[:], w2t_s[:], h5[:], start=True, stop=True)
                a20 = P2.tile([T, 2], F32, tag="a20", name="a20")
                nc.scalar.activation(a20[:], ps20[:], AF.Copy)
                attp = P2.tile([T, 1], F32, tag="attp", name="attp")
                nc.vector.tensor_tensor(attp[:], a20[:, 0:1], a20[:, 1:2],
                                        op=OP.add)
                # sigmoid via exp + reciprocal (tighter than the Sigmoid table)
                expz = P2.tile([T, 1], F32, tag="expz", name="expz")
                nc.scalar.activation(expz[:], attp[:], AF.Exp, scale=-1.0)
                att1 = P2.tile([T, 1], F32, tag="att1", name="att1")
                nc.vector.tensor_scalar_add(att1[:], expz[:], 1.0)
                att = P2.tile([T, 1], F32, tag="att", name="att")
                nc.vector.reciprocal(att[:], att1[:])
                asc = P2.tile([1, T + 1], F32, tag="asc", name="asc")
                nc.sync.dma_start(asc[0:1, 1:T + 1], att[:, 0:1])
                nc.sync.dma_start(asc[0:1, 0:1], att[0:1, 0:1])
                rec = P2.tile([1, T], F32, tag="rec", name="rec")
                nc.vector.reciprocal(rec[:], asc[0:1, 1:T + 1])
                rhs_bc = P2.tile([1, 2 * T], F32, tag="rhs_bc", name="rhs_bc")
                nc.vector.scalar_tensor_tensor(
                    rhs_bc[0:1, 0:T], asc[0:1, 0:T], ALPHA, rec[:],
                    op0=OP.mult, op1=OP.mult)
                nc.vector.tensor_scalar_mul(rhs_bc[0:1, T:2 * T], rec[:], -VTH)
                ps_bc = PP.tile([128, 2 * T], F32, tag="psA", name="ps_bc")
                nc.tensor.matmul(ps_bc[:], ones_t[:], rhs_bc[:], start=True,
                                 stop=True)
                nc.scalar.activation(bc[s][:], ps_bc[:], AF.Copy)

            def scan_step(s, t, splits=1):
                f = s * T + t
                if t == 0:
                    nc.vector.memset(g_t[:], 0.0)
                yv = yview(ys[f % NY])[:, 1:33, 1:33]
                v = P2.tile([128, HW], F32, tag="v", name="v")
                spm = P2.tile([128, HW], F32, tag="spm", name="spm")
                so = P3.tile([128, HW], F32, tag="so", name="so")
                gv = g_t.rearrange("p (r c) -> p r c", c=W)
                vv = v.rearrange("p (r c) -> p r c", c=W)
                rows = H // splits
                for i in range(splits):
                    r0, r1 = i * rows, (i + 1) * rows
                    sl = slice(r0 * W, r1 * W)
                    nc.vector.scalar_tensor_tensor(
                        vv[:, r0:r1, :], gv[:, r0:r1, :], bc[s][:, t:t + 1],
                        yv[:, r0:r1, :], op0=OP.mult, op1=OP.add)
                    nc.scalar.activation(spm[:, sl], v[:, sl], AF.Sign,
                                         bias=bc[s][:, T + t:T + t + 1])
                    nc.vector.scalar_tensor_tensor(
                        g_t[:, sl], spm[:, sl], 0.0, v[:, sl],
                        op0=OP.is_lt, op1=OP.mult)
                    nc.scalar.activation(so[:, sl], spm[:, sl], AF.Relu)
                nc.sync.dma_start(
                    spk[s, t].rearrange("ch r c -> ch (r c)"), so[:])

            for t in range(T):
                conv_frame(0, t)
            attention(0)
            for t in range(T):
                scan_step(0, t)
                conv_frame(1, t)
            attention(1)
            for t in range(T):
                scan_step(1, t, splits=4)

    nc.compile()
    return nc


def _trunc13(a):
    # fp32r = round-to-nearest, 11 explicit mantissa bits (HW-verified via
    # DMA roundtrip). Split values must be 11-bit so the hardware re-round
    # is a no-op and x_hi + x_lo == x exactly.
    u = np.ascontiguousarray(a, np.float32).view(np.uint32)
    r = (u + np.uint32(0x800)) & np.uint32(0xFFFFF000)
    return r.view(np.float32)


def _pad_frames(x):
    """[.., 64, 32, 32] -> [.., 64, XCOL] host-padded flat frames."""
    lead = x.shape[:-2]
    out = np.zeros(lead + (XCOL,), np.float32)
    padded = np.zeros(lead + (PW, PW), np.float32)
    padded[..., 1:33, 1:33] = x
    out[..., 1:1 + FLAT] = padded.reshape(lead + (FLAT,))
    return out


def _prep_host_inputs(conv_w, conv_b, mlp_w1, mlp_w2):
    wT = np.ascontiguousarray(np.transpose(conv_w, (1, 0, 2, 3)))  # [64,128,3,3]
    blocks = [wT[:, :, dy + 1, dx + 1] for dy, dx in TAPS]
    w9 = np.concatenate(blocks, axis=1)                            # [64, 9*128]
    wtap = np.concatenate([w9, w9], axis=0).astype(np.float32)     # [128, 9*128]
    common = {
        "bias": np.ascontiguousarray(conv_b.reshape(128, 1), np.float32),
        "w1t": np.ascontiguousarray(mlp_w1.T).astype(np.float32),
        "w2t": np.ascontiguousarray(mlp_w2.T).astype(np.float32),
        "ident": np.eye(128, dtype=np.float32),
    }
    if CONV_MODE == "f32r3":
        w9_hi = _trunc13(w9)
        w9_lo = (w9 - w9_hi).astype(np.float32)
        common["wtap"] = np.concatenate([w9_hi, w9_hi], axis=0)
        common["wcorr"] = np.concatenate([w9_lo, w9_hi], axis=0)
    else:
        common["wtap"] = wtap
    return common


_CACHED = {}


def make_in_maps(data, conv_w, conv_b, mlp_w1, mlp_w2):
    data = np.ascontiguousarray(data, np.float32)
    common = _prep_host_inputs(np.asarray(conv_w, np.float32),
                               np.asarray(conv_b, np.float32),
                               np.asarray(mlp_w1, np.float32),
                               np.asarray(mlp_w2, np.float32))
    in_maps = []
    for c in range(N_CORES):
        m = dict(common)
        shard = _pad_frames(data[c * BPC:(c + 1) * BPC])
        if CONV_MODE == "f32r3":
            hi = _trunc13(shard)
            m["xhi"] = hi
            m["xlo"] = (shard - hi).astype(np.float32)
        else:
            m["xhi"] = shard
        in_maps.append(m)
    return in_maps


def kernel(data, conv_w, conv_b, mlp_w1, mlp_w2):
    if "prog" not in _CACHED:
        _CACHED["prog"] = _build_program()
    nc = _CACHED["prog"]
    in_maps = make_in_maps(data, conv_w, conv_b, mlp_w1, mlp_w2)
    res = run_bass_kernel_spmd(nc, in_maps, list(range(N_CORES)))
    out = np.concatenate([res.results[c]["spk"] for c in range(N_CORES)], axis=0)
    return out.reshape(B, T, CH, H, W)



# revision 5
# speedup vs baseline: 1.5907x; 1.5907x over previous
"""Trainium2 Bass kernel for nn_ConvAttLIF (conv3x3 + temporal attention + LIF scan).

Sharding: data-parallel over batch B=16 across 8 NeuronCores (2 samples/core).

Layout: frames host-packed with shared row halos (33-wide rows: the right
halo of row r is the left halo of row r+1, both zero), so a frame is 1124
contiguous cols and the conv output span is 1056 cols = 3 psum chunks of 352.

Conv: per chunk, 15 f32r matmuls accumulate one psum bank:
  - 3 "pair" units (K=128): taps (-1,dx) and (+1,dx) fused by storing a
    second frame copy shifted 2 rows (66 cols) in partitions 64-127.
  - 3 "single" units (K=64): taps (0,dx) on partitions 0-63.
  - 9 "corr" units (K=128): [x_hi; x_lo] . [w_lo; w_hi] per tap, restoring
    ~fp32 accuracy from the 12-bit f32r operands (x_hi = trunc13(x)).
Chunks are processed in rotating order (frame f starts at chunk f%3) so each
frame's first psum bank was drained one chunk-stream earlier - no PE stall.

LIF scan: attention folded in via v_t = u_t/att_t, so each step is
v = g*c_t + y (STT), g = (v < thr_t)*v (STT, same engine - no cross-engine
hop in the serial chain), spike = (v >= thr_t) off-chain. The sample-1 tail
(no conv left to overlap) splits rows across DVE/Pool/ACT.

kernel(**inputs) takes the FULL unsharded inputs, returns the FULL output.
"""
import sys

sys.path.insert(0, "/opt/trn_rl_repo")

import numpy as np
import concourse.bass as bass
import concourse.bacc as bacc
import concourse.tile as tile
import concourse.mybir as mybir
from concourse.bass_utils import run_bass_kernel_spmd

F32 = mybir.dt.float32
F32R = mybir.dt.float32r
AF = mybir.ActivationFunctionType
OP = mybir.AluOpType
AX = mybir.AxisListType

B, T, CIN, H, W = 16, 20, 64, 32, 32
CH = 128
N_CORES = 8
BPC = B // N_CORES
ALPHA, VTH = 0.3, 0.6
HW = H * W                     # 1024
PW = W + 1                     # 33: row stride (shared halo col)
XCOL = 34 * PW + 2             # 1124 packed frame cols (+2 guard)
MAR = 2 * PW                   # 66: left margin in XA for the shifted copy
CN = 352                       # psum chunk cols (3 x 352 = 1056 out span)
OUT0 = PW + 1                  # 34: first out position in frame coords
NY = 25                        # y-tile ring size
TAPS = [(dy, dx) for dy in (-1, 0, 1) for dx in (-1, 0, 1)]


def _build_program():
    nc = bacc.Bacc("TRN2", target_bir_lowering=False, debug=False,
                   num_devices=N_CORES)

    xhi_d = nc.dram_tensor("xhi", [BPC, T, CIN, XCOL], F32,
                           kind="ExternalInput").ap()
    xlo_d = nc.dram_tensor("xlo", [BPC, T, CIN, XCOL], F32,
                           kind="ExternalInput").ap()
    wpair_d = nc.dram_tensor("wpair", [128, 3 * 128], F32,
                             kind="ExternalInput").ap()
    wsing_d = nc.dram_tensor("wsing", [64, 3 * 128], F32,
                             kind="ExternalInput").ap()
    wcorr_d = nc.dram_tensor("wcorr", [128, 9 * 128], F32,
                             kind="ExternalInput").ap()
    bias_d = nc.dram_tensor("bias", [128, 1], F32, kind="ExternalInput").ap()
    w1t_d = nc.dram_tensor("w1t", [T, 5], F32, kind="ExternalInput").ap()
    w2t_d = nc.dram_tensor("w2t", [5, T], F32, kind="ExternalInput").ap()
    ident_d = nc.dram_tensor("ident", [128, 128], F32, kind="ExternalInput").ap()
    spk = nc.dram_tensor("spk", [BPC, T, CH, HW], F32, kind="ExternalOutput").ap()

    with tile.TileContext(nc) as tc:
        with tc.tile_pool(name="sb", bufs=1) as P1, \
             tc.tile_pool(name="scr", bufs=2) as P2, \
             tc.tile_pool(name="so", bufs=3) as P3, \
             tc.tile_pool(name="ps", bufs=1, space="PSUM") as PP:

            # ---- persistent tiles ----
            wpair = P1.tile([128, 3 * 128], F32R, tag="wpair", name="wpair")
            nc.sync.dma_start(wpair[:], wpair_d[:].bitcast(F32R))
            wsing = P1.tile([64, 3 * 128], F32R, tag="wsing", name="wsing")
            nc.sync.dma_start(wsing[:], wsing_d[:].bitcast(F32R))
            bias_t = P1.tile([128, 1], F32, tag="bias", name="bias")
            nc.sync.dma_start(bias_t[:], bias_d[:])

            xas = [P1.tile([128, MAR + XCOL], F32R, tag=f"xa{i}", name=f"xa{i}")
                   for i in range(3)]
            xcs = [P1.tile([128, XCOL], F32R, tag=f"xc{i}", name=f"xc{i}")
                   for i in range(3)]

            def x_dma(s, t):
                f = s * T + t
                xa, xc = xas[f % 3], xcs[f % 3]
                src = xhi_d[s, t].bitcast(F32R)
                nc.sync.dma_start(xa[0:64, MAR:MAR + XCOL], src)
                nc.sync.dma_start(xa[64:128, 0:XCOL], src)
                nc.sync.dma_start(xc[0:64, :], src)
                nc.sync.dma_start(xc[64:128, :], xlo_d[s, t].bitcast(F32R))

            # frame 0 input DMA before the remaining (weight) loads
            x_dma(0, 0)

            wcorr = P1.tile([128, 9 * 128], F32R, tag="wcorr", name="wcorr")
            nc.sync.dma_start(wcorr[:], wcorr_d[:].bitcast(F32R))
            w1t_s = P1.tile([T, 5], F32, tag="w1t", name="w1t")
            nc.sync.dma_start(w1t_s[:], w1t_d[:])
            w2t_s = P1.tile([5, T], F32, tag="w2t", name="w2t")
            nc.sync.dma_start(w2t_s[:], w2t_d[:])
            ident = P1.tile([128, 128], F32, tag="ident", name="ident")
            nc.sync.dma_start(ident[:], ident_d[:])
            ones_t = P1.tile([1, 128], F32, tag="ones", name="ones")
            nc.vector.memset(ones_t[:], 1.0)

            ys = [P1.tile([128, XCOL], F32, tag=f"y{i}", name=f"y{i}")
                  for i in range(NY)]
            gs = [P1.tile([128, HW], F32, tag=f"g{s}", name=f"g{s}")
                  for s in range(BPC)]
            # stats rows: 0-2 chunk sums, 3 -junk, 4 total, 5 max
            s_st = [P1.tile([128, 6 * T], F32, tag=f"S{s}", name=f"S{s}")
                    for s in range(BPC)]
            bc = [P1.tile([128, 3 * T], F32, tag=f"bc{s}", name=f"bc{s}")
                  for s in range(BPC)]

            engines = {"v": nc.vector, "p": nc.gpsimd}

            def conv_frame(s, t, skip_dma=False):
                f = s * T + t
                if not skip_dma:
                    x_dma(s, t)
                xa, xc = xas[f % 3], xcs[f % 3]
                y = ys[f % NY]
                for ci in range(3):
                    c = (f + ci) % 3
                    o = OUT0 + CN * c
                    ps = PP.tile([128, CN], F32, tag=f"p{c}", name=f"p{c}")
                    units = []
                    for i, dx in enumerate((-1, 0, 1)):
                        units.append((wpair[:, i * 128:(i + 1) * 128],
                                      xa[0:128, MAR + o - PW + dx:
                                         MAR + o - PW + dx + CN]))
                    for i, dx in enumerate((-1, 0, 1)):
                        units.append((wsing[:, i * 128:(i + 1) * 128],
                                      xa[0:64, MAR + o + dx:MAR + o + dx + CN]))
                    for j, (dy, dx) in enumerate(TAPS):
                        b0 = o + dy * PW + dx
                        units.append((wcorr[:, j * 128:(j + 1) * 128],
                                      xc[0:128, b0:b0 + CN]))
                    for k, (w_ap, x_ap) in enumerate(units):
                        nc.tensor.matmul(ps[:], w_ap, x_ap,
                                         start=(k == 0),
                                         stop=(k == len(units) - 1))
                    nc.scalar.activation(
                        y[:, o:o + CN], ps[:], AF.Identity,
                        bias=bias_t[:, 0:1],
                        accum_out=s_st[s][:, c * T + t:c * T + t + 1])
                # stats: -junk sum, max over real cols, total
                yj = y[:, MAR:MAR + 32 * PW].rearrange(
                    "p (r c) -> p r c", c=PW)
                nc.vector.reduce_sum(s_st[s][:, 3 * T + t:3 * T + t + 1],
                                     yj[:, :, 0:1], axis=AX.XY, negate=True)
                ym = y[:, OUT0:OUT0 + 32 * PW].rearrange(
                    "p (r c) -> p r c", c=PW)
                nc.vector.reduce_max(s_st[s][:, 5 * T + t:5 * T + t + 1],
                                     ym[:, :, 0:W], axis=AX.XY)
                sv = s_st[s].rearrange("p (k t) -> p k t", t=T)
                nc.vector.reduce_sum(sv[:, 4:5, t:t + 1], sv[:, 0:4, t:t + 1],
                                     axis=AX.XY)

            def attention(s):
                S = s_st[s]
                psT1 = PP.tile([T, 128], F32, tag="pa0", name="psT1")
                nc.tensor.transpose(psT1[:], S[:, 4 * T:5 * T], ident[:])
                psT2 = PP.tile([T, 128], F32, tag="pa1", name="psT2")
                nc.tensor.transpose(psT2[:], S[:, 5 * T:6 * T], ident[:])
                tmp = P2.tile([T, 1], F32, tag="att_tmp", name="att_tmp")
                nc.vector.reduce_sum(tmp[:], psT1[:], axis=AX.X)
                att_in = P2.tile([T, 2], F32, tag="att_in", name="att_in")
                nc.vector.tensor_scalar_mul(att_in[:, 0:1], tmp[:],
                                            1.0 / (CH * HW))
                nc.vector.reduce_max(att_in[:, 1:2], psT2[:], axis=AX.X)
                ps5 = PP.tile([5, 2], F32, tag="pa0", name="ps5")
                nc.tensor.matmul(ps5[:], w1t_s[:], att_in[:], start=True,
                                 stop=True)
                h5 = P2.tile([5, 2], F32, tag="h5", name="h5")
                nc.scalar.activation(h5[:], ps5[:], AF.Relu)
                ps20 = PP.tile([T, 2], F32, tag="pa1", name="ps20")
                nc.tensor.matmul(ps20[:], w2t_s[:], h5[:], start=True, stop=True)
                a20 = P2.tile([T, 2], F32, tag="a20", name="a20")
                nc.scalar.activation(a20[:], ps20[:], AF.Copy)
                attp = P2.tile([T, 1], F32, tag="attp", name="attp")
                nc.vector.tensor_tensor(attp[:], a20[:, 0:1], a20[:, 1:2],
                                        op=OP.add)
                # sigmoid via exp + reciprocal (tighter than the Sigmoid table)
                expz = P2.tile([T, 1], F32, tag="expz", name="expz")
                nc.scalar.activation(expz[:], attp[:], AF.Exp, scale=-1.0)
                att1 = P2.tile([T, 1], F32, tag="att1", name="att1")
                nc.vector.tensor_scalar_add(att1[:], expz[:], 1.0)
                att = P2.tile([T, 1], F32, tag="att", name="att")
                nc.vector.reciprocal(att[:], att1[:])
                psT3 = PP.tile([1, T], F32, tag="pa0", name="psT3")
                nc.tensor.transpose(psT3[:], att[:, 0:1], ident[0:T, 0:T])
                atts = P2.tile([1, T + 1], F32, tag="atts", name="atts")
                nc.scalar.activation(atts[0:1, 1:T + 1], psT3[:], AF.Copy)
                nc.scalar.activation(atts[0:1, 0:1], psT3[0:1, 0:1], AF.Copy)
                rec = P2.tile([1, T], F32, tag="rec", name="rec")
                nc.vector.reciprocal(rec[:], atts[0:1, 1:T + 1])
                rhs3 = P2.tile([1, 3 * T], F32, tag="rhs3", name="rhs3")
                nc.vector.scalar_tensor_tensor(
                    rhs3[0:1, 0:T], atts[0:1, 0:T], ALPHA, rec[:],
                    op0=OP.mult, op1=OP.mult)
                nc.vector.tensor_scalar_mul(rhs3[0:1, T:2 * T], rec[:], VTH)
                nc.vector.tensor_scalar_mul(rhs3[0:1, 2 * T:3 * T], rec[:],
                                            -VTH)
                ps_bc = PP.tile([128, 3 * T], F32, tag="pa1", name="ps_bc")
                nc.tensor.matmul(ps_bc[:], ones_t[:], rhs3[:], start=True,
                                 stop=True)
                nc.scalar.activation(bc[s][:], ps_bc[:], AF.Copy)

            def scan_step(s, t, vg, sp):
                f = s * T + t
                g = gs[s]
                if t == 0:
                    nc.vector.memset(g[:], 0.0)
                y = ys[f % NY]
                yv = y[:, OUT0:OUT0 + 32 * PW].rearrange(
                    "p (r c) -> p r c", c=PW)
                v = P2.tile([128, HW], F32, tag="v", name="v")
                spm = P2.tile([128, HW], F32, tag="spm", name="spm")
                m = P2.tile([128, HW], F32, tag="m", name="m")
                so = P3.tile([128, HW], F32, tag="so", name="so")
                vv = v.rearrange("p (r c) -> p r c", c=W)
                gv = g.rearrange("p (r c) -> p r c", c=W)
                cb = bc[s][:, t:t + 1]
                tn = min(t + 1, T - 1)
                cbn = bc[s][:, tn:tn + 1]
                thr = bc[s][:, T + t:T + t + 1]
                nthr = bc[s][:, 2 * T + t:2 * T + t + 1]
                for eng, r0, r1 in vg:
                    R = slice(r0 // W, r1 // W)
                    if eng == "v":
                        nc.vector.scalar_tensor_tensor(
                            vv[:, R, :], gv[:, R, :], cb, yv[:, R, 0:W],
                            op0=OP.mult, op1=OP.add)
                        nc.vector.scalar_tensor_tensor(
                            g[:, r0:r1], v[:, r0:r1], thr, v[:, r0:r1],
                            op0=OP.is_lt, op1=OP.mult)
                    else:
                        # Pool rows keep g pre-multiplied by c_{t+1}:
                        # v = g + y; m = (v<thr)*c_next; g = m*v
                        nc.gpsimd.tensor_tensor(
                            vv[:, R, :], gv[:, R, :], yv[:, R, 0:W],
                            op=OP.add)
                        nc.gpsimd.tensor_scalar(
                            m[:, r0:r1], v[:, r0:r1], thr, cbn,
                            op0=OP.is_lt, op1=OP.mult)
                        nc.gpsimd.tensor_tensor(
                            g[:, r0:r1], m[:, r0:r1], v[:, r0:r1],
                            op=OP.mult)
                for eng, r0, r1 in sp:
                    if eng == "a":
                        nc.scalar.activation(spm[:, r0:r1], v[:, r0:r1],
                                             AF.Sign, bias=nthr)
                        nc.scalar.activation(so[:, r0:r1], spm[:, r0:r1],
                                             AF.Relu)
                    else:
                        nc.vector.tensor_scalar(
                            so[:, r0:r1], v[:, r0:r1], thr, None,
                            op0=OP.is_ge)
                nc.sync.dma_start(spk[s, t], so[:])

            OVERLAP_VG = [("v", 0, HW)]
            OVERLAP_SP = [("a", 0, HW)]
            TAIL_VG = [("v", 0, 672), ("p", 672, HW)]
            TAIL_SP = [("a", 0, 864), ("v", 864, HW)]

            conv_frame(0, 0, skip_dma=True)
            for t in range(1, T):
                conv_frame(0, t)
            conv_frame(1, 0)
            conv_frame(1, 1)
            attention(0)
            for t in range(T - 2):
                scan_step(0, t, OVERLAP_VG, OVERLAP_SP)
                conv_frame(1, t + 2)
            attention(1)
            scan_step(0, T - 2, OVERLAP_VG, OVERLAP_SP)
            scan_step(0, T - 1, OVERLAP_VG, OVERLAP_SP)
            for t in range(T):
                scan_step(1, t, TAIL_VG, TAIL_SP)

    nc.compile()
    return nc


def _trunc13(a):
    # f32r = round-to-nearest, 11 explicit mantissa bits (HW-verified via
    # DMA roundtrip). Split values must be 11-bit so the hardware re-round
    # is a no-op and x_hi + x_lo == x exactly.
    u = np.ascontiguousarray(a, np.float32).view(np.uint32)
    r = (u + np.uint32(0x800)) & np.uint32(0xFFFFF000)
    return r.view(np.float32)


def _pad_frames(x):
    """[.., 64, 32, 32] -> [.., 64, XCOL] host-packed shared-halo frames."""
    lead = x.shape[:-2]
    padded = np.zeros(lead + (34, PW), np.float32)
    padded[..., 1:33, 1:33] = x
    out = np.zeros(lead + (XCOL,), np.float32)
    out[..., :34 * PW] = padded.reshape(lead + (34 * PW,))
    return out


def _prep_host_inputs(conv_w, conv_b, mlp_w1, mlp_w2):
    wT = np.ascontiguousarray(np.transpose(conv_w, (1, 0, 2, 3)))  # [64,128,3,3]
    hi = {}
    lo = {}
    for dy, dx in TAPS:
        blk = np.ascontiguousarray(wT[:, :, dy + 1, dx + 1])
        h = _trunc13(blk)
        hi[(dy, dx)] = h
        lo[(dy, dx)] = (blk - h).astype(np.float32)
    wpair = np.concatenate(
        [np.concatenate([hi[(-1, dx)], hi[(1, dx)]], axis=0)
         for dx in (-1, 0, 1)], axis=1)                            # [128, 384]
    wsing = np.concatenate([hi[(0, dx)] for dx in (-1, 0, 1)], axis=1)
    wcorr = np.concatenate(
        [np.concatenate([lo[tap], hi[tap]], axis=0) for tap in TAPS],
        axis=1)                                                    # [128, 1152]
    return {
        "wpair": np.ascontiguousarray(wpair, np.float32),
        "wsing": np.ascontiguousarray(wsing, np.float32),
        "wcorr": np.ascontiguousarray(wcorr, np.float32),
        "bias": np.ascontiguousarray(conv_b.reshape(128, 1), np.float32),
        "w1t": np.ascontiguousarray(mlp_w1.T).astype(np.float32),
        "w2t": np.ascontiguousarray(mlp_w2.T).astype(np.float32),
        "ident": np.eye(128, dtype=np.float32),
    }


_CACHED = {}


def make_in_maps(data, conv_w, conv_b, mlp_w1, mlp_w2):
    data = np.ascontiguousarray(data, np.float32)
    common = _prep_host_inputs(np.asarray(conv_w, np.float32),
                               np.asarray(conv_b, np.float32),
                               np.asarray(mlp_w1, np.float32),
                               np.asarray(mlp_w2, np.float32))
    in_maps = []
    for c in range(N_CORES):
        m = dict(common)
        shard = _pad_frames(data[c * BPC:(c + 1) * BPC])
        h = _trunc13(shard)
        m["xhi"] = h
        m["xlo"] = (shard - h).astype(np.float32)
        in_maps.append(m)
    return in_maps


def kernel(data, conv_w, conv_b, mlp_w1, mlp_w2):
    if "prog" not in _CACHED:
        _CACHED["prog"] = _build_program()
    nc = _CACHED["prog"]
    in_maps = make_in_maps(data, conv_w, conv_b, mlp_w1, mlp_w2)
    res = run_bass_kernel_spmd(nc, in_maps, list(range(N_CORES)))
    out = np.concatenate([res.results[c]["spk"] for c in range(N_CORES)], axis=0)
    return out.reshape(B, T, CH, H, W)


# revision 8
# speedup vs baseline: 1.6146x; 1.0150x over previous
"""Trainium2 Bass kernel for nn_ConvAttLIF (conv3x3 + temporal attention + LIF scan).

Sharding: data-parallel over batch B=16 across 8 NeuronCores (2 samples/core).

Layout: frames host-packed with shared row halos (33-wide rows: the right
halo of row r is the left halo of row r+1, both zero), so a frame is 1124
contiguous cols and the conv output span is 1056 cols = 3 psum chunks of 352.

Conv: per chunk, 15 f32r matmuls accumulate one psum bank:
  - 3 "pair" units (K=128): taps (-1,dx) and (+1,dx) fused by storing a
    second frame copy shifted 2 rows (66 cols) in partitions 64-127.
  - 3 "single" units (K=64): taps (0,dx) on partitions 0-63.
  - 9 "corr" units (K=128): [x_hi; x_lo] . [w_lo; w_hi] per tap, restoring
    ~fp32 accuracy from the 12-bit f32r operands (x_hi = trunc13(x)).
Chunks are processed in rotating order (frame f starts at chunk f%3) so each
frame's first psum bank was drained one chunk-stream earlier - no PE stall.

LIF scan: attention folded in via v_t = u_t/att_t, so each step is
v = g*c_t + y (STT), g = (v < thr_t)*v (STT, same engine - no cross-engine
hop in the serial chain), spike = (v >= thr_t) off-chain. The sample-1 tail
(no conv left to overlap) splits rows across DVE/Pool/ACT.

kernel(**inputs) takes the FULL unsharded inputs, returns the FULL output.
"""
import sys

sys.path.insert(0, "/opt/trn_rl_repo")

import numpy as np
import concourse.bass as bass
import concourse.bacc as bacc
import concourse.tile as tile
import concourse.mybir as mybir
from concourse.bass_utils import run_bass_kernel_spmd

F32 = mybir.dt.float32
F32R = mybir.dt.float32r
AF = mybir.ActivationFunctionType
OP = mybir.AluOpType
AX = mybir.AxisListType

B, T, CIN, H, W = 16, 20, 64, 32, 32
CH = 128
N_CORES = 8
BPC = B // N_CORES
ALPHA, VTH = 0.3, 0.6
HW = H * W                     # 1024
PW = W + 1                     # 33: row stride (shared halo col)
XCOL = 34 * PW + 2             # 1124 packed frame cols (+2 guard)
MAR = 2 * PW                   # 66: left margin in XA for the shifted copy
CN = 352                       # psum chunk cols (3 x 352 = 1056 out span)
OUT0 = PW + 1                  # 34: first out position in frame coords
NY = 25                        # y-tile ring size
TAPS = [(dy, dx) for dy in (-1, 0, 1) for dx in (-1, 0, 1)]


def _build_program():
    nc = bacc.Bacc("TRN2", target_bir_lowering=False, debug=False,
                   num_devices=N_CORES)

    xhi_d = nc.dram_tensor("xhi", [BPC, T, CIN, XCOL], F32,
                           kind="ExternalInput").ap()
    xlo_d = nc.dram_tensor("xlo", [BPC, T, CIN, XCOL], F32,
                           kind="ExternalInput").ap()
    wpair_d = nc.dram_tensor("wpair", [128, 3 * 128], F32,
                             kind="ExternalInput").ap()
    wsing_d = nc.dram_tensor("wsing", [64, 3 * 128], F32,
                             kind="ExternalInput").ap()
    wcorr_d = nc.dram_tensor("wcorr", [128, 9 * 128], F32,
                             kind="ExternalInput").ap()
    bias_d = nc.dram_tensor("bias", [128, 1], F32, kind="ExternalInput").ap()
    w1t_d = nc.dram_tensor("w1t", [T, 5], F32, kind="ExternalInput").ap()
    w2t_d = nc.dram_tensor("w2t", [5, T], F32, kind="ExternalInput").ap()
    ident_d = nc.dram_tensor("ident", [128, 128], F32, kind="ExternalInput").ap()
    spk = nc.dram_tensor("spk", [BPC, T, CH, HW], F32, kind="ExternalOutput").ap()

    with tile.TileContext(nc) as tc:
        with tc.tile_pool(name="sb", bufs=1) as P1, \
             tc.tile_pool(name="scr", bufs=2) as P2, \
             tc.tile_pool(name="so", bufs=3) as P3, \
             tc.tile_pool(name="ps", bufs=1, space="PSUM") as PP:

            # ---- persistent tiles ----
            xas = [P1.tile([128, MAR + XCOL], F32R, tag=f"xa{i}", name=f"xa{i}")
                   for i in range(3)]
            xcs = [P1.tile([128, XCOL], F32R, tag=f"xc{i}", name=f"xc{i}")
                   for i in range(3)]

            def x_dma(s, t):
                f = s * T + t
                xa, xc = xas[f % 3], xcs[f % 3]
                src = xhi_d[s, t].bitcast(F32R)
                nc.sync.dma_start(xa[0:64, MAR:MAR + XCOL], src)
                nc.sync.dma_start(xa[64:128, 0:XCOL], src)
                nc.sync.dma_start(xc[0:64, :], src)
                nc.sync.dma_start(xc[64:128, :], xlo_d[s, t].bitcast(F32R))

            # frame 0 input DMA before everything (first matmuls need it)
            x_dma(0, 0)
            wpair = P1.tile([128, 3 * 128], F32R, tag="wpair", name="wpair")
            nc.sync.dma_start(wpair[:], wpair_d[:].bitcast(F32R))
            wsing = P1.tile([64, 3 * 128], F32R, tag="wsing", name="wsing")
            nc.sync.dma_start(wsing[:], wsing_d[:].bitcast(F32R))
            bias_t = P1.tile([128, 1], F32, tag="bias", name="bias")
            nc.sync.dma_start(bias_t[:], bias_d[:])

            wcorr = P1.tile([128, 9 * 128], F32R, tag="wcorr", name="wcorr")
            nc.sync.dma_start(wcorr[:], wcorr_d[:].bitcast(F32R))
            w1t_s = P1.tile([T, 5], F32, tag="w1t", name="w1t")
            nc.sync.dma_start(w1t_s[:], w1t_d[:])
            w2t_s = P1.tile([5, T], F32, tag="w2t", name="w2t")
            nc.sync.dma_start(w2t_s[:], w2t_d[:])
            ident = P1.tile([128, 128], F32, tag="ident", name="ident")
            nc.sync.dma_start(ident[:], ident_d[:])
            ones_t = P1.tile([1, 128], F32, tag="ones", name="ones")
            nc.vector.memset(ones_t[:], 1.0)

            ys = [P1.tile([128, XCOL], F32, tag=f"y{i}", name=f"y{i}")
                  for i in range(NY)]
            gs = [P1.tile([128, HW], F32, tag=f"g{s}", name=f"g{s}")
                  for s in range(BPC)]
            # stats rows: 0-2 chunk sums, 3 -junk, 4 total, 5 max
            s_st = [P1.tile([128, 6 * T], F32, tag=f"S{s}", name=f"S{s}")
                    for s in range(BPC)]
            bc = [P1.tile([128, 3 * T], F32, tag=f"bc{s}", name=f"bc{s}")
                  for s in range(BPC)]

            engines = {"v": nc.vector, "p": nc.gpsimd}

            def conv_frame(s, t, skip_dma=False):
                f = s * T + t
                if not skip_dma:
                    x_dma(s, t)
                xa, xc = xas[f % 3], xcs[f % 3]
                y = ys[f % NY]
                for ci in range(3):
                    c = (f + ci) % 3
                    o = OUT0 + CN * c
                    ps = PP.tile([128, CN], F32, tag=f"p{c}", name=f"p{c}")
                    units = []
                    for i, dx in enumerate((-1, 0, 1)):
                        units.append((wpair[:, i * 128:(i + 1) * 128],
                                      xa[0:128, MAR + o - PW + dx:
                                         MAR + o - PW + dx + CN]))
                    for i, dx in enumerate((-1, 0, 1)):
                        units.append((wsing[:, i * 128:(i + 1) * 128],
                                      xa[0:64, MAR + o + dx:MAR + o + dx + CN]))
                    for j, (dy, dx) in enumerate(TAPS):
                        b0 = o + dy * PW + dx
                        units.append((wcorr[:, j * 128:(j + 1) * 128],
                                      xc[0:128, b0:b0 + CN]))
                    for k, (w_ap, x_ap) in enumerate(units):
                        nc.tensor.matmul(ps[:], w_ap, x_ap,
                                         start=(k == 0),
                                         stop=(k == len(units) - 1))
                    nc.scalar.activation(
                        y[:, o:o + CN], ps[:], AF.Identity,
                        bias=bias_t[:, 0:1],
                        accum_out=s_st[s][:, c * T + t:c * T + t + 1])
                # stats: -junk sum, max over real cols, total
                yj = y[:, MAR:MAR + 32 * PW].rearrange(
                    "p (r c) -> p r c", c=PW)
                nc.vector.reduce_sum(s_st[s][:, 3 * T + t:3 * T + t + 1],
                                     yj[:, :, 0:1], axis=AX.XY, negate=True)
                ym = y[:, OUT0:OUT0 + 32 * PW].rearrange(
                    "p (r c) -> p r c", c=PW)
                nc.vector.reduce_max(s_st[s][:, 5 * T + t:5 * T + t + 1],
                                     ym[:, :, 0:W], axis=AX.XY)
                sv = s_st[s].rearrange("p (k t) -> p k t", t=T)
                nc.vector.reduce_sum(sv[:, 4:5, t:t + 1], sv[:, 0:4, t:t + 1],
                                     axis=AX.XY)

            def attention(s):
                S = s_st[s]
                psT1 = PP.tile([T, 128], F32, tag="pa0", name="psT1")
                nc.tensor.transpose(psT1[:], S[:, 4 * T:5 * T], ident[:])
                psT2 = PP.tile([T, 128], F32, tag="pa1", name="psT2")
                nc.tensor.transpose(psT2[:], S[:, 5 * T:6 * T], ident[:])
                tmp = P2.tile([T, 1], F32, tag="att_tmp", name="att_tmp")
                nc.vector.reduce_sum(tmp[:], psT1[:], axis=AX.X)
                att_in = P2.tile([T, 2], F32, tag="att_in", name="att_in")
                nc.vector.tensor_scalar_mul(att_in[:, 0:1], tmp[:],
                                            1.0 / (CH * HW))
                nc.vector.reduce_max(att_in[:, 1:2], psT2[:], axis=AX.X)
                ps5 = PP.tile([5, 2], F32, tag="pa0", name="ps5")
                nc.tensor.matmul(ps5[:], w1t_s[:], att_in[:], start=True,
                                 stop=True)
                h5 = P2.tile([5, 2], F32, tag="h5", name="h5")
                nc.scalar.activation(h5[:], ps5[:], AF.Relu)
                ps20 = PP.tile([T, 2], F32, tag="pa1", name="ps20")
                nc.tensor.matmul(ps20[:], w2t_s[:], h5[:], start=True, stop=True)
                a20 = P2.tile([T, 2], F32, tag="a20", name="a20")
                nc.scalar.activation(a20[:], ps20[:], AF.Copy)
                attp = P2.tile([T, 1], F32, tag="attp", name="attp")
                nc.vector.tensor_tensor(attp[:], a20[:, 0:1], a20[:, 1:2],
                                        op=OP.add)
                # sigmoid via exp + reciprocal (tighter than the Sigmoid table)
                expz = P2.tile([T, 1], F32, tag="expz", name="expz")
                nc.scalar.activation(expz[:], attp[:], AF.Exp, scale=-1.0)
                att1 = P2.tile([T, 1], F32, tag="att1", name="att1")
                nc.vector.tensor_scalar_add(att1[:], expz[:], 1.0)
                att = P2.tile([T, 1], F32, tag="att", name="att")
                nc.vector.reciprocal(att[:], att1[:])
                psT3 = PP.tile([1, T], F32, tag="pa0", name="psT3")
                nc.tensor.transpose(psT3[:], att[:, 0:1], ident[0:T, 0:T])
                atts = P2.tile([1, T + 1], F32, tag="atts", name="atts")
                nc.scalar.activation(atts[0:1, 1:T + 1], psT3[:], AF.Copy)
                nc.scalar.activation(atts[0:1, 0:1], psT3[0:1, 0:1], AF.Copy)
                rec = P2.tile([1, T], F32, tag="rec", name="rec")
                nc.vector.reciprocal(rec[:], atts[0:1, 1:T + 1])
                rhs3 = P2.tile([1, 3 * T], F32, tag="rhs3", name="rhs3")
                nc.vector.scalar_tensor_tensor(
                    rhs3[0:1, 0:T], atts[0:1, 0:T], ALPHA, rec[:],
                    op0=OP.mult, op1=OP.mult)
                nc.vector.tensor_scalar_mul(rhs3[0:1, T:2 * T], rec[:], VTH)
                nc.vector.tensor_scalar_mul(rhs3[0:1, 2 * T:3 * T], rec[:],
                                            -VTH)
                ps_bc = PP.tile([128, 3 * T], F32, tag="pa1", name="ps_bc")
                nc.tensor.matmul(ps_bc[:], ones_t[:], rhs3[:], start=True,
                                 stop=True)
                nc.scalar.activation(bc[s][:], ps_bc[:], AF.Copy)

            def scan_step(s, t, vg, sp):
                f = s * T + t
                g = gs[s]
                if t == 0:
                    nc.vector.memset(g[:], 0.0)
                y = ys[f % NY]
                yv = y[:, OUT0:OUT0 + 32 * PW].rearrange(
                    "p (r c) -> p r c", c=PW)
                v = P2.tile([128, HW], F32, tag="v", name="v")
                spm = P2.tile([128, HW], F32, tag="spm", name="spm")
                m = P2.tile([128, HW], F32, tag="m", name="m")
                so = P3.tile([128, HW], F32, tag="so", name="so")
                vv = v.rearrange("p (r c) -> p r c", c=W)
                gv = g.rearrange("p (r c) -> p r c", c=W)
                cb = bc[s][:, t:t + 1]
                tn = min(t + 1, T - 1)
                cbn = bc[s][:, tn:tn + 1]
                thr = bc[s][:, T + t:T + t + 1]
                nthr = bc[s][:, 2 * T + t:2 * T + t + 1]
                for eng, r0, r1 in vg:
                    R = slice(r0 // W, r1 // W)
                    if eng == "v":
                        nc.vector.scalar_tensor_tensor(
                            vv[:, R, :], gv[:, R, :], cb, yv[:, R, 0:W],
                            op0=OP.mult, op1=OP.add)
                        nc.vector.scalar_tensor_tensor(
                            g[:, r0:r1], v[:, r0:r1], thr, v[:, r0:r1],
                            op0=OP.is_lt, op1=OP.mult)
                    else:
                        # Pool rows keep g pre-multiplied by c_{t+1}:
                        # v = g + y; m = (v<thr)*c_next; g = m*v
                        nc.gpsimd.tensor_tensor(
                            vv[:, R, :], gv[:, R, :], yv[:, R, 0:W],
                            op=OP.add)
                        nc.gpsimd.tensor_scalar(
                            m[:, r0:r1], v[:, r0:r1], thr, cbn,
                            op0=OP.is_lt, op1=OP.mult)
                        nc.gpsimd.tensor_tensor(
                            g[:, r0:r1], m[:, r0:r1], v[:, r0:r1],
                            op=OP.mult)
                for eng, r0, r1 in sp:
                    if eng == "a":
                        nc.scalar.activation(spm[:, r0:r1], v[:, r0:r1],
                                             AF.Sign, bias=nthr)
                        nc.scalar.activation(so[:, r0:r1], spm[:, r0:r1],
                                             AF.Relu)
                    elif eng == "pm":
                        # spike from m (= (v<thr)*c_next): exactly 0 iff spike
                        nc.gpsimd.tensor_scalar(
                            so[:, r0:r1], m[:, r0:r1], 0.0, None,
                            op0=OP.is_equal)
                    else:
                        nc.vector.tensor_scalar(
                            so[:, r0:r1], v[:, r0:r1], thr, None,
                            op0=OP.is_ge)
                nc.sync.dma_start(spk[s, t], so[:])

            OVERLAP_VG = [("v", 0, HW)]
            OVERLAP_SP = [("a", 0, HW)]
            TAIL_VG = [("v", 0, 800), ("p", 800, HW)]
            TAIL_SP = [("a", 0, 864), ("pm", 864, HW)]

            conv_frame(0, 0, skip_dma=True)
            for t in range(1, T):
                conv_frame(0, t)
            conv_frame(1, 0)
            conv_frame(1, 1)
            attention(0)
            for t in range(T - 2):
                # input DMAs first: the spk DMA inside scan_step waits on
                # the scan result and would block them on the SP queue
                x_dma(1, t + 2)
                scan_step(0, t, OVERLAP_VG, OVERLAP_SP)
                conv_frame(1, t + 2, skip_dma=True)
            attention(1)
            scan_step(0, T - 2, OVERLAP_VG, OVERLAP_SP)
            scan_step(0, T - 1, OVERLAP_VG, OVERLAP_SP)
            for t in range(T):
                scan_step(1, t, TAIL_VG, TAIL_SP)

    nc.compile()
    return nc


def _trunc13(a):
    # f32r = round-to-nearest, 11 explicit mantissa bits (HW-verified via
    # DMA roundtrip). Split values must be 11-bit so the hardware re-round
    # is a no-op and x_hi + x_lo == x exactly.
    u = np.ascontiguousarray(a, np.float32).view(np.uint32)
    r = (u + np.uint32(0x800)) & np.uint32(0xFFFFF000)
    return r.view(np.float32)


def _pad_frames(x):
    """[.., 64, 32, 32] -> [.., 64, XCOL] host-packed shared-halo frames."""
    lead = x.shape[:-2]
    padded = np.zeros(lead + (34, PW), np.float32)
    padded[..., 1:33, 1:33] = x
    out = np.zeros(lead + (XCOL,), np.float32)
    out[..., :34 * PW] = padded.reshape(lead + (34 * PW,))
    return out


def _prep_host_inputs(conv_w, conv_b, mlp_w1, mlp_w2):
    wT = np.ascontiguousarray(np.transpose(conv_w, (1, 0, 2, 3)))  # [64,128,3,3]
    hi = {}
    lo = {}
    for dy, dx in TAPS:
        blk = np.ascontiguousarray(wT[:, :, dy + 1, dx + 1])
        h = _trunc13(blk)
        hi[(dy, dx)] = h
        lo[(dy, dx)] = (blk - h).astype(np.float32)
    wpair = np.concatenate(
        [np.concatenate([hi[(-1, dx)], hi[(1, dx)]], axis=0)
         for dx in (-1, 0, 1)], axis=1)                            # [128, 384]
    wsing = np.concatenate([hi[(0, dx)] for dx in (-1, 0, 1)], axis=1)
    wcorr = np.concatenate(
        [np.concatenate([lo[tap], hi[tap]], axis=0) for tap in TAPS],
        axis=1)                                                    # [128, 1152]
    return {
        "wpair": np.ascontiguousarray(wpair, np.float32),
        "wsing": np.ascontiguousarray(wsing, np.float32),
        "wcorr": np.ascontiguousarray(wcorr, np.float32),
        "bias": np.ascontiguousarray(conv_b.reshape(128, 1), np.float32),
        "w1t": np.ascontiguousarray(mlp_w1.T).astype(np.float32),
        "w2t": np.ascontiguousarray(mlp_w2.T).astype(np.float32),
        "ident": np.eye(128, dtype=np.float32),
    }


_CACHED = {}


def make_in_maps(data, conv_w, conv_b, mlp_w1, mlp_w2):
    data = np.ascontiguousarray(data, np.float32)
    common = _prep_host_inputs(np.asarray(conv_w, np.float32),
                               np.asarray(conv_b, np.float32),
                               np.asarray(mlp_w1, np.float32),
                               np.asarray(mlp_w2, np.float32))
    in_maps = []
    for c in range(N_CORES):
        m = dict(common)
        shard = _pad_frames(data[c * BPC:(c + 1) * BPC])
        h = _trunc13(shard)
        m["xhi"] = h
        m["xlo"] = (shard - h).astype(np.float32)
        in_maps.append(m)
    return in_maps


def kernel(data, conv_w, conv_b, mlp_w1, mlp_w2):
    if "prog" not in _CACHED:
        _CACHED["prog"] = _build_program()
    nc = _CACHED["prog"]
    in_maps = make_in_maps(data, conv_w, conv_b, mlp_w1, mlp_w2)
    res = run_bass_kernel_spmd(nc, in_maps, list(range(N_CORES)))
    out = np.concatenate([res.results[c]["spk"] for c in range(N_CORES)], axis=0)
    return out.reshape(B, T, CH, H, W)


# revision 10
# speedup vs baseline: 1.7156x; 1.0625x over previous
"""Trainium2 Bass kernel for nn_ConvAttLIF (conv3x3 + temporal attention + LIF scan).

Sharding: data-parallel over batch B=16 across 8 NeuronCores (2 samples/core).

Layout: frames host-packed with shared row halos (33-wide rows: the right
halo of row r is the left halo of row r+1, both zero), so a frame is 1124
contiguous cols and the conv output span is 1056 cols = 3 psum chunks of 352.

Conv: per chunk, 15 f32r matmuls accumulate one psum bank:
  - 3 "pair" units (K=128): taps (-1,dx) and (+1,dx) fused by storing a
    second frame copy shifted 2 rows (66 cols) in partitions 64-127.
  - 3 "single" units (K=64): taps (0,dx) on partitions 0-63.
  - 9 "corr" units (K=128): [x_hi; x_lo] . [w_lo; w_hi] per tap, restoring
    ~fp32 accuracy from the 12-bit f32r operands (x_hi = trunc13(x)).
Chunks are processed in rotating order (frame f starts at chunk f%3) so each
frame's first psum bank was drained one chunk-stream earlier - no PE stall.

LIF scan: attention folded in via v_t = u_t/att_t, so each step is
v = g*c_t + y (STT), g = (v < thr_t)*v (STT, same engine - no cross-engine
hop in the serial chain), spike = (v >= thr_t) off-chain. The sample-1 tail
(no conv left to overlap) splits rows across DVE/Pool/ACT.

kernel(**inputs) takes the FULL unsharded inputs, returns the FULL output.
"""
import sys

sys.path.insert(0, "/opt/trn_rl_repo")

import numpy as np
import concourse.bass as bass
import concourse.bacc as bacc
import concourse.tile as tile
import concourse.mybir as mybir
from concourse.bass_utils import run_bass_kernel_spmd

F32 = mybir.dt.float32
F32R = mybir.dt.float32r
AF = mybir.ActivationFunctionType
OP = mybir.AluOpType
AX = mybir.AxisListType

B, T, CIN, H, W = 16, 20, 64, 32, 32
CH = 128
N_CORES = 8
BPC = B // N_CORES
ALPHA, VTH = 0.3, 0.6
HW = H * W                     # 1024
PW = W + 1                     # 33: row stride (shared halo col)
XCOL = 34 * PW + 2             # 1124 packed frame cols (+2 guard)
MAR = 2 * PW                   # 66: left margin in XA for the shifted copy
CN = 352                       # psum chunk cols (3 x 352 = 1056 out span)
OUT0 = PW + 1                  # 34: first out position in frame coords
NY = 24                        # y-tile ring size
TAPS = [(dy, dx) for dy in (-1, 0, 1) for dx in (-1, 0, 1)]


def _build_program():
    nc = bacc.Bacc("TRN2", target_bir_lowering=False, debug=False,
                   num_devices=N_CORES)

    xhi_d = nc.dram_tensor("xhi", [BPC, T, CIN, XCOL], F32,
                           kind="ExternalInput").ap()
    xlo_d = nc.dram_tensor("xlo", [BPC, T, CIN, XCOL], F32,
                           kind="ExternalInput").ap()
    wpair_d = nc.dram_tensor("wpair", [128, 3 * 128], F32,
                             kind="ExternalInput").ap()
    wsing_d = nc.dram_tensor("wsing", [64, 3 * 128], F32,
                             kind="ExternalInput").ap()
    wcorr_d = nc.dram_tensor("wcorr", [128, 9 * 128], F32,
                             kind="ExternalInput").ap()
    bias_d = nc.dram_tensor("bias", [128, 1], F32, kind="ExternalInput").ap()
    w1t_d = nc.dram_tensor("w1t", [T, 5], F32, kind="ExternalInput").ap()
    w2t_d = nc.dram_tensor("w2t", [5, T], F32, kind="ExternalInput").ap()
    ident_d = nc.dram_tensor("ident", [128, 128], F32, kind="ExternalInput").ap()
    spk = nc.dram_tensor("spk", [BPC, T, CH, HW], F32, kind="ExternalOutput").ap()

    with tile.TileContext(nc) as tc:
        with tc.tile_pool(name="sb", bufs=1) as P1, \
             tc.tile_pool(name="scr", bufs=2) as P2, \
             tc.tile_pool(name="so", bufs=3) as P3, \
             tc.tile_pool(name="ps", bufs=1, space="PSUM") as PP:

            # ---- persistent tiles ----
            xas = [P1.tile([128, MAR + XCOL], F32R, tag=f"xa{i}", name=f"xa{i}")
                   for i in range(4)]
            xcs = [P1.tile([128, XCOL], F32R, tag=f"xc{i}", name=f"xc{i}")
                   for i in range(4)]

            def x_dma(s, t):
                f = s * T + t
                xa, xc = xas[f % 4], xcs[f % 4]
                src = xhi_d[s, t].bitcast(F32R)
                nc.sync.dma_start(xa[0:64, MAR:MAR + XCOL], src)
                nc.sync.dma_start(xa[64:128, 0:XCOL], src)
                nc.sync.dma_start(xc[0:64, :], src)
                nc.sync.dma_start(xc[64:128, :], xlo_d[s, t].bitcast(F32R))

            # startup order: frame-0 XA halves, pair/single weights (first
            # units of the first chunk), then the corr inputs
            f0src = xhi_d[0, 0].bitcast(F32R)
            nc.sync.dma_start(xas[0][0:64, MAR:MAR + XCOL], f0src)
            nc.sync.dma_start(xas[0][64:128, 0:XCOL], f0src)
            wpair = P1.tile([128, 3 * 128], F32R, tag="wpair", name="wpair")
            nc.sync.dma_start(wpair[:], wpair_d[:].bitcast(F32R))
            wsing = P1.tile([64, 3 * 128], F32R, tag="wsing", name="wsing")
            nc.sync.dma_start(wsing[:], wsing_d[:].bitcast(F32R))
            nc.sync.dma_start(xcs[0][0:64, :], f0src)
            nc.sync.dma_start(xcs[0][64:128, :], xlo_d[0, 0].bitcast(F32R))
            bias_t = P1.tile([128, 1], F32, tag="bias", name="bias")
            nc.sync.dma_start(bias_t[:], bias_d[:])

            wcorr = P1.tile([128, 9 * 128], F32R, tag="wcorr", name="wcorr")
            nc.sync.dma_start(wcorr[:], wcorr_d[:].bitcast(F32R))
            w1t_s = P1.tile([T, 5], F32, tag="w1t", name="w1t")
            nc.sync.dma_start(w1t_s[:], w1t_d[:])
            w2t_s = P1.tile([5, T], F32, tag="w2t", name="w2t")
            nc.sync.dma_start(w2t_s[:], w2t_d[:])
            ident = P1.tile([128, 128], F32, tag="ident", name="ident")
            nc.sync.dma_start(ident[:], ident_d[:])
            ones_t = P1.tile([1, 128], F32, tag="ones", name="ones")
            nc.vector.memset(ones_t[:], 1.0)

            ys = [P1.tile([128, XCOL], F32, tag=f"y{i}", name=f"y{i}")
                  for i in range(NY)]
            gs = [P1.tile([128, HW], F32, tag=f"g{s}", name=f"g{s}")
                  for s in range(BPC)]
            # stats rows: 0-2 chunk sums, 3 -junk, 4 total, 5 max
            s_st = [P1.tile([128, 6 * T], F32, tag=f"S{s}", name=f"S{s}")
                    for s in range(BPC)]
            bc = [P1.tile([128, 4 * T], F32, tag=f"bc{s}", name=f"bc{s}")
                  for s in range(BPC)]

            engines = {"v": nc.vector, "p": nc.gpsimd}

            def conv_frame(s, t, skip_dma=False):
                f = s * T + t
                if not skip_dma:
                    x_dma(s, t)
                xa, xc = xas[f % 4], xcs[f % 4]
                y = ys[f % NY]
                for ci in range(3):
                    c = (f + ci) % 3
                    o = OUT0 + CN * c
                    ps = PP.tile([128, CN], F32, tag=f"p{c}", name=f"p{c}")
                    units = []
                    for i, dx in enumerate((-1, 0, 1)):
                        units.append((wpair[:, i * 128:(i + 1) * 128],
                                      xa[0:128, MAR + o - PW + dx:
                                         MAR + o - PW + dx + CN]))
                    for i, dx in enumerate((-1, 0, 1)):
                        units.append((wsing[:, i * 128:(i + 1) * 128],
                                      xa[0:64, MAR + o + dx:MAR + o + dx + CN]))
                    for j, (dy, dx) in enumerate(TAPS):
                        b0 = o + dy * PW + dx
                        units.append((wcorr[:, j * 128:(j + 1) * 128],
                                      xc[0:128, b0:b0 + CN]))
                    for k, (w_ap, x_ap) in enumerate(units):
                        nc.tensor.matmul(ps[:], w_ap, x_ap,
                                         start=(k == 0),
                                         stop=(k == len(units) - 1))
                    nc.scalar.activation(
                        y[:, o:o + CN], ps[:], AF.Identity,
                        bias=bias_t[:, 0:1],
                        accum_out=s_st[s][:, c * T + t:c * T + t + 1])
                # stats: -junk sum, max over real cols, total
                yj = y[:, MAR:MAR + 32 * PW].rearrange(
                    "p (r c) -> p r c", c=PW)
                nc.vector.reduce_sum(s_st[s][:, 3 * T + t:3 * T + t + 1],
                                     yj[:, :, 0:1], axis=AX.XY, negate=True)
                ym = y[:, OUT0:OUT0 + 32 * PW].rearrange(
                    "p (r c) -> p r c", c=PW)
                nc.vector.reduce_max(s_st[s][:, 5 * T + t:5 * T + t + 1],
                                     ym[:, :, 0:W], axis=AX.XY)
                sv = s_st[s].rearrange("p (k t) -> p k t", t=T)
                nc.vector.reduce_sum(sv[:, 4:5, t:t + 1], sv[:, 0:4, t:t + 1],
                                     axis=AX.XY)

            def attention(s):
                S = s_st[s]
                psT1 = PP.tile([T, 128], F32, tag="pa0", name="psT1")
                nc.tensor.transpose(psT1[:], S[:, 4 * T:5 * T], ident[:])
                psT2 = PP.tile([T, 128], F32, tag="pa1", name="psT2")
                nc.tensor.transpose(psT2[:], S[:, 5 * T:6 * T], ident[:])
                tmp = P2.tile([T, 1], F32, tag="att_tmp", name="att_tmp")
                nc.vector.reduce_sum(tmp[:], psT1[:], axis=AX.X)
                att_in = P2.tile([T, 2], F32, tag="att_in", name="att_in")
                nc.vector.tensor_scalar_mul(att_in[:, 0:1], tmp[:],
                                            1.0 / (CH * HW))
                nc.vector.reduce_max(att_in[:, 1:2], psT2[:], axis=AX.X)
                ps5 = PP.tile([5, 2], F32, tag="pa0", name="ps5")
                nc.tensor.matmul(ps5[:], w1t_s[:], att_in[:], start=True,
                                 stop=True)
                h5 = P2.tile([5, 2], F32, tag="h5", name="h5")
                nc.scalar.activation(h5[:], ps5[:], AF.Relu)
                ps20 = PP.tile([T, 2], F32, tag="pa1", name="ps20")
                nc.tensor.matmul(ps20[:], w2t_s[:], h5[:], start=True, stop=True)
                a20 = P2.tile([T, 2], F32, tag="a20", name="a20")
                nc.scalar.activation(a20[:], ps20[:], AF.Copy)
                attp = P2.tile([T, 1], F32, tag="attp", name="attp")
                nc.vector.tensor_tensor(attp[:], a20[:, 0:1], a20[:, 1:2],
                                        op=OP.add)
                # sigmoid via exp + reciprocal (tighter than the Sigmoid table)
                expz = P2.tile([T, 1], F32, tag="expz", name="expz")
                nc.scalar.activation(expz[:], attp[:], AF.Exp, scale=-1.0)
                att1 = P2.tile([T, 1], F32, tag="att1", name="att1")
                nc.vector.tensor_scalar_add(att1[:], expz[:], 1.0)
                att = P2.tile([T, 1], F32, tag="att", name="att")
                nc.vector.reciprocal(att[:], att1[:])
                psT3 = PP.tile([1, T], F32, tag="pa0", name="psT3")
                nc.tensor.transpose(psT3[:], att[:, 0:1], ident[0:T, 0:T])
                atts = P2.tile([1, T + 1], F32, tag="atts", name="atts")
                nc.scalar.activation(atts[0:1, 1:T + 1], psT3[:], AF.Copy)
                nc.scalar.activation(atts[0:1, 0:1], psT3[0:1, 0:1], AF.Copy)
                rec = P2.tile([1, T], F32, tag="rec", name="rec")
                nc.vector.reciprocal(rec[:], atts[0:1, 1:T + 1])
                rhs3 = P2.tile([1, 4 * T], F32, tag="rhs3", name="rhs3")
                nc.vector.scalar_tensor_tensor(
                    rhs3[0:1, 0:T], atts[0:1, 0:T], ALPHA, rec[:],
                    op0=OP.mult, op1=OP.mult)
                nc.vector.tensor_scalar_mul(rhs3[0:1, T:2 * T], rec[:], VTH)
                nc.vector.tensor_scalar_mul(rhs3[0:1, 2 * T:3 * T], rec[:],
                                            -VTH)
                nc.vector.tensor_scalar_mul(rhs3[0:1, 3 * T:4 * T], rec[:],
                                            -VTH * 1e8)
                ps_bc = PP.tile([128, 4 * T], F32, tag="pa1", name="ps_bc")
                nc.tensor.matmul(ps_bc[:], ones_t[:], rhs3[:], start=True,
                                 stop=True)
                nc.scalar.activation(bc[s][:], ps_bc[:], AF.Copy)

            def scan_step(s, t, vg, sp):
                f = s * T + t
                g = gs[s]
                if t == 0:
                    nc.vector.memset(g[:], 0.0)
                y = ys[f % NY]
                yv = y[:, OUT0:OUT0 + 32 * PW].rearrange(
                    "p (r c) -> p r c", c=PW)
                v = P2.tile([128, HW], F32, tag="v", name="v")
                m = (P2.tile([128, HW], F32, tag="m", name="m")
                     if any(e == "p" for e, _, _ in vg) else None)
                so = P3.tile([128, HW], F32, tag="so", name="so")
                vv = v.rearrange("p (r c) -> p r c", c=W)
                gv = g.rearrange("p (r c) -> p r c", c=W)
                cb = bc[s][:, t:t + 1]
                tn = min(t + 1, T - 1)
                cbn = bc[s][:, tn:tn + 1]
                thr = bc[s][:, T + t:T + t + 1]
                nthr = bc[s][:, 2 * T + t:2 * T + t + 1]
                nthr8 = bc[s][:, 3 * T + t:3 * T + t + 1]
                for eng, r0, r1 in vg:
                    R = slice(r0 // W, r1 // W)
                    if eng == "v":
                        nc.vector.scalar_tensor_tensor(
                            vv[:, R, :], gv[:, R, :], cb, yv[:, R, 0:W],
                            op0=OP.mult, op1=OP.add)
                        nc.vector.scalar_tensor_tensor(
                            g[:, r0:r1], v[:, r0:r1], thr, v[:, r0:r1],
                            op0=OP.is_lt, op1=OP.mult)
                    else:
                        # Pool rows keep g pre-multiplied by c_{t+1}:
                        # v = g + y; m = (v<thr)*c_next; g = m*v
                        nc.gpsimd.tensor_tensor(
                            vv[:, R, :], gv[:, R, :], yv[:, R, 0:W],
                            op=OP.add)
                        nc.gpsimd.tensor_scalar(
                            m[:, r0:r1], v[:, r0:r1], thr, cbn,
                            op0=OP.is_lt, op1=OP.mult)
                        nc.gpsimd.tensor_tensor(
                            g[:, r0:r1], m[:, r0:r1], v[:, r0:r1],
                            op=OP.mult)
                for eng, r0, r1 in sp:
                    if eng == "sig":
                        # saturated sigmoid: 1e8*(v - thr) is past the f32
                        # sigmoid saturation point except ~1e-7 from thr
                        nc.scalar.activation(so[:, r0:r1], v[:, r0:r1],
                                             AF.Sigmoid, bias=nthr8,
                                             scale=1e8)
                    elif eng == "pm":
                        # spike from m (= (v<thr)*c_next): exactly 0 iff spike
                        nc.gpsimd.tensor_scalar(
                            so[:, r0:r1], m[:, r0:r1], 0.0, None,
                            op0=OP.is_equal)
                    else:
                        nc.vector.tensor_scalar(
                            so[:, r0:r1], v[:, r0:r1], thr, None,
                            op0=OP.is_ge)
                nc.sync.dma_start(spk[s, t], so[:])

            OVERLAP_VG = [("v", 0, HW)]
            OVERLAP_SP = [("sig", 0, HW)]
            TAIL_VG = [("v", 0, 384), ("v", 384, 768), ("p", 768, HW)]
            TAIL_SP = [("sig", 0, HW)]

            conv_frame(0, 0, skip_dma=True)
            for t in range(1, T):
                conv_frame(0, t)
            conv_frame(1, 0)
            conv_frame(1, 1)
            attention(0)
            # input prefetch 2 frames ahead: the spk DMA inside scan_step
            # waits on the scan result and blocks the SP queue behind it
            x_dma(1, 2)
            x_dma(1, 3)
            for t in range(T - 2):
                scan_step(0, t, OVERLAP_VG, OVERLAP_SP)
                conv_frame(1, t + 2, skip_dma=True)
                if t + 4 < T:
                    x_dma(1, t + 4)
            attention(1)
            scan_step(0, T - 2, OVERLAP_VG, OVERLAP_SP)
            scan_step(0, T - 1, OVERLAP_VG, OVERLAP_SP)
            for t in range(T):
                scan_step(1, t, TAIL_VG, TAIL_SP)

    nc.compile()
    return nc


def _trunc13(a):
    # f32r = round-to-nearest, 11 explicit mantissa bits (HW-verified via
    # DMA roundtrip). Split values must be 11-bit so the hardware re-round
    # is a no-op and x_hi + x_lo == x exactly.
    u = np.ascontiguousarray(a, np.float32).view(np.uint32)
    r = (u + np.uint32(0x800)) & np.uint32(0xFFFFF000)
    return r.view(np.float32)


def _pad_frames(x):
    """[.., 64, 32, 32] -> [.., 64, XCOL] host-packed shared-halo frames."""
    lead = x.shape[:-2]
    padded = np.zeros(lead + (34, PW), np.float32)
    padded[..., 1:33, 1:33] = x
    out = np.zeros(lead + (XCOL,), np.float32)
    out[..., :34 * PW] = padded.reshape(lead + (34 * PW,))
    return out


def _prep_host_inputs(conv_w, conv_b, mlp_w1, mlp_w2):
    wT = np.ascontiguousarray(np.transpose(conv_w, (1, 0, 2, 3)))  # [64,128,3,3]
    hi = {}
    lo = {}
    for dy, dx in TAPS:
        blk = np.ascontiguousarray(wT[:, :, dy + 1, dx + 1])
        h = _trunc13(blk)
        hi[(dy, dx)] = h
        lo[(dy, dx)] = (blk - h).astype(np.float32)
    wpair = np.concatenate(
        [np.concatenate([hi[(-1, dx)], hi[(1, dx)]], axis=0)
         for dx in (-1, 0, 1)], axis=1)                            # [128, 384]
    wsing = np.concatenate([hi[(0, dx)] for dx in (-1, 0, 1)], axis=1)
    wcorr = np.concatenate(
        [np.concatenate([lo[tap], hi[tap]], axis=0) for tap in TAPS],
        axis=1)                                                    # [128, 1152]
    return {
        "wpair": np.ascontiguousarray(wpair, np.float32),
        "wsing": np.ascontiguousarray(wsing, np.float32),
        "wcorr": np.ascontiguousarray(wcorr, np.float32),
        "bias": np.ascontiguousarray(conv_b.reshape(128, 1), np.float32),
        "w1t": np.ascontiguousarray(mlp_w1.T).astype(np.float32),
        "w2t": np.ascontiguousarray(mlp_w2.T).astype(np.float32),
        "ident": np.eye(128, dtype=np.float32),
    }


_CACHED = {}


def make_in_maps(data, conv_w, conv_b, mlp_w1, mlp_w2):
    data = np.ascontiguousarray(data, np.float32)
    common = _prep_host_inputs(np.asarray(conv_w, np.float32),
                               np.asarray(conv_b, np.float32),
                               np.asarray(mlp_w1, np.float32),
                               np.asarray(mlp_w2, np.float32))
    in_maps = []
    for c in range(N_CORES):
        m = dict(common)
        shard = _pad_frames(data[c * BPC:(c + 1) * BPC])
        h = _trunc13(shard)
        m["xhi"] = h
        m["xlo"] = (shard - h).astype(np.float32)
        in_maps.append(m)
    return in_maps


def kernel(data, conv_w, conv_b, mlp_w1, mlp_w2):
    if "prog" not in _CACHED:
        _CACHED["prog"] = _build_program()
    nc = _CACHED["prog"]
    in_maps = make_in_maps(data, conv_w, conv_b, mlp_w1, mlp_w2)
    res = run_bass_kernel_spmd(nc, in_maps, list(range(N_CORES)))
    out = np.concatenate([res.results[c]["spk"] for c in range(N_CORES)], axis=0)
    return out.reshape(B, T, CH, H, W)


# revision 12
# speedup vs baseline: 2.4383x; 1.4213x over previous
"""Trainium2 Bass kernel for nn_ConvAttLIF (conv3x3 + temporal attention + LIF scan).

Sharding: data-parallel over batch B=16 across 8 NeuronCores (2 samples/core).

Layout: frames host-packed with shared row halos (33-wide rows: the right
halo of row r is the left halo of row r+1, both zero), so a frame is 1124
contiguous cols and the conv output span is 1056 cols = 3 psum chunks of 352.

Conv: per chunk, 15 f32r matmuls accumulate one psum bank:
  - 3 "pair" units (K=128): taps (-1,dx) and (+1,dx) fused by storing a
    second frame copy shifted 2 rows (66 cols) in partitions 64-127.
  - 3 "single" units (K=64): taps (0,dx) on partitions 0-63.
  - 9 "corr" units (K=128): [x_hi; x_lo] . [w_lo; w_hi] per tap, restoring
    ~fp32 accuracy from the 12-bit f32r operands (x_hi = trunc13(x)).
Chunks are processed in rotating order (frame f starts at chunk f%3) so each
frame's first psum bank was drained one chunk-stream earlier - no PE stall.

LIF scan: attention folded in via v_t = u_t/att_t, so each step is
v = g*c_t + y (STT), g = (v < thr_t)*v (STT, same engine - no cross-engine
hop in the serial chain), spike = (v >= thr_t) off-chain. The sample-1 tail
(no conv left to overlap) splits rows across DVE/Pool/ACT.

kernel(**inputs) takes the FULL unsharded inputs, returns the FULL output.
"""
import sys

sys.path.insert(0, "/opt/trn_rl_repo")

import numpy as np
import ml_dtypes
import concourse.bass as bass
import concourse.bacc as bacc
import concourse.tile as tile
import concourse.mybir as mybir
from concourse.bass_utils import run_bass_kernel_spmd

F32 = mybir.dt.float32
F32R = mybir.dt.float32r
FP8 = mybir.dt.float8e4
BF16 = mybir.dt.bfloat16
DR = mybir.MatmulPerfMode.DoubleRow
AF = mybir.ActivationFunctionType
OP = mybir.AluOpType
AX = mybir.AxisListType

B, T, CIN, H, W = 16, 20, 64, 32, 32
CH = 128
N_CORES = 8
BPC = B // N_CORES
ALPHA, VTH = 0.3, 0.6
HW = H * W                     # 1024
PW = W + 1                     # 33: row stride (shared halo col)
XCOL = 34 * PW + 2             # 1124 packed frame cols (+2 guard)
MAR = 2 * PW                   # 66: left margin in XA for the shifted copy
CN = 352                       # psum chunk cols (3 x 352 = 1056 out span)
OUT0 = PW + 1                  # 34: first out position in frame coords
NY = 24                        # y-tile ring size
TAPS = [(dy, dx) for dy in (-1, 0, 1) for dx in (-1, 0, 1)]


def _build_program():
    nc = bacc.Bacc("TRN2", target_bir_lowering=False, debug=False,
                   num_devices=N_CORES)

    xhi_d = nc.dram_tensor("xhi", [BPC, T, CIN, XCOL], F32,
                           kind="ExternalInput").ap()
    xc66_d = nc.dram_tensor("xc66", [BPC, T, 128, 2, MAR + XCOL], FP8,
                            kind="ExternalInput").ap()
    xc2_d = nc.dram_tensor("xc2", [BPC, T, 128, 2, 2 + XCOL], FP8,
                           kind="ExternalInput").ap()
    wpair_d = nc.dram_tensor("wpair", [128, 3 * 128], F32,
                             kind="ExternalInput").ap()
    wsing_d = nc.dram_tensor("wsing", [64, 3 * 128], F32,
                             kind="ExternalInput").ap()
    wc66_d = nc.dram_tensor("wc66", [128, 2, 3 * 128], FP8,
                            kind="ExternalInput").ap()
    wc2_d = nc.dram_tensor("wc2", [128, 2, 128], FP8,
                           kind="ExternalInput").ap()
    wc0_d = nc.dram_tensor("wc0", [128, 128], FP8,
                           kind="ExternalInput").ap()
    bias_d = nc.dram_tensor("bias", [128, 1], F32, kind="ExternalInput").ap()
    w1t_d = nc.dram_tensor("w1t", [T, 5], F32, kind="ExternalInput").ap()
    w2t_d = nc.dram_tensor("w2t", [5, T], F32, kind="ExternalInput").ap()
    ident_d = nc.dram_tensor("ident", [128, 128], F32, kind="ExternalInput").ap()
    spk = nc.dram_tensor("spk", [BPC, T, CH, HW], BF16,
                         kind="ExternalOutput").ap()

    with tile.TileContext(nc) as tc:
        with tc.tile_pool(name="sb", bufs=1) as P1, \
             tc.tile_pool(name="scr", bufs=2) as P2, \
             tc.tile_pool(name="so", bufs=3) as P3, \
             tc.tile_pool(name="ps", bufs=1, space="PSUM") as PP:

            # ---- persistent tiles ----
            xas = [P1.tile([128, MAR + XCOL], F32R, tag=f"xa{i}", name=f"xa{i}")
                   for i in range(4)]
            xc66s = [P1.tile([128, 2, MAR + XCOL], FP8, tag=f"x6{i}",
                             name=f"x6{i}") for i in range(4)]
            xc2s = [P1.tile([128, 2, 2 + XCOL], FP8, tag=f"x2{i}",
                            name=f"x2{i}") for i in range(4)]

            def x_dma(s, t):
                f = s * T + t
                src = xhi_d[s, t].bitcast(F32R)
                nc.sync.dma_start(xas[f % 4][0:64, MAR:MAR + XCOL], src)
                nc.sync.dma_start(xas[f % 4][64:128, 0:XCOL], src)
                nc.sync.dma_start(xc66s[f % 4][:], xc66_d[s, t])
                nc.sync.dma_start(xc2s[f % 4][:], xc2_d[s, t])

            # startup order: frame-0 XA halves, pair/single weights (first
            # units of the first chunk), then the corr inputs
            f0src = xhi_d[0, 0].bitcast(F32R)
            nc.sync.dma_start(xas[0][0:64, MAR:MAR + XCOL], f0src)
            nc.sync.dma_start(xas[0][64:128, 0:XCOL], f0src)
            wpair = P1.tile([128, 3 * 128], F32R, tag="wpair", name="wpair")
            nc.sync.dma_start(wpair[:], wpair_d[:].bitcast(F32R))
            wsing = P1.tile([64, 3 * 128], F32R, tag="wsing", name="wsing")
            nc.sync.dma_start(wsing[:], wsing_d[:].bitcast(F32R))
            nc.sync.dma_start(xc66s[0][:], xc66_d[0, 0])
            nc.sync.dma_start(xc2s[0][:], xc2_d[0, 0])
            bias_t = P1.tile([128, 1], F32, tag="bias", name="bias")
            nc.sync.dma_start(bias_t[:], bias_d[:])

            wc66_s = P1.tile([128, 2, 3 * 128], FP8, tag="wc66", name="wc66")
            nc.sync.dma_start(wc66_s[:], wc66_d[:])
            wc2_s = P1.tile([128, 2, 128], FP8, tag="wc2", name="wc2")
            nc.sync.dma_start(wc2_s[:], wc2_d[:])
            wc0_s = P1.tile([128, 128], FP8, tag="wc0", name="wc0")
            nc.sync.dma_start(wc0_s[:], wc0_d[:])
            w1t_s = P1.tile([T, 5], F32, tag="w1t", name="w1t")
            nc.sync.dma_start(w1t_s[:], w1t_d[:])
            w2t_s = P1.tile([5, T], F32, tag="w2t", name="w2t")
            nc.sync.dma_start(w2t_s[:], w2t_d[:])
            ident = P1.tile([128, 128], F32, tag="ident", name="ident")
            nc.sync.dma_start(ident[:], ident_d[:])
            ones_t = P1.tile([1, 128], F32, tag="ones", name="ones")
            nc.vector.memset(ones_t[:], 1.0)

            ys = [P1.tile([128, XCOL], F32, tag=f"y{i}", name=f"y{i}")
                  for i in range(NY)]
            gs = [P1.tile([128, HW], F32, tag=f"g{s}", name=f"g{s}")
                  for s in range(BPC)]
            # stats rows: 0-2 chunk sums, 3 -junk, 4 total, 5 max
            s_st = [P1.tile([128, 6 * T], F32, tag=f"S{s}", name=f"S{s}")
                    for s in range(BPC)]
            bc = [P1.tile([128, 4 * T], F32, tag=f"bc{s}", name=f"bc{s}")
                  for s in range(BPC)]

            engines = {"v": nc.vector, "p": nc.gpsimd}

            def conv_frame(s, t, skip_dma=False):
                f = s * T + t
                if not skip_dma:
                    x_dma(s, t)
                xa, x6, x2 = xas[f % 4], xc66s[f % 4], xc2s[f % 4]
                y = ys[f % NY]
                for ci in range(3):
                    c = (f + ci) % 3
                    o = OUT0 + CN * c
                    ps = PP.tile([128, CN], F32, tag=f"p{c}", name=f"p{c}")
                    units = []
                    for i, dx in enumerate((-1, 0, 1)):
                        units.append((wpair[:, i * 128:(i + 1) * 128],
                                      xa[0:128, MAR + o - PW + dx:
                                         MAR + o - PW + dx + CN], None))
                    for i, dx in enumerate((-1, 0, 1)):
                        units.append((wsing[:, i * 128:(i + 1) * 128],
                                      xa[0:64, MAR + o + dx:MAR + o + dx + CN],
                                      None))
                    # fp8 DoubleRow corr: plane0/plane1 pair taps (-1,dx)
                    # with (+1,dx) (delta 2*PW) and (0,-1) with (0,+1)
                    for i, dx in enumerate((-1, 0, 1)):
                        b0 = MAR + o - PW + dx
                        units.append((wc66_s[:, 0:2, i * 128:(i + 1) * 128],
                                      x6[:, 0:2, b0:b0 + CN], DR))
                    units.append((wc2_s[:, 0:2, :],
                                  x2[:, 0:2, 2 + o - 1:2 + o - 1 + CN], DR))
                    units.append((wc0_s[:], x6[:, 1, o:o + CN], None))
                    for k, (w_ap, x_ap, pm) in enumerate(units):
                        nc.tensor.matmul(ps[:], w_ap, x_ap,
                                         start=(k == 0),
                                         stop=(k == len(units) - 1),
                                         perf_mode=pm)
                    nc.scalar.activation(
                        y[:, o:o + CN], ps[:], AF.Identity,
                        bias=bias_t[:, 0:1], scale=1.0 / 65536.0,
                        accum_out=s_st[s][:, c * T + t:c * T + t + 1])
                # stats: -junk sum, max over real cols, total
                yj = y[:, MAR:MAR + 32 * PW].rearrange(
                    "p (r c) -> p r c", c=PW)
                nc.vector.reduce_sum(s_st[s][:, 3 * T + t:3 * T + t + 1],
                                     yj[:, :, 0:1], axis=AX.XY, negate=True)
                ym = y[:, OUT0:OUT0 + 32 * PW].rearrange(
                    "p (r c) -> p r c", c=PW)
                nc.vector.reduce_max(s_st[s][:, 5 * T + t:5 * T + t + 1],
                                     ym[:, :, 0:W], axis=AX.XY)
                sv = s_st[s].rearrange("p (k t) -> p k t", t=T)
                nc.vector.reduce_sum(sv[:, 4:5, t:t + 1], sv[:, 0:4, t:t + 1],
                                     axis=AX.XY)

            def attention(s):
                S = s_st[s]
                psT1 = PP.tile([T, 128], F32, tag="pa0", name="psT1")
                nc.tensor.transpose(psT1[:], S[:, 4 * T:5 * T], ident[:])
                psT2 = PP.tile([T, 128], F32, tag="pa1", name="psT2")
                nc.tensor.transpose(psT2[:], S[:, 5 * T:6 * T], ident[:])
                tmp = P2.tile([T, 1], F32, tag="att_tmp", name="att_tmp")
                nc.vector.reduce_sum(tmp[:], psT1[:], axis=AX.X)
                att_in = P2.tile([T, 2], F32, tag="att_in", name="att_in")
                nc.vector.tensor_scalar_mul(att_in[:, 0:1], tmp[:],
                                            1.0 / (CH * HW))
                nc.vector.reduce_max(att_in[:, 1:2], psT2[:], axis=AX.X)
                ps5 = PP.tile([5, 2], F32, tag="pa0", name="ps5")
                nc.tensor.matmul(ps5[:], w1t_s[:], att_in[:], start=True,
                                 stop=True)
                h5 = P2.tile([5, 2], F32, tag="h5", name="h5")
                nc.scalar.activation(h5[:], ps5[:], AF.Relu)
                ps20 = PP.tile([T, 2], F32, tag="pa1", name="ps20")
                nc.tensor.matmul(ps20[:], w2t_s[:], h5[:], start=True, stop=True)
                a20 = P2.tile([T, 2], F32, tag="a20", name="a20")
                nc.scalar.activation(a20[:], ps20[:], AF.Copy)
                attp = P2.tile([T, 1], F32, tag="attp", name="attp")
                nc.vector.tensor_tensor(attp[:], a20[:, 0:1], a20[:, 1:2],
                                        op=OP.add)
                # sigmoid via exp + reciprocal (tighter than the Sigmoid table)
                expz = P2.tile([T, 1], F32, tag="expz", name="expz")
                nc.scalar.activation(expz[:], attp[:], AF.Exp, scale=-1.0)
                att1 = P2.tile([T, 1], F32, tag="att1", name="att1")
                nc.vector.tensor_scalar_add(att1[:], expz[:], 1.0)
                att = P2.tile([T, 1], F32, tag="att", name="att")
                nc.vector.reciprocal(att[:], att1[:])
                psT3 = PP.tile([1, T], F32, tag="pa0", name="psT3")
                nc.tensor.transpose(psT3[:], att[:, 0:1], ident[0:T, 0:T])
                atts = P2.tile([1, T + 1], F32, tag="atts", name="atts")
                nc.scalar.activation(atts[0:1, 1:T + 1], psT3[:], AF.Copy)
                nc.scalar.activation(atts[0:1, 0:1], psT3[0:1, 0:1], AF.Copy)
                rec = P2.tile([1, T], F32, tag="rec", name="rec")
                nc.vector.reciprocal(rec[:], atts[0:1, 1:T + 1])
                rhs3 = P2.tile([1, 4 * T], F32, tag="rhs3", name="rhs3")
                nc.vector.scalar_tensor_tensor(
                    rhs3[0:1, 0:T], atts[0:1, 0:T], ALPHA, rec[:],
                    op0=OP.mult, op1=OP.mult)
                nc.vector.tensor_scalar_mul(rhs3[0:1, T:2 * T], rec[:], VTH)
                nc.vector.tensor_scalar_mul(rhs3[0:1, 2 * T:3 * T], rec[:],
                                            -VTH)
                nc.vector.tensor_scalar_mul(rhs3[0:1, 3 * T:4 * T], rec[:],
                                            -VTH * 1e8)
                ps_bc = PP.tile([128, 4 * T], F32, tag="pa1", name="ps_bc")
                nc.tensor.matmul(ps_bc[:], ones_t[:], rhs3[:], start=True,
                                 stop=True)
                nc.scalar.activation(bc[s][:], ps_bc[:], AF.Copy)

            def scan_step(s, t, vg, sp):
                f = s * T + t
                g = gs[s]
                if t == 0:
                    nc.vector.memset(g[:], 0.0)
                y = ys[f % NY]
                yv = y[:, OUT0:OUT0 + 32 * PW].rearrange(
                    "p (r c) -> p r c", c=PW)
                v = P2.tile([128, HW], F32, tag="v", name="v")
                m = (P2.tile([128, HW], F32, tag="m", name="m")
                     if any(e == "p" for e, _, _ in vg) else None)
                so = P3.tile([128, HW], BF16, tag="so", name="so")
                vv = v.rearrange("p (r c) -> p r c", c=W)
                gv = g.rearrange("p (r c) -> p r c", c=W)
                cb = bc[s][:, t:t + 1]
                tn = min(t + 1, T - 1)
                cbn = bc[s][:, tn:tn + 1]
                thr = bc[s][:, T + t:T + t + 1]
                nthr = bc[s][:, 2 * T + t:2 * T + t + 1]
                nthr8 = bc[s][:, 3 * T + t:3 * T + t + 1]
                for eng, r0, r1 in vg:
                    R = slice(r0 // W, r1 // W)
                    if eng == "v":
                        nc.vector.scalar_tensor_tensor(
                            vv[:, R, :], gv[:, R, :], cb, yv[:, R, 0:W],
                            op0=OP.mult, op1=OP.add)
                        nc.vector.scalar_tensor_tensor(
                            g[:, r0:r1], v[:, r0:r1], thr, v[:, r0:r1],
                            op0=OP.is_lt, op1=OP.mult)
                    else:
                        # Pool rows keep g pre-multiplied by c_{t+1}:
                        # v = g + y; m = (v<thr)*c_next; g = m*v
                        nc.gpsimd.tensor_tensor(
                            vv[:, R, :], gv[:, R, :], yv[:, R, 0:W],
                            op=OP.add)
                        nc.gpsimd.tensor_scalar(
                            m[:, r0:r1], v[:, r0:r1], thr, cbn,
                            op0=OP.is_lt, op1=OP.mult)
                        nc.gpsimd.tensor_tensor(
                            g[:, r0:r1], m[:, r0:r1], v[:, r0:r1],
                            op=OP.mult)
                for eng, r0, r1 in sp:
                    if eng == "sig":
                        # saturated sigmoid: 1e8*(v - thr) is past the f32
                        # sigmoid saturation point except ~1e-7 from thr
                        nc.scalar.activation(so[:, r0:r1], v[:, r0:r1],
                                             AF.Sigmoid, bias=nthr8,
                                             scale=1e8)
                    elif eng == "pm":
                        # spike from m (= (v<thr)*c_next): exactly 0 iff spike
                        nc.gpsimd.tensor_scalar(
                            so[:, r0:r1], m[:, r0:r1], 0.0, None,
                            op0=OP.is_equal)
                    else:
                        nc.vector.tensor_scalar(
                            so[:, r0:r1], v[:, r0:r1], thr, None,
                            op0=OP.is_ge)
                nc.sync.dma_start(spk[s, t], so[:])

            OVERLAP_VG = [("v", 0, 896), ("p", 896, HW)]
            OVERLAP_SP = [("sig", 0, HW)]
            TAIL_VG = [("v", 0, 384), ("v", 384, 768), ("p", 768, HW)]
            TAIL_SP = [("sig", 0, HW)]

            conv_frame(0, 0, skip_dma=True)
            for t in range(1, T):
                conv_frame(0, t)
            conv_frame(1, 0)
            conv_frame(1, 1)
            attention(0)
            # input prefetch 2 frames ahead: the spk DMA inside scan_step
            # waits on the scan result and blocks the SP queue behind it
            x_dma(1, 2)
            x_dma(1, 3)
            for t in range(T - 2):
                scan_step(0, t, OVERLAP_VG, OVERLAP_SP)
                conv_frame(1, t + 2, skip_dma=True)
                if t + 4 < T:
                    x_dma(1, t + 4)
            attention(1)
            scan_step(0, T - 2, OVERLAP_VG, OVERLAP_SP)
            scan_step(0, T - 1, OVERLAP_VG, OVERLAP_SP)
            for t in range(T):
                scan_step(1, t, TAIL_VG, TAIL_SP)

    nc.compile()
    return nc


def _trunc13(a):
    # f32r = round-to-nearest, 11 explicit mantissa bits (HW-verified via
    # DMA roundtrip). Split values must be 11-bit so the hardware re-round
    # is a no-op and x_hi + x_lo == x exactly.
    u = np.ascontiguousarray(a, np.float32).view(np.uint32)
    r = (u + np.uint32(0x800)) & np.uint32(0xFFFFF000)
    return r.view(np.float32)


def _pad_frames(x):
    """[.., 64, 32, 32] -> [.., 64, XCOL] host-packed shared-halo frames."""
    lead = x.shape[:-2]
    padded = np.zeros(lead + (34, PW), np.float32)
    padded[..., 1:33, 1:33] = x
    out = np.zeros(lead + (XCOL,), np.float32)
    out[..., :34 * PW] = padded.reshape(lead + (34 * PW,))
    return out


E4M3 = ml_dtypes.float8_e4m3fn


def _fp8(a):
    return np.asarray(a, np.float32).astype(E4M3)


def _prep_host_inputs(conv_w, conv_b, mlp_w1, mlp_w2):
    wT = np.ascontiguousarray(np.transpose(conv_w, (1, 0, 2, 3)))  # [64,128,3,3]
    hi = {}
    c8 = {}
    for dy, dx in TAPS:
        blk = np.ascontiguousarray(wT[:, :, dy + 1, dx + 1])
        h = _trunc13(blk)
        lo = (blk - h).astype(np.float32)
        hi[(dy, dx)] = h
        # fp8 corr weights: [w_lo*2^16 ; w_hi*2^4] (psum scale 2^16 with
        # x_lo prescaled by 2^12 on the data side)
        c8[(dy, dx)] = np.concatenate(
            [_fp8(lo * 65536.0), _fp8(h * 16.0)], axis=0)          # [128,128]
    # T1 weights prescaled by 2^16 (exact) to share the corr psum scale
    wpair = np.concatenate(
        [np.concatenate([hi[(-1, dx)], hi[(1, dx)]], axis=0)
         for dx in (-1, 0, 1)], axis=1) * 65536.0                  # [128, 384]
    wsing = np.concatenate(
        [hi[(0, dx)] for dx in (-1, 0, 1)], axis=1) * 65536.0
    wc66 = np.stack(
        [np.concatenate([c8[(-1, dx)] for dx in (-1, 0, 1)], axis=1),
         np.concatenate([c8[(1, dx)] for dx in (-1, 0, 1)], axis=1)],
        axis=1)                                                    # [128,2,384]
    wc2 = np.stack([c8[(0, -1)], c8[(0, 1)]], axis=1)              # [128,2,128]
    return {
        "wpair": np.ascontiguousarray(wpair, np.float32),
        "wsing": np.ascontiguousarray(wsing, np.float32),
        "wc66": np.ascontiguousarray(wc66),
        "wc2": np.ascontiguousarray(wc2),
        "wc0": np.ascontiguousarray(c8[(0, 0)]),
        "bias": np.ascontiguousarray(conv_b.reshape(128, 1), np.float32),
        "w1t": np.ascontiguousarray(mlp_w1.T).astype(np.float32),
        "w2t": np.ascontiguousarray(mlp_w2.T).astype(np.float32),
        "ident": np.eye(128, dtype=np.float32),
    }


_CACHED = {}


def make_in_maps(data, conv_w, conv_b, mlp_w1, mlp_w2):
    data = np.ascontiguousarray(data, np.float32)
    common = _prep_host_inputs(np.asarray(conv_w, np.float32),
                               np.asarray(conv_b, np.float32),
                               np.asarray(mlp_w1, np.float32),
                               np.asarray(mlp_w2, np.float32))
    in_maps = []
    for c in range(N_CORES):
        m = dict(common)
        shard = _pad_frames(data[c * BPC:(c + 1) * BPC])
        h = _trunc13(shard)
        m["xhi"] = h
        # fp8 corr data: [fp8(x_hi) ; fp8(x_lo*2^12)] in two shifted planes
        c8 = np.concatenate(
            [_fp8(h), _fp8((shard - h) * 4096.0)], axis=2)  # [BPC,T,128,XCOL]
        x66 = np.zeros((BPC, T, 128, 2, MAR + XCOL), E4M3)
        x66[:, :, :, 0, MAR:MAR + XCOL] = c8
        x66[:, :, :, 1, 0:XCOL] = c8
        m["xc66"] = x66
        x2 = np.zeros((BPC, T, 128, 2, 2 + XCOL), E4M3)
        x2[:, :, :, 0, 2:2 + XCOL] = c8
        x2[:, :, :, 1, 0:XCOL] = c8
        m["xc2"] = x2
        in_maps.append(m)
    return in_maps


def kernel(data, conv_w, conv_b, mlp_w1, mlp_w2):
    if "prog" not in _CACHED:
        _CACHED["prog"] = _build_program()
    nc = _CACHED["prog"]
    in_maps = make_in_maps(data, conv_w, conv_b, mlp_w1, mlp_w2)
    res = run_bass_kernel_spmd(nc, in_maps, list(range(N_CORES)))
    out = np.concatenate(
        [np.asarray(res.results[c]["spk"]).astype(np.float32)
         for c in range(N_CORES)], axis=0)
    return out.reshape(B, T, CH, H, W)


# revision 13
# speedup vs baseline: 2.4922x; 1.0221x over previous
"""Trainium2 Bass kernel for nn_ConvAttLIF (conv3x3 + temporal attention + LIF scan).

Sharding: data-parallel over batch B=16 across 8 NeuronCores (2 samples/core).

Layout: frames host-packed with shared row halos (33-wide rows: the right
halo of row r is the left halo of row r+1, both zero), so a frame is 1124
contiguous cols and the conv output span is 1056 cols = 3 psum chunks of 352.

Conv: per chunk, 15 f32r matmuls accumulate one psum bank:
  - 3 "pair" units (K=128): taps (-1,dx) and (+1,dx) fused by storing a
    second frame copy shifted 2 rows (66 cols) in partitions 64-127.
  - 3 "single" units (K=64): taps (0,dx) on partitions 0-63.
  - 9 "corr" units (K=128): [x_hi; x_lo] . [w_lo; w_hi] per tap, restoring
    ~fp32 accuracy from the 12-bit f32r operands (x_hi = trunc13(x)).
Chunks are processed in rotating order (frame f starts at chunk f%3) so each
frame's first psum bank was drained one chunk-stream earlier - no PE stall.

LIF scan: attention folded in via v_t = u_t/att_t, so each step is
v = g*c_t + y (STT), g = (v < thr_t)*v (STT, same engine - no cross-engine
hop in the serial chain), spike = (v >= thr_t) off-chain. The sample-1 tail
(no conv left to overlap) splits rows across DVE/Pool/ACT.

kernel(**inputs) takes the FULL unsharded inputs, returns the FULL output.
"""
import sys

sys.path.insert(0, "/opt/trn_rl_repo")

import numpy as np
import ml_dtypes
import concourse.bass as bass
import concourse.bacc as bacc
import concourse.tile as tile
import concourse.mybir as mybir
from concourse.bass_utils import run_bass_kernel_spmd

F32 = mybir.dt.float32
F32R = mybir.dt.float32r
FP8 = mybir.dt.float8e4
BF16 = mybir.dt.bfloat16
DR = mybir.MatmulPerfMode.DoubleRow
AF = mybir.ActivationFunctionType
OP = mybir.AluOpType
AX = mybir.AxisListType

B, T, CIN, H, W = 16, 20, 64, 32, 32
CH = 128
N_CORES = 8
BPC = B // N_CORES
ALPHA, VTH = 0.3, 0.6
HW = H * W                     # 1024
PW = W + 1                     # 33: row stride (shared halo col)
XCOL = 34 * PW + 2             # 1124 packed frame cols (+2 guard)
MAR = 2 * PW                   # 66: left margin in XA for the shifted copy
CN = 352                       # psum chunk cols (3 x 352 = 1056 out span)
OUT0 = PW + 1                  # 34: first out position in frame coords
NY = 24                        # y-tile ring size
TAPS = [(dy, dx) for dy in (-1, 0, 1) for dx in (-1, 0, 1)]


def _build_program():
    nc = bacc.Bacc("TRN2", target_bir_lowering=False, debug=False,
                   num_devices=N_CORES)

    xhi_d = nc.dram_tensor("xhi", [BPC, T, CIN, XCOL], F32,
                           kind="ExternalInput").ap()
    xc66_d = nc.dram_tensor("xc66", [BPC, T, 128, 2, MAR + XCOL], FP8,
                            kind="ExternalInput").ap()
    xc2_d = nc.dram_tensor("xc2", [BPC, T, 128, 2, 2 + XCOL], FP8,
                           kind="ExternalInput").ap()
    wpair_d = nc.dram_tensor("wpair", [128, 3 * 128], F32,
                             kind="ExternalInput").ap()
    wsing_d = nc.dram_tensor("wsing", [64, 3 * 128], F32,
                             kind="ExternalInput").ap()
    wc66_d = nc.dram_tensor("wc66", [128, 2, 3 * 128], FP8,
                            kind="ExternalInput").ap()
    wc2_d = nc.dram_tensor("wc2", [128, 2, 128], FP8,
                           kind="ExternalInput").ap()
    wc0_d = nc.dram_tensor("wc0", [128, 2, 128], FP8,
                           kind="ExternalInput").ap()
    bias_d = nc.dram_tensor("bias", [128, 1], F32, kind="ExternalInput").ap()
    w1t_d = nc.dram_tensor("w1t", [T, 5], F32, kind="ExternalInput").ap()
    w2t_d = nc.dram_tensor("w2t", [5, T], F32, kind="ExternalInput").ap()
    ident_d = nc.dram_tensor("ident", [128, 128], F32, kind="ExternalInput").ap()
    spk = nc.dram_tensor("spk", [BPC, T, CH, HW], FP8,
                         kind="ExternalOutput").ap()

    with tile.TileContext(nc) as tc:
        with tc.tile_pool(name="sb", bufs=1) as P1, \
             tc.tile_pool(name="scr", bufs=2) as P2, \
             tc.tile_pool(name="so", bufs=3) as P3, \
             tc.tile_pool(name="ps", bufs=1, space="PSUM") as PP:

            # ---- persistent tiles ----
            xas = [P1.tile([128, MAR + XCOL], F32R, tag=f"xa{i}", name=f"xa{i}")
                   for i in range(4)]
            xc66s = [P1.tile([128, 2, MAR + XCOL], FP8, tag=f"x6{i}",
                             name=f"x6{i}") for i in range(4)]
            xc2s = [P1.tile([128, 2, 2 + XCOL], FP8, tag=f"x2{i}",
                            name=f"x2{i}") for i in range(4)]

            def x_dma(s, t):
                f = s * T + t
                src = xhi_d[s, t].bitcast(F32R)
                nc.sync.dma_start(xas[f % 4][0:64, MAR:MAR + XCOL], src)
                nc.sync.dma_start(xas[f % 4][64:128, 0:XCOL], src)
                nc.sync.dma_start(xc66s[f % 4][:], xc66_d[s, t])
                nc.sync.dma_start(xc2s[f % 4][:], xc2_d[s, t])

            # startup order: frame-0 XA halves, pair/single weights (first
            # units of the first chunk), then the corr inputs
            f0src = xhi_d[0, 0].bitcast(F32R)
            nc.sync.dma_start(xas[0][0:64, MAR:MAR + XCOL], f0src)
            nc.sync.dma_start(xas[0][64:128, 0:XCOL], f0src)
            wpair = P1.tile([128, 3 * 128], F32R, tag="wpair", name="wpair")
            nc.sync.dma_start(wpair[:], wpair_d[:].bitcast(F32R))
            wsing = P1.tile([64, 3 * 128], F32R, tag="wsing", name="wsing")
            nc.sync.dma_start(wsing[:], wsing_d[:].bitcast(F32R))
            nc.sync.dma_start(xc66s[0][:], xc66_d[0, 0])
            nc.sync.dma_start(xc2s[0][:], xc2_d[0, 0])
            bias_t = P1.tile([128, 1], F32, tag="bias", name="bias")
            nc.sync.dma_start(bias_t[:], bias_d[:])

            wc66_s = P1.tile([128, 2, 3 * 128], FP8, tag="wc66", name="wc66")
            nc.sync.dma_start(wc66_s[:], wc66_d[:])
            wc2_s = P1.tile([128, 2, 128], FP8, tag="wc2", name="wc2")
            nc.sync.dma_start(wc2_s[:], wc2_d[:])
            wc0_s = P1.tile([128, 2, 128], FP8, tag="wc0", name="wc0")
            nc.sync.dma_start(wc0_s[:], wc0_d[:])
            w1t_s = P1.tile([T, 5], F32, tag="w1t", name="w1t")
            nc.sync.dma_start(w1t_s[:], w1t_d[:])
            w2t_s = P1.tile([5, T], F32, tag="w2t", name="w2t")
            nc.sync.dma_start(w2t_s[:], w2t_d[:])
            ident = P1.tile([128, 128], F32, tag="ident", name="ident")
            nc.sync.dma_start(ident[:], ident_d[:])
            ones_t = P1.tile([1, 128], F32, tag="ones", name="ones")
            nc.vector.memset(ones_t[:], 1.0)

            ys = [P1.tile([128, XCOL], F32, tag=f"y{i}", name=f"y{i}")
                  for i in range(NY)]
            gs = [P1.tile([128, HW], F32, tag=f"g{s}", name=f"g{s}")
                  for s in range(BPC)]
            # stats rows: 0-2 chunk sums, 3 -junk, 4 total, 5 max
            s_st = [P1.tile([128, 6 * T], F32, tag=f"S{s}", name=f"S{s}")
                    for s in range(BPC)]
            bc = [P1.tile([128, 4 * T], F32, tag=f"bc{s}", name=f"bc{s}")
                  for s in range(BPC)]

            engines = {"v": nc.vector, "p": nc.gpsimd}

            def conv_frame(s, t, skip_dma=False):
                f = s * T + t
                if not skip_dma:
                    x_dma(s, t)
                xa, x6, x2 = xas[f % 4], xc66s[f % 4], xc2s[f % 4]
                y = ys[f % NY]
                for ci in range(3):
                    c = (f + ci) % 3
                    o = OUT0 + CN * c
                    ps = PP.tile([128, CN], F32, tag=f"p{c}", name=f"p{c}")
                    units = []
                    for i, dx in enumerate((-1, 0, 1)):
                        units.append((wpair[:, i * 128:(i + 1) * 128],
                                      xa[0:128, MAR + o - PW + dx:
                                         MAR + o - PW + dx + CN], None))
                    for i, dx in enumerate((-1, 0, 1)):
                        units.append((wsing[:, i * 128:(i + 1) * 128],
                                      xa[0:64, MAR + o + dx:MAR + o + dx + CN],
                                      None))
                    # fp8 DoubleRow corr: plane0/plane1 pair taps (-1,dx)
                    # with (+1,dx) (delta 2*PW) and (0,-1) with (0,+1)
                    for i, dx in enumerate((-1, 0, 1)):
                        b0 = MAR + o - PW + dx
                        units.append((wc66_s[:, 0:2, i * 128:(i + 1) * 128],
                                      x6[:, 0:2, b0:b0 + CN], DR))
                    units.append((wc2_s[:, 0:2, :],
                                  x2[:, 0:2, 2 + o - 1:2 + o - 1 + CN], DR))
                    # tap (0,0) as DoubleRow with a zeroed second plane
                    units.append((wc0_s[:, 0:2, :],
                                  x6[:, 0:2, MAR + o:MAR + o + CN], DR))
                    for k, (w_ap, x_ap, pm) in enumerate(units):
                        nc.tensor.matmul(ps[:], w_ap, x_ap,
                                         start=(k == 0),
                                         stop=(k == len(units) - 1),
                                         perf_mode=pm)
                    nc.scalar.activation(
                        y[:, o:o + CN], ps[:], AF.Identity,
                        bias=bias_t[:, 0:1], scale=1.0 / 65536.0,
                        accum_out=s_st[s][:, c * T + t:c * T + t + 1])
                # stats: -junk sum, max over real cols, total
                yj = y[:, MAR:MAR + 32 * PW].rearrange(
                    "p (r c) -> p r c", c=PW)
                nc.vector.reduce_sum(s_st[s][:, 3 * T + t:3 * T + t + 1],
                                     yj[:, :, 0:1], axis=AX.XY, negate=True)
                ym = y[:, OUT0:OUT0 + 32 * PW].rearrange(
                    "p (r c) -> p r c", c=PW)
                nc.vector.reduce_max(s_st[s][:, 5 * T + t:5 * T + t + 1],
                                     ym[:, :, 0:W], axis=AX.XY)
                sv = s_st[s].rearrange("p (k t) -> p k t", t=T)
                nc.vector.reduce_sum(sv[:, 4:5, t:t + 1], sv[:, 0:4, t:t + 1],
                                     axis=AX.XY)

            def attention(s):
                S = s_st[s]
                psT1 = PP.tile([T, 128], F32, tag="pa0", name="psT1")
                nc.tensor.transpose(psT1[:], S[:, 4 * T:5 * T], ident[:])
                psT2 = PP.tile([T, 128], F32, tag="pa1", name="psT2")
                nc.tensor.transpose(psT2[:], S[:, 5 * T:6 * T], ident[:])
                tmp = P2.tile([T, 1], F32, tag="att_tmp", name="att_tmp")
                nc.vector.reduce_sum(tmp[:], psT1[:], axis=AX.X)
                att_in = P2.tile([T, 2], F32, tag="att_in", name="att_in")
                nc.vector.tensor_scalar_mul(att_in[:, 0:1], tmp[:],
                                            1.0 / (CH * HW))
                nc.vector.reduce_max(att_in[:, 1:2], psT2[:], axis=AX.X)
                ps5 = PP.tile([5, 2], F32, tag="pa0", name="ps5")
                nc.tensor.matmul(ps5[:], w1t_s[:], att_in[:], start=True,
                                 stop=True)
                h5 = P2.tile([5, 2], F32, tag="h5", name="h5")
                nc.scalar.activation(h5[:], ps5[:], AF.Relu)
                ps20 = PP.tile([T, 2], F32, tag="pa1", name="ps20")
                nc.tensor.matmul(ps20[:], w2t_s[:], h5[:], start=True, stop=True)
                a20 = P2.tile([T, 2], F32, tag="a20", name="a20")
                nc.scalar.activation(a20[:], ps20[:], AF.Copy)
                attp = P2.tile([T, 1], F32, tag="attp", name="attp")
                nc.vector.tensor_tensor(attp[:], a20[:, 0:1], a20[:, 1:2],
                                        op=OP.add)
                # sigmoid via exp + reciprocal (tighter than the Sigmoid table)
                expz = P2.tile([T, 1], F32, tag="expz", name="expz")
                nc.scalar.activation(expz[:], attp[:], AF.Exp, scale=-1.0)
                att1 = P2.tile([T, 1], F32, tag="att1", name="att1")
                nc.vector.tensor_scalar_add(att1[:], expz[:], 1.0)
                att = P2.tile([T, 1], F32, tag="att", name="att")
                nc.vector.reciprocal(att[:], att1[:])
                psT3 = PP.tile([1, T], F32, tag="pa0", name="psT3")
                nc.tensor.transpose(psT3[:], att[:, 0:1], ident[0:T, 0:T])
                atts = P2.tile([1, T + 1], F32, tag="atts", name="atts")
                nc.scalar.activation(atts[0:1, 1:T + 1], psT3[:], AF.Copy)
                nc.scalar.activation(atts[0:1, 0:1], psT3[0:1, 0:1], AF.Copy)
                rec = P2.tile([1, T], F32, tag="rec", name="rec")
                nc.vector.reciprocal(rec[:], atts[0:1, 1:T + 1])
                rhs3 = P2.tile([1, 4 * T], F32, tag="rhs3", name="rhs3")
                nc.vector.scalar_tensor_tensor(
                    rhs3[0:1, 0:T], atts[0:1, 0:T], ALPHA, rec[:],
                    op0=OP.mult, op1=OP.mult)
                nc.vector.tensor_scalar_mul(rhs3[0:1, T:2 * T], rec[:], VTH)
                nc.vector.tensor_scalar_mul(rhs3[0:1, 2 * T:3 * T], rec[:],
                                            -VTH)
                nc.vector.tensor_scalar_mul(rhs3[0:1, 3 * T:4 * T], rec[:],
                                            -VTH * 1e8)
                ps_bc = PP.tile([128, 4 * T], F32, tag="pa1", name="ps_bc")
                nc.tensor.matmul(ps_bc[:], ones_t[:], rhs3[:], start=True,
                                 stop=True)
                nc.scalar.activation(bc[s][:], ps_bc[:], AF.Copy)

            def scan_step(s, t, vg, sp):
                f = s * T + t
                g = gs[s]
                if t == 0:
                    nc.vector.memset(g[:], 0.0)
                y = ys[f % NY]
                yv = y[:, OUT0:OUT0 + 32 * PW].rearrange(
                    "p (r c) -> p r c", c=PW)
                v = P2.tile([128, HW], F32, tag="v", name="v")
                m = (P2.tile([128, HW], F32, tag="m", name="m")
                     if any(e == "p" for e, _, _ in vg) else None)
                so = P3.tile([128, HW], FP8, tag="so", name="so")
                vv = v.rearrange("p (r c) -> p r c", c=W)
                gv = g.rearrange("p (r c) -> p r c", c=W)
                cb = bc[s][:, t:t + 1]
                tn = min(t + 1, T - 1)
                cbn = bc[s][:, tn:tn + 1]
                thr = bc[s][:, T + t:T + t + 1]
                nthr = bc[s][:, 2 * T + t:2 * T + t + 1]
                nthr8 = bc[s][:, 3 * T + t:3 * T + t + 1]
                for eng, r0, r1 in vg:
                    R = slice(r0 // W, r1 // W)
                    if eng == "v":
                        nc.vector.scalar_tensor_tensor(
                            vv[:, R, :], gv[:, R, :], cb, yv[:, R, 0:W],
                            op0=OP.mult, op1=OP.add)
                        nc.vector.scalar_tensor_tensor(
                            g[:, r0:r1], v[:, r0:r1], thr, v[:, r0:r1],
                            op0=OP.is_lt, op1=OP.mult)
                    else:
                        # Pool rows keep g pre-multiplied by c_{t+1}:
                        # v = g + y; m = (v<thr)*c_next; g = m*v
                        nc.gpsimd.tensor_tensor(
                            vv[:, R, :], gv[:, R, :], yv[:, R, 0:W],
                            op=OP.add)
                        nc.gpsimd.tensor_scalar(
                            m[:, r0:r1], v[:, r0:r1], thr, cbn,
                            op0=OP.is_lt, op1=OP.mult)
                        nc.gpsimd.tensor_tensor(
                            g[:, r0:r1], m[:, r0:r1], v[:, r0:r1],
                            op=OP.mult)
                for eng, r0, r1 in sp:
                    if eng == "sig":
                        # saturated sigmoid: 1e8*(v - thr) is past the f32
                        # sigmoid saturation point except ~1e-7 from thr
                        nc.scalar.activation(so[:, r0:r1], v[:, r0:r1],
                                             AF.Sigmoid, bias=nthr8,
                                             scale=1e8)
                    elif eng == "pm":
                        # spike from m (= (v<thr)*c_next): exactly 0 iff spike
                        nc.gpsimd.tensor_scalar(
                            so[:, r0:r1], m[:, r0:r1], 0.0, None,
                            op0=OP.is_equal)
                    else:
                        nc.vector.tensor_scalar(
                            so[:, r0:r1], v[:, r0:r1], thr, None,
                            op0=OP.is_ge)
                nc.sync.dma_start(spk[s, t], so[:])

            OVERLAP_VG = [("v", 0, 896), ("p", 896, HW)]
            OVERLAP_SP = [("sig", 0, HW)]
            TAIL_VG = [("v", 0, 384), ("v", 384, 768), ("p", 768, HW)]
            TAIL_SP = [("sig", 0, HW)]

            conv_frame(0, 0, skip_dma=True)
            for t in range(1, T):
                conv_frame(0, t)
            conv_frame(1, 0)
            conv_frame(1, 1)
            attention(0)
            # input prefetch 2 frames ahead: the spk DMA inside scan_step
            # waits on the scan result and blocks the SP queue behind it
            x_dma(1, 2)
            x_dma(1, 3)
            for t in range(T - 2):
                scan_step(0, t, OVERLAP_VG, OVERLAP_SP)
                conv_frame(1, t + 2, skip_dma=True)
                if t + 4 < T:
                    x_dma(1, t + 4)
            attention(1)
            scan_step(0, T - 2, OVERLAP_VG, OVERLAP_SP)
            scan_step(0, T - 1, OVERLAP_VG, OVERLAP_SP)
            for t in range(T):
                scan_step(1, t, TAIL_VG, TAIL_SP)

    nc.compile()
    return nc


def _trunc13(a):
    # f32r = round-to-nearest, 11 explicit mantissa bits (HW-verified via
    # DMA roundtrip). Split values must be 11-bit so the hardware re-round
    # is a no-op and x_hi + x_lo == x exactly.
    u = np.ascontiguousarray(a, np.float32).view(np.uint32)
    r = (u + np.uint32(0x800)) & np.uint32(0xFFFFF000)
    return r.view(np.float32)


def _pad_frames(x):
    """[.., 64, 32, 32] -> [.., 64, XCOL] host-packed shared-halo frames."""
    lead = x.shape[:-2]
    padded = np.zeros(lead + (34, PW), np.float32)
    padded[..., 1:33, 1:33] = x
    out = np.zeros(lead + (XCOL,), np.float32)
    out[..., :34 * PW] = padded.reshape(lead + (34 * PW,))
    return out


E4M3 = ml_dtypes.float8_e4m3fn


def _fp8(a):
    return np.asarray(a, np.float32).astype(E4M3)


def _prep_host_inputs(conv_w, conv_b, mlp_w1, mlp_w2):
    wT = np.ascontiguousarray(np.transpose(conv_w, (1, 0, 2, 3)))  # [64,128,3,3]
    hi = {}
    c8 = {}
    for dy, dx in TAPS:
        blk = np.ascontiguousarray(wT[:, :, dy + 1, dx + 1])
        h = _trunc13(blk)
        lo = (blk - h).astype(np.float32)
        hi[(dy, dx)] = h
        # fp8 corr weights: [w_lo*2^16 ; w_hi*2^4] (psum scale 2^16 with
        # x_lo prescaled by 2^12 on the data side)
        c8[(dy, dx)] = np.concatenate(
            [_fp8(lo * 65536.0), _fp8(h * 16.0)], axis=0)          # [128,128]
    # T1 weights prescaled by 2^16 (exact) to share the corr psum scale
    wpair = np.concatenate(
        [np.concatenate([hi[(-1, dx)], hi[(1, dx)]], axis=0)
         for dx in (-1, 0, 1)], axis=1) * 65536.0                  # [128, 384]
    wsing = np.concatenate(
        [hi[(0, dx)] for dx in (-1, 0, 1)], axis=1) * 65536.0
    wc66 = np.stack(
        [np.concatenate([c8[(-1, dx)] for dx in (-1, 0, 1)], axis=1),
         np.concatenate([c8[(1, dx)] for dx in (-1, 0, 1)], axis=1)],
        axis=1)                                                    # [128,2,384]
    wc2 = np.stack([c8[(0, -1)], c8[(0, 1)]], axis=1)              # [128,2,128]
    return {
        "wpair": np.ascontiguousarray(wpair, np.float32),
        "wsing": np.ascontiguousarray(wsing, np.float32),
        "wc66": np.ascontiguousarray(wc66),
        "wc2": np.ascontiguousarray(wc2),
        "wc0": np.ascontiguousarray(
            np.stack([c8[(0, 0)], np.zeros_like(c8[(0, 0)])], axis=1)),
        "bias": np.ascontiguousarray(conv_b.reshape(128, 1), np.float32),
        "w1t": np.ascontiguousarray(mlp_w1.T).astype(np.float32),
        "w2t": np.ascontiguousarray(mlp_w2.T).astype(np.float32),
        "ident": np.eye(128, dtype=np.float32),
    }


_CACHED = {}


def make_in_maps(data, conv_w, conv_b, mlp_w1, mlp_w2):
    data = np.ascontiguousarray(data, np.float32)
    common = _prep_host_inputs(np.asarray(conv_w, np.float32),
                               np.asarray(conv_b, np.float32),
                               np.asarray(mlp_w1, np.float32),
                               np.asarray(mlp_w2, np.float32))
    in_maps = []
    for c in range(N_CORES):
        m = dict(common)
        shard = _pad_frames(data[c * BPC:(c + 1) * BPC])
        h = _trunc13(shard)
        m["xhi"] = h
        # fp8 corr data: [fp8(x_hi) ; fp8(x_lo*2^12)] in two shifted planes
        c8 = np.concatenate(
            [_fp8(h), _fp8((shard - h) * 4096.0)], axis=2)  # [BPC,T,128,XCOL]
        x66 = np.zeros((BPC, T, 128, 2, MAR + XCOL), E4M3)
        x66[:, :, :, 0, MAR:MAR + XCOL] = c8
        x66[:, :, :, 1, 0:XCOL] = c8
        m["xc66"] = x66
        x2 = np.zeros((BPC, T, 128, 2, 2 + XCOL), E4M3)
        x2[:, :, :, 0, 2:2 + XCOL] = c8
        x2[:, :, :, 1, 0:XCOL] = c8
        m["xc2"] = x2
        in_maps.append(m)
    return in_maps


def kernel(data, conv_w, conv_b, mlp_w1, mlp_w2):
    if "prog" not in _CACHED:
        _CACHED["prog"] = _build_program()
    nc = _CACHED["prog"]
    in_maps = make_in_maps(data, conv_w, conv_b, mlp_w1, mlp_w2)
    res = run_bass_kernel_spmd(nc, in_maps, list(range(N_CORES)))
    out = np.concatenate(
        [np.asarray(res.results[c]["spk"]).astype(np.float32)
         for c in range(N_CORES)], axis=0)
    return out.reshape(B, T, CH, H, W)


# revision 14
# speedup vs baseline: 2.6166x; 1.0499x over previous
"""Trainium2 Bass kernel for nn_ConvAttLIF (conv3x3 + temporal attention + LIF scan).

Sharding: data-parallel over batch B=16 across 8 NeuronCores (2 samples/core).

Layout: frames host-packed with shared row halos (33-wide rows: the right
halo of row r is the left halo of row r+1, both zero), so a frame is 1124
contiguous cols and the conv output span is 1056 cols = 3 psum chunks of 352.

Conv: per chunk, 15 f32r matmuls accumulate one psum bank:
  - 3 "pair" units (K=128): taps (-1,dx) and (+1,dx) fused by storing a
    second frame copy shifted 2 rows (66 cols) in partitions 64-127.
  - 3 "single" units (K=64): taps (0,dx) on partitions 0-63.
  - 9 "corr" units (K=128): [x_hi; x_lo] . [w_lo; w_hi] per tap, restoring
    ~fp32 accuracy from the 12-bit f32r operands (x_hi = trunc13(x)).
Chunks are processed in rotating order (frame f starts at chunk f%3) so each
frame's first psum bank was drained one chunk-stream earlier - no PE stall.

LIF scan: attention folded in via v_t = u_t/att_t, so each step is
v = g*c_t + y (STT), g = (v < thr_t)*v (STT, same engine - no cross-engine
hop in the serial chain), spike = (v >= thr_t) off-chain. The sample-1 tail
(no conv left to overlap) splits rows across DVE/Pool/ACT.

kernel(**inputs) takes the FULL unsharded inputs, returns the FULL output.
"""
import sys

sys.path.insert(0, "/opt/trn_rl_repo")

import numpy as np
import ml_dtypes
import concourse.bass as bass
import concourse.bacc as bacc
import concourse.tile as tile
import concourse.mybir as mybir
from concourse.bass_utils import run_bass_kernel_spmd

F32 = mybir.dt.float32
F32R = mybir.dt.float32r
FP8 = mybir.dt.float8e4
BF16 = mybir.dt.bfloat16
DR = mybir.MatmulPerfMode.DoubleRow
AF = mybir.ActivationFunctionType
OP = mybir.AluOpType
AX = mybir.AxisListType

B, T, CIN, H, W = 16, 20, 64, 32, 32
CH = 128
N_CORES = 8
BPC = B // N_CORES
ALPHA, VTH = 0.3, 0.6
HW = H * W                     # 1024
PW = W + 1                     # 33: row stride (shared halo col)
XCOL = 34 * PW + 2             # 1124 packed frame cols (+2 guard)
MAR = 2 * PW                   # 66: left margin in XA for the shifted copy
CN = 352                       # psum chunk cols (3 x 352 = 1056 out span)
OUT0 = PW + 1                  # 34: first out position in frame coords
NY = 24                        # y-tile ring size
TAPS = [(dy, dx) for dy in (-1, 0, 1) for dx in (-1, 0, 1)]


def _build_program():
    nc = bacc.Bacc("TRN2", target_bir_lowering=False, debug=False,
                   num_devices=N_CORES)

    xhi_d = nc.dram_tensor("xhi", [BPC, T, CIN, XCOL], F32,
                           kind="ExternalInput").ap()
    xc66_d = nc.dram_tensor("xc66", [BPC, T, 128, 2, MAR + XCOL], FP8,
                            kind="ExternalInput").ap()
    xc2_d = nc.dram_tensor("xc2", [BPC, T, 128, 2, 2 + XCOL], FP8,
                           kind="ExternalInput").ap()
    wpair_d = nc.dram_tensor("wpair", [128, 3 * 128], F32,
                             kind="ExternalInput").ap()
    wsing_d = nc.dram_tensor("wsing", [64, 3 * 128], F32,
                             kind="ExternalInput").ap()
    wc66_d = nc.dram_tensor("wc66", [128, 2, 3 * 128], FP8,
                            kind="ExternalInput").ap()
    wc2_d = nc.dram_tensor("wc2", [128, 2, 128], FP8,
                           kind="ExternalInput").ap()
    wc0_d = nc.dram_tensor("wc0", [128, 2, 128], FP8,
                           kind="ExternalInput").ap()
    bias_d = nc.dram_tensor("bias", [128, 1], F32, kind="ExternalInput").ap()
    w1t_d = nc.dram_tensor("w1t", [T, 5], F32, kind="ExternalInput").ap()
    w2t_d = nc.dram_tensor("w2t", [5, T], F32, kind="ExternalInput").ap()
    ident_d = nc.dram_tensor("ident", [128, 128], F32, kind="ExternalInput").ap()
    spk = nc.dram_tensor("spk", [BPC, T, CH, HW], FP8,
                         kind="ExternalOutput").ap()

    with tile.TileContext(nc) as tc:
        with tc.tile_pool(name="sb", bufs=1) as P1, \
             tc.tile_pool(name="scr", bufs=2) as P2, \
             tc.tile_pool(name="so", bufs=3) as P3, \
             tc.tile_pool(name="ps", bufs=1, space="PSUM") as PP:

            # ---- persistent tiles ----
            xas = [P1.tile([128, MAR + XCOL], F32R, tag=f"xa{i}", name=f"xa{i}")
                   for i in range(4)]
            xc66s = [P1.tile([128, 2, MAR + XCOL], FP8, tag=f"x6{i}",
                             name=f"x6{i}") for i in range(4)]
            xc2s = [P1.tile([128, 2, 2 + XCOL], FP8, tag=f"x2{i}",
                            name=f"x2{i}") for i in range(4)]

            def x_dma(s, t):
                f = s * T + t
                src = xhi_d[s, t].bitcast(F32R)
                nc.sync.dma_start(xas[f % 4][0:64, MAR:MAR + XCOL], src)
                nc.sync.dma_start(xas[f % 4][64:128, 0:XCOL], src)
                nc.sync.dma_start(xc66s[f % 4][:], xc66_d[s, t])
                nc.sync.dma_start(xc2s[f % 4][:], xc2_d[s, t])

            # startup order: frame-0 XA halves, pair/single weights (first
            # units of the first chunk), then the corr inputs
            f0src = xhi_d[0, 0].bitcast(F32R)
            nc.sync.dma_start(xas[0][0:64, MAR:MAR + XCOL], f0src)
            nc.sync.dma_start(xas[0][64:128, 0:XCOL], f0src)
            wpair = P1.tile([128, 3 * 128], F32R, tag="wpair", name="wpair")
            nc.sync.dma_start(wpair[:], wpair_d[:].bitcast(F32R))
            wsing = P1.tile([64, 3 * 128], F32R, tag="wsing", name="wsing")
            nc.sync.dma_start(wsing[:], wsing_d[:].bitcast(F32R))
            nc.sync.dma_start(xc66s[0][:], xc66_d[0, 0])
            nc.sync.dma_start(xc2s[0][:], xc2_d[0, 0])
            bias_t = P1.tile([128, 1], F32, tag="bias", name="bias")
            nc.sync.dma_start(bias_t[:], bias_d[:])

            wc66_s = P1.tile([128, 2, 3 * 128], FP8, tag="wc66", name="wc66")
            nc.sync.dma_start(wc66_s[:], wc66_d[:])
            wc2_s = P1.tile([128, 2, 128], FP8, tag="wc2", name="wc2")
            nc.sync.dma_start(wc2_s[:], wc2_d[:])
            wc0_s = P1.tile([128, 2, 128], FP8, tag="wc0", name="wc0")
            nc.sync.dma_start(wc0_s[:], wc0_d[:])
            w1t_s = P1.tile([T, 5], F32, tag="w1t", name="w1t")
            nc.sync.dma_start(w1t_s[:], w1t_d[:])
            w2t_s = P1.tile([5, T], F32, tag="w2t", name="w2t")
            nc.sync.dma_start(w2t_s[:], w2t_d[:])
            ident = P1.tile([128, 128], F32, tag="ident", name="ident")
            nc.sync.dma_start(ident[:], ident_d[:])
            ones_t = P1.tile([1, 128], F32, tag="ones", name="ones")
            nc.vector.memset(ones_t[:], 1.0)

            ys = [P1.tile([128, XCOL], F32, tag=f"y{i}", name=f"y{i}")
                  for i in range(NY)]
            gs = [P1.tile([128, HW], F32, tag=f"g{s}", name=f"g{s}")
                  for s in range(BPC)]
            # stats rows: 0-2 chunk sums, 3 -junk, 4 total, 5 max
            s_st = [P1.tile([128, 6 * T], F32, tag=f"S{s}", name=f"S{s}")
                    for s in range(BPC)]
            bc = [P1.tile([128, 4 * T], F32, tag=f"bc{s}", name=f"bc{s}")
                  for s in range(BPC)]

            engines = {"v": nc.vector, "p": nc.gpsimd}

            def conv_frame(s, t, skip_dma=False):
                f = s * T + t
                if not skip_dma:
                    x_dma(s, t)
                xa, x6, x2 = xas[f % 4], xc66s[f % 4], xc2s[f % 4]
                y = ys[f % NY]
                for ci in range(3):
                    c = (f + ci) % 3
                    o = OUT0 + CN * c
                    ps = PP.tile([128, CN], F32, tag=f"p{c}{f % 2}",
                                 name=f"p{c}{f % 2}")
                    units = []
                    for i, dx in enumerate((-1, 0, 1)):
                        units.append((wpair[:, i * 128:(i + 1) * 128],
                                      xa[0:128, MAR + o - PW + dx:
                                         MAR + o - PW + dx + CN], None))
                    for i, dx in enumerate((-1, 0, 1)):
                        units.append((wsing[:, i * 128:(i + 1) * 128],
                                      xa[0:64, MAR + o + dx:MAR + o + dx + CN],
                                      None))
                    # fp8 DoubleRow corr: plane0/plane1 pair taps (-1,dx)
                    # with (+1,dx) (delta 2*PW) and (0,-1) with (0,+1)
                    for i, dx in enumerate((-1, 0, 1)):
                        b0 = MAR + o - PW + dx
                        units.append((wc66_s[:, 0:2, i * 128:(i + 1) * 128],
                                      x6[:, 0:2, b0:b0 + CN], DR))
                    units.append((wc2_s[:, 0:2, :],
                                  x2[:, 0:2, 2 + o - 1:2 + o - 1 + CN], DR))
                    # tap (0,0) as DoubleRow with a zeroed second plane
                    units.append((wc0_s[:, 0:2, :],
                                  x6[:, 0:2, MAR + o:MAR + o + CN], DR))
                    for k, (w_ap, x_ap, pm) in enumerate(units):
                        nc.tensor.matmul(ps[:], w_ap, x_ap,
                                         start=(k == 0),
                                         stop=(k == len(units) - 1),
                                         perf_mode=pm)
                    nc.scalar.activation(
                        y[:, o:o + CN], ps[:], AF.Identity,
                        bias=bias_t[:, 0:1], scale=1.0 / 65536.0,
                        accum_out=s_st[s][:, c * T + t:c * T + t + 1])
                # stats: -junk sum, max over real cols, total
                yj = y[:, MAR:MAR + 32 * PW].rearrange(
                    "p (r c) -> p r c", c=PW)
                nc.vector.reduce_sum(s_st[s][:, 3 * T + t:3 * T + t + 1],
                                     yj[:, :, 0:1], axis=AX.XY, negate=True)
                ym = y[:, OUT0:OUT0 + 32 * PW].rearrange(
                    "p (r c) -> p r c", c=PW)
                nc.vector.reduce_max(s_st[s][:, 5 * T + t:5 * T + t + 1],
                                     ym[:, :, 0:W], axis=AX.XY)
                sv = s_st[s].rearrange("p (k t) -> p k t", t=T)
                nc.vector.reduce_sum(sv[:, 4:5, t:t + 1], sv[:, 0:4, t:t + 1],
                                     axis=AX.XY)

            def attention(s):
                S = s_st[s]
                psT1 = PP.tile([T, 128], F32, tag="pa0", name="psT1")
                nc.tensor.transpose(psT1[:], S[:, 4 * T:5 * T], ident[:])
                psT2 = PP.tile([T, 128], F32, tag="pa1", name="psT2")
                nc.tensor.transpose(psT2[:], S[:, 5 * T:6 * T], ident[:])
                tmp = P2.tile([T, 1], F32, tag="att_tmp", name="att_tmp")
                nc.vector.reduce_sum(tmp[:], psT1[:], axis=AX.X)
                att_in = P2.tile([T, 2], F32, tag="att_in", name="att_in")
                nc.vector.tensor_scalar_mul(att_in[:, 0:1], tmp[:],
                                            1.0 / (CH * HW))
                nc.vector.reduce_max(att_in[:, 1:2], psT2[:], axis=AX.X)
                ps5 = PP.tile([5, 2], F32, tag="pa0", name="ps5")
                nc.tensor.matmul(ps5[:], w1t_s[:], att_in[:], start=True,
                                 stop=True)
                h5 = P2.tile([5, 2], F32, tag="h5", name="h5")
                nc.scalar.activation(h5[:], ps5[:], AF.Relu)
                ps20 = PP.tile([T, 2], F32, tag="pa1", name="ps20")
                nc.tensor.matmul(ps20[:], w2t_s[:], h5[:], start=True, stop=True)
                a20 = P2.tile([T, 2], F32, tag="a20", name="a20")
                nc.scalar.activation(a20[:], ps20[:], AF.Copy)
                attp = P2.tile([T, 1], F32, tag="attp", name="attp")
                nc.vector.tensor_tensor(attp[:], a20[:, 0:1], a20[:, 1:2],
                                        op=OP.add)
                # sigmoid via exp + reciprocal (tighter than the Sigmoid table)
                expz = P2.tile([T, 1], F32, tag="expz", name="expz")
                nc.scalar.activation(expz[:], attp[:], AF.Exp, scale=-1.0)
                att1 = P2.tile([T, 1], F32, tag="att1", name="att1")
                nc.vector.tensor_scalar_add(att1[:], expz[:], 1.0)
                att = P2.tile([T, 1], F32, tag="att", name="att")
                nc.vector.reciprocal(att[:], att1[:])
                psT3 = PP.tile([1, T], F32, tag="pa0", name="psT3")
                nc.tensor.transpose(psT3[:], att[:, 0:1], ident[0:T, 0:T])
                atts = P2.tile([1, T + 1], F32, tag="atts", name="atts")
                nc.scalar.activation(atts[0:1, 1:T + 1], psT3[:], AF.Copy)
                nc.scalar.activation(atts[0:1, 0:1], psT3[0:1, 0:1], AF.Copy)
                rec = P2.tile([1, T], F32, tag="rec", name="rec")
                nc.vector.reciprocal(rec[:], atts[0:1, 1:T + 1])
                rhs3 = P2.tile([1, 4 * T], F32, tag="rhs3", name="rhs3")
                nc.vector.scalar_tensor_tensor(
                    rhs3[0:1, 0:T], atts[0:1, 0:T], ALPHA, rec[:],
                    op0=OP.mult, op1=OP.mult)
                nc.vector.tensor_scalar_mul(rhs3[0:1, T:2 * T], rec[:], VTH)
                nc.vector.tensor_scalar_mul(rhs3[0:1, 2 * T:3 * T], rec[:],
                                            -VTH)
                nc.vector.tensor_scalar_mul(rhs3[0:1, 3 * T:4 * T], rec[:],
                                            -VTH * 1e8)
                ps_bc = PP.tile([128, 4 * T], F32, tag="pa1", name="ps_bc")
                nc.tensor.matmul(ps_bc[:], ones_t[:], rhs3[:], start=True,
                                 stop=True)
                nc.scalar.activation(bc[s][:], ps_bc[:], AF.Copy)

            def scan_step(s, t, vg, sp):
                f = s * T + t
                g = gs[s]
                if t == 0:
                    nc.vector.memset(g[:], 0.0)
                y = ys[f % NY]
                yv = y[:, OUT0:OUT0 + 32 * PW].rearrange(
                    "p (r c) -> p r c", c=PW)
                v = P2.tile([128, HW], F32, tag="v", name="v")
                m = (P2.tile([128, HW], F32, tag="m", name="m")
                     if any(e == "p" for e, _, _ in vg) else None)
                so = P3.tile([128, HW], FP8, tag="so", name="so")
                vv = v.rearrange("p (r c) -> p r c", c=W)
                gv = g.rearrange("p (r c) -> p r c", c=W)
                cb = bc[s][:, t:t + 1]
                tn = min(t + 1, T - 1)
                cbn = bc[s][:, tn:tn + 1]
                thr = bc[s][:, T + t:T + t + 1]
                nthr = bc[s][:, 2 * T + t:2 * T + t + 1]
                nthr8 = bc[s][:, 3 * T + t:3 * T + t + 1]
                for eng, r0, r1 in vg:
                    R = slice(r0 // W, r1 // W)
                    if eng == "v":
                        nc.vector.scalar_tensor_tensor(
                            vv[:, R, :], gv[:, R, :], cb, yv[:, R, 0:W],
                            op0=OP.mult, op1=OP.add)
                        nc.vector.scalar_tensor_tensor(
                            g[:, r0:r1], v[:, r0:r1], thr, v[:, r0:r1],
                            op0=OP.is_lt, op1=OP.mult)
                    else:
                        # Pool rows keep g pre-multiplied by c_{t+1}:
                        # v = g + y; m = (v<thr)*c_next; g = m*v
                        nc.gpsimd.tensor_tensor(
                            vv[:, R, :], gv[:, R, :], yv[:, R, 0:W],
                            op=OP.add)
                        nc.gpsimd.tensor_scalar(
                            m[:, r0:r1], v[:, r0:r1], thr, cbn,
                            op0=OP.is_lt, op1=OP.mult)
                        nc.gpsimd.tensor_tensor(
                            g[:, r0:r1], m[:, r0:r1], v[:, r0:r1],
                            op=OP.mult)
                for eng, r0, r1 in sp:
                    if eng == "sig":
                        # saturated sigmoid: 1e8*(v - thr) is past the f32
                        # sigmoid saturation point except ~1e-7 from thr
                        nc.scalar.activation(so[:, r0:r1], v[:, r0:r1],
                                             AF.Sigmoid, bias=nthr8,
                                             scale=1e8)
                    elif eng == "pm":
                        # spike from m (= (v<thr)*c_next): exactly 0 iff spike
                        nc.gpsimd.tensor_scalar(
                            so[:, r0:r1], m[:, r0:r1], 0.0, None,
                            op0=OP.is_equal)
                    else:
                        nc.vector.tensor_scalar(
                            so[:, r0:r1], v[:, r0:r1], thr, None,
                            op0=OP.is_ge)
                nc.sync.dma_start(spk[s, t], so[:])

            OVERLAP_VG = [("v", 0, 896), ("p", 896, HW)]
            OVERLAP_SP = [("sig", 0, HW)]
            TAIL_VG = [("v", 0, 384), ("v", 384, 768), ("p", 768, HW)]
            TAIL_SP = [("sig", 0, HW)]

            conv_frame(0, 0, skip_dma=True)
            for t in range(1, T):
                conv_frame(0, t)
            conv_frame(1, 0)
            conv_frame(1, 1)
            attention(0)
            # input prefetch 2 frames ahead: the spk DMA inside scan_step
            # waits on the scan result and blocks the SP queue behind it
            x_dma(1, 2)
            x_dma(1, 3)
            for t in range(T - 2):
                scan_step(0, t, OVERLAP_VG, OVERLAP_SP)
                conv_frame(1, t + 2, skip_dma=True)
                if t + 4 < T:
                    x_dma(1, t + 4)
            attention(1)
            scan_step(0, T - 2, OVERLAP_VG, OVERLAP_SP)
            scan_step(0, T - 1, OVERLAP_VG, OVERLAP_SP)
            for t in range(T):
                scan_step(1, t, TAIL_VG, TAIL_SP)

    nc.compile()
    return nc


def _trunc13(a):
    # f32r = round-to-nearest, 11 explicit mantissa bits (HW-verified via
    # DMA roundtrip). Split values must be 11-bit so the hardware re-round
    # is a no-op and x_hi + x_lo == x exactly.
    u = np.ascontiguousarray(a, np.float32).view(np.uint32)
    r = (u + np.uint32(0x800)) & np.uint32(0xFFFFF000)
    return r.view(np.float32)


def _pad_frames(x):
    """[.., 64, 32, 32] -> [.., 64, XCOL] host-packed shared-halo frames."""
    lead = x.shape[:-2]
    padded = np.zeros(lead + (34, PW), np.float32)
    padded[..., 1:33, 1:33] = x
    out = np.zeros(lead + (XCOL,), np.float32)
    out[..., :34 * PW] = padded.reshape(lead + (34 * PW,))
    return out


E4M3 = ml_dtypes.float8_e4m3fn


def _fp8(a):
    return np.asarray(a, np.float32).astype(E4M3)


def _prep_host_inputs(conv_w, conv_b, mlp_w1, mlp_w2):
    wT = np.ascontiguousarray(np.transpose(conv_w, (1, 0, 2, 3)))  # [64,128,3,3]
    hi = {}
    c8 = {}
    for dy, dx in TAPS:
        blk = np.ascontiguousarray(wT[:, :, dy + 1, dx + 1])
        h = _trunc13(blk)
        lo = (blk - h).astype(np.float32)
        hi[(dy, dx)] = h
        # fp8 corr weights: [w_lo*2^16 ; w_hi*2^4] (psum scale 2^16 with
        # x_lo prescaled by 2^12 on the data side)
        c8[(dy, dx)] = np.concatenate(
            [_fp8(lo * 65536.0), _fp8(h * 16.0)], axis=0)          # [128,128]
    # T1 weights prescaled by 2^16 (exact) to share the corr psum scale
    wpair = np.concatenate(
        [np.concatenate([hi[(-1, dx)], hi[(1, dx)]], axis=0)
         for dx in (-1, 0, 1)], axis=1) * 65536.0                  # [128, 384]
    wsing = np.concatenate(
        [hi[(0, dx)] for dx in (-1, 0, 1)], axis=1) * 65536.0
    wc66 = np.stack(
        [np.concatenate([c8[(-1, dx)] for dx in (-1, 0, 1)], axis=1),
         np.concatenate([c8[(1, dx)] for dx in (-1, 0, 1)], axis=1)],
        axis=1)                                                    # [128,2,384]
    wc2 = np.stack([c8[(0, -1)], c8[(0, 1)]], axis=1)              # [128,2,128]
    return {
        "wpair": np.ascontiguousarray(wpair, np.float32),
        "wsing": np.ascontiguousarray(wsing, np.float32),
        "wc66": np.ascontiguousarray(wc66),
        "wc2": np.ascontiguousarray(wc2),
        "wc0": np.ascontiguousarray(
            np.stack([c8[(0, 0)], np.zeros_like(c8[(0, 0)])], axis=1)),
        "bias": np.ascontiguousarray(conv_b.reshape(128, 1), np.float32),
        "w1t": np.ascontiguousarray(mlp_w1.T).astype(np.float32),
        "w2t": np.ascontiguousarray(mlp_w2.T).astype(np.float32),
        "ident": np.eye(128, dtype=np.float32),
    }


_CACHED = {}


def make_in_maps(data, conv_w, conv_b, mlp_w1, mlp_w2):
    data = np.ascontiguousarray(data, np.float32)
    common = _prep_host_inputs(np.asarray(conv_w, np.float32),
                               np.asarray(conv_b, np.float32),
                               np.asarray(mlp_w1, np.float32),
                               np.asarray(mlp_w2, np.float32))
    in_maps = []
    for c in range(N_CORES):
        m = dict(common)
        shard = _pad_frames(data[c * BPC:(c + 1) * BPC])
        h = _trunc13(shard)
        m["xhi"] = h
        # fp8 corr data: [fp8(x_hi) ; fp8(x_lo*2^12)] in two shifted planes
        c8 = np.concatenate(
            [_fp8(h), _fp8((shard - h) * 4096.0)], axis=2)  # [BPC,T,128,XCOL]
        x66 = np.zeros((BPC, T, 128, 2, MAR + XCOL), E4M3)
        x66[:, :, :, 0, MAR:MAR + XCOL] = c8
        x66[:, :, :, 1, 0:XCOL] = c8
        m["xc66"] = x66
        x2 = np.zeros((BPC, T, 128, 2, 2 + XCOL), E4M3)
        x2[:, :, :, 0, 2:2 + XCOL] = c8
        x2[:, :, :, 1, 0:XCOL] = c8
        m["xc2"] = x2
        in_maps.append(m)
    return in_maps


def kernel(data, conv_w, conv_b, mlp_w1, mlp_w2):
    if "prog" not in _CACHED:
        _CACHED["prog"] = _build_program()
    nc = _CACHED["prog"]
    in_maps = make_in_maps(data, conv_w, conv_b, mlp_w1, mlp_w2)
    res = run_bass_kernel_spmd(nc, in_maps, list(range(N_CORES)))
    out = np.concatenate(
        [np.asarray(res.results[c]["spk"]).astype(np.float32)
         for c in range(N_CORES)], axis=0)
    return out.reshape(B, T, CH, H, W)


# revision 15
# speedup vs baseline: 2.6414x; 1.0095x over previous
"""Trainium2 Bass kernel for nn_ConvAttLIF (conv3x3 + temporal attention + LIF scan).

Sharding: data-parallel over batch B=16 across 8 NeuronCores (2 samples/core).

Layout: frames host-packed with shared row halos (33-wide rows: the right
halo of row r is the left halo of row r+1, both zero), so a frame is 1124
contiguous cols and the conv output span is 1056 cols = 3 psum chunks of 352.

Conv: per chunk, 15 f32r matmuls accumulate one psum bank:
  - 3 "pair" units (K=128): taps (-1,dx) and (+1,dx) fused by storing a
    second frame copy shifted 2 rows (66 cols) in partitions 64-127.
  - 3 "single" units (K=64): taps (0,dx) on partitions 0-63.
  - 9 "corr" units (K=128): [x_hi; x_lo] . [w_lo; w_hi] per tap, restoring
    ~fp32 accuracy from the 12-bit f32r operands (x_hi = trunc13(x)).
Chunks are processed in rotating order (frame f starts at chunk f%3) so each
frame's first psum bank was drained one chunk-stream earlier - no PE stall.

LIF scan: attention folded in via v_t = u_t/att_t, so each step is
v = g*c_t + y (STT), g = (v < thr_t)*v (STT, same engine - no cross-engine
hop in the serial chain), spike = (v >= thr_t) off-chain. The sample-1 tail
(no conv left to overlap) splits rows across DVE/Pool/ACT.

kernel(**inputs) takes the FULL unsharded inputs, returns the FULL output.
"""
import sys

sys.path.insert(0, "/opt/trn_rl_repo")

import numpy as np
import ml_dtypes
import concourse.bass as bass
import concourse.bacc as bacc
import concourse.tile as tile
import concourse.mybir as mybir
from concourse.bass_utils import run_bass_kernel_spmd

F32 = mybir.dt.float32
F32R = mybir.dt.float32r
FP8 = mybir.dt.float8e4
BF16 = mybir.dt.bfloat16
DR = mybir.MatmulPerfMode.DoubleRow
AF = mybir.ActivationFunctionType
OP = mybir.AluOpType
AX = mybir.AxisListType

B, T, CIN, H, W = 16, 20, 64, 32, 32
CH = 128
N_CORES = 8
BPC = B // N_CORES
ALPHA, VTH = 0.3, 0.6
HW = H * W                     # 1024
PW = W + 1                     # 33: row stride (shared halo col)
XCOL = 34 * PW + 2             # 1124 packed frame cols (+2 guard)
MAR = 2 * PW                   # 66: left margin in XA for the shifted copy
CN = 352                       # psum chunk cols (3 x 352 = 1056 out span)
OUT0 = PW + 1                  # 34: first out position in frame coords
NY = 25                        # y-tile ring size
TAPS = [(dy, dx) for dy in (-1, 0, 1) for dx in (-1, 0, 1)]


def _build_program():
    nc = bacc.Bacc("TRN2", target_bir_lowering=False, debug=False,
                   num_devices=N_CORES)

    xhi_d = nc.dram_tensor("xhi", [BPC, T, CIN, XCOL], F32,
                           kind="ExternalInput").ap()
    xc66_d = nc.dram_tensor("xc66", [BPC, T, 128, 2, MAR + XCOL], FP8,
                            kind="ExternalInput").ap()
    xc2_d = nc.dram_tensor("xc2", [BPC, T, 128, 2, 2 + XCOL], FP8,
                           kind="ExternalInput").ap()
    wpair_d = nc.dram_tensor("wpair", [128, 3 * 128], F32,
                             kind="ExternalInput").ap()
    wsing_d = nc.dram_tensor("wsing", [64, 3 * 128], F32,
                             kind="ExternalInput").ap()
    wc66_d = nc.dram_tensor("wc66", [128, 2, 3 * 128], FP8,
                            kind="ExternalInput").ap()
    wc2_d = nc.dram_tensor("wc2", [128, 2, 128], FP8,
                           kind="ExternalInput").ap()
    wc0_d = nc.dram_tensor("wc0", [128, 2, 128], FP8,
                           kind="ExternalInput").ap()
    bias_d = nc.dram_tensor("bias", [128, 1], F32, kind="ExternalInput").ap()
    w1t_d = nc.dram_tensor("w1t", [T, 5], F32, kind="ExternalInput").ap()
    w2t_d = nc.dram_tensor("w2t", [5, T], F32, kind="ExternalInput").ap()
    ident_d = nc.dram_tensor("ident", [128, 128], F32, kind="ExternalInput").ap()
    spk = nc.dram_tensor("spk", [BPC, T, CH, HW], FP8,
                         kind="ExternalOutput").ap()

    with tile.TileContext(nc) as tc:
        with tc.tile_pool(name="sb", bufs=1) as P1, \
             tc.tile_pool(name="scr", bufs=2) as P2, \
             tc.tile_pool(name="so", bufs=3) as P3, \
             tc.tile_pool(name="ps", bufs=1, space="PSUM") as PP:

            # ---- persistent tiles ----
            xas = [P1.tile([128, MAR + XCOL], F32R, tag=f"xa{i}", name=f"xa{i}")
                   for i in range(4)]
            xc66s = [P1.tile([128, 2, MAR + XCOL], FP8, tag=f"x6{i}",
                             name=f"x6{i}") for i in range(4)]
            xc2s = [P1.tile([128, 2, 2 + XCOL], FP8, tag=f"x2{i}",
                            name=f"x2{i}") for i in range(4)]

            def x_dma(s, t):
                f = s * T + t
                src = xhi_d[s, t].bitcast(F32R)
                nc.sync.dma_start(xas[f % 4][0:64, MAR:MAR + XCOL], src)
                nc.sync.dma_start(xas[f % 4][64:128, 0:XCOL], src)
                nc.sync.dma_start(xc66s[f % 4][:], xc66_d[s, t])
                nc.sync.dma_start(xc2s[f % 4][:], xc2_d[s, t])

            # startup order: frame-0 XA halves, pair/single weights (first
            # units of the first chunk), then the corr inputs
            f0src = xhi_d[0, 0].bitcast(F32R)
            nc.sync.dma_start(xas[0][0:64, MAR:MAR + 420], f0src[:, 0:420])
            nc.sync.dma_start(xas[0][64:128, 0:486], f0src[:, 0:486])
            wpair = P1.tile([128, 3 * 128], F32R, tag="wpair", name="wpair")
            nc.sync.dma_start(wpair[:], wpair_d[:].bitcast(F32R))
            nc.sync.dma_start(xas[0][0:64, MAR + 420:MAR + XCOL],
                              f0src[:, 420:XCOL])
            nc.sync.dma_start(xas[0][64:128, 486:XCOL], f0src[:, 486:XCOL])
            wsing = P1.tile([64, 3 * 128], F32R, tag="wsing", name="wsing")
            nc.sync.dma_start(wsing[:], wsing_d[:].bitcast(F32R))
            nc.sync.dma_start(xc66s[0][:], xc66_d[0, 0])
            nc.sync.dma_start(xc2s[0][:], xc2_d[0, 0])
            bias_t = P1.tile([128, 1], F32, tag="bias", name="bias")
            nc.sync.dma_start(bias_t[:], bias_d[:])

            wc66_s = P1.tile([128, 2, 3 * 128], FP8, tag="wc66", name="wc66")
            nc.sync.dma_start(wc66_s[:], wc66_d[:])
            wc2_s = P1.tile([128, 2, 128], FP8, tag="wc2", name="wc2")
            nc.sync.dma_start(wc2_s[:], wc2_d[:])
            wc0_s = P1.tile([128, 2, 128], FP8, tag="wc0", name="wc0")
            nc.sync.dma_start(wc0_s[:], wc0_d[:])
            w1t_s = P1.tile([T, 5], F32, tag="w1t", name="w1t")
            nc.sync.dma_start(w1t_s[:], w1t_d[:])
            w2t_s = P1.tile([5, T], F32, tag="w2t", name="w2t")
            nc.sync.dma_start(w2t_s[:], w2t_d[:])
            ident = P1.tile([128, 128], F32, tag="ident", name="ident")
            nc.sync.dma_start(ident[:], ident_d[:])
            ones_t = P1.tile([1, 128], F32, tag="ones", name="ones")
            nc.vector.memset(ones_t[:], 1.0)

            ys = [P1.tile([128, XCOL], F32, tag=f"y{i}", name=f"y{i}")
                  for i in range(NY)]
            gs = [P1.tile([128, HW], F32, tag=f"g{s}", name=f"g{s}")
                  for s in range(BPC)]
            # stats rows: 0-2 chunk sums, 3 -junk, 4 total, 5 max
            s_st = [P1.tile([128, 6 * T], F32, tag=f"S{s}", name=f"S{s}")
                    for s in range(BPC)]
            bc = [P1.tile([128, 4 * T], F32, tag=f"bc{s}", name=f"bc{s}")
                  for s in range(BPC)]

            engines = {"v": nc.vector, "p": nc.gpsimd}

            def conv_frame(s, t, skip_dma=False):
                f = s * T + t
                if not skip_dma:
                    x_dma(s, t)
                xa, x6, x2 = xas[f % 4], xc66s[f % 4], xc2s[f % 4]
                y = ys[f % NY]
                for ci in range(3):
                    c = (f + ci) % 3
                    o = OUT0 + CN * c
                    ps = PP.tile([128, CN], F32, tag=f"p{c}{f % 2}",
                                 name=f"p{c}{f % 2}")
                    units = []
                    for i, dx in enumerate((-1, 0, 1)):
                        units.append((wpair[:, i * 128:(i + 1) * 128],
                                      xa[0:128, MAR + o - PW + dx:
                                         MAR + o - PW + dx + CN], None))
                    for i, dx in enumerate((-1, 0, 1)):
                        units.append((wsing[:, i * 128:(i + 1) * 128],
                                      xa[0:64, MAR + o + dx:MAR + o + dx + CN],
                                      None))
                    # fp8 DoubleRow corr: plane0/plane1 pair taps (-1,dx)
                    # with (+1,dx) (delta 2*PW) and (0,-1) with (0,+1)
                    for i, dx in enumerate((-1, 0, 1)):
                        b0 = MAR + o - PW + dx
                        units.append((wc66_s[:, 0:2, i * 128:(i + 1) * 128],
                                      x6[:, 0:2, b0:b0 + CN], DR))
                    units.append((wc2_s[:, 0:2, :],
                                  x2[:, 0:2, 2 + o - 1:2 + o - 1 + CN], DR))
                    # tap (0,0) as DoubleRow with a zeroed second plane
                    units.append((wc0_s[:, 0:2, :],
                                  x6[:, 0:2, MAR + o:MAR + o + CN], DR))
                    for k, (w_ap, x_ap, pm) in enumerate(units):
                        nc.tensor.matmul(ps[:], w_ap, x_ap,
                                         start=(k == 0),
                                         stop=(k == len(units) - 1),
                                         perf_mode=pm)
                    nc.scalar.activation(
                        y[:, o:o + CN], ps[:], AF.Identity,
                        bias=bias_t[:, 0:1], scale=1.0 / 65536.0,
                        accum_out=s_st[s][:, c * T + t:c * T + t + 1])
                # stats: -junk sum, max over real cols, total
                yj = y[:, MAR:MAR + 32 * PW].rearrange(
                    "p (r c) -> p r c", c=PW)
                nc.vector.reduce_sum(s_st[s][:, 3 * T + t:3 * T + t + 1],
                                     yj[:, :, 0:1], axis=AX.XY, negate=True)
                ym = y[:, OUT0:OUT0 + 32 * PW].rearrange(
                    "p (r c) -> p r c", c=PW)
                nc.vector.reduce_max(s_st[s][:, 5 * T + t:5 * T + t + 1],
                                     ym[:, :, 0:W], axis=AX.XY)
                sv = s_st[s].rearrange("p (k t) -> p k t", t=T)
                nc.vector.reduce_sum(sv[:, 4:5, t:t + 1], sv[:, 0:4, t:t + 1],
                                     axis=AX.XY)

            def attention(s):
                S = s_st[s]
                psT1 = PP.tile([T, 128], F32, tag="pa0", name="psT1")
                nc.tensor.transpose(psT1[:], S[:, 4 * T:5 * T], ident[:])
                psT2 = PP.tile([T, 128], F32, tag="pa1", name="psT2")
                nc.tensor.transpose(psT2[:], S[:, 5 * T:6 * T], ident[:])
                tmp = P2.tile([T, 1], F32, tag="att_tmp", name="att_tmp")
                nc.vector.reduce_sum(tmp[:], psT1[:], axis=AX.X)
                att_in = P2.tile([T, 2], F32, tag="att_in", name="att_in")
                nc.vector.tensor_scalar_mul(att_in[:, 0:1], tmp[:],
                                            1.0 / (CH * HW))
                nc.vector.reduce_max(att_in[:, 1:2], psT2[:], axis=AX.X)
                ps5 = PP.tile([5, 2], F32, tag="pa0", name="ps5")
                nc.tensor.matmul(ps5[:], w1t_s[:], att_in[:], start=True,
                                 stop=True)
                h5 = P2.tile([5, 2], F32, tag="h5", name="h5")
                nc.vector.tensor_scalar_max(h5[:], ps5[:], 0.0)
                ps20 = PP.tile([T, 2], F32, tag="pa1", name="ps20")
                nc.tensor.matmul(ps20[:], w2t_s[:], h5[:], start=True, stop=True)
                a20 = P2.tile([T, 2], F32, tag="a20", name="a20")
                nc.vector.tensor_scalar_add(a20[:], ps20[:], 0.0)
                attp = P2.tile([T, 1], F32, tag="attp", name="attp")
                nc.vector.tensor_tensor(attp[:], a20[:, 0:1], a20[:, 1:2],
                                        op=OP.add)
                # sigmoid via exp + reciprocal (tighter than the Sigmoid table)
                expz = P2.tile([T, 1], F32, tag="expz", name="expz")
                nc.scalar.activation(expz[:], attp[:], AF.Exp, scale=-1.0)
                att1 = P2.tile([T, 1], F32, tag="att1", name="att1")
                nc.vector.tensor_scalar_add(att1[:], expz[:], 1.0)
                att = P2.tile([T, 1], F32, tag="att", name="att")
                nc.vector.reciprocal(att[:], att1[:])
                psT3 = PP.tile([1, T], F32, tag="pa0", name="psT3")
                nc.tensor.transpose(psT3[:], att[:, 0:1], ident[0:T, 0:T])
                atts = P2.tile([1, T + 1], F32, tag="atts", name="atts")
                nc.vector.tensor_scalar_add(atts[0:1, 1:T + 1], psT3[:], 0.0)
                nc.vector.tensor_scalar_add(atts[0:1, 0:1], psT3[0:1, 0:1],
                                            0.0)
                rec = P2.tile([1, T], F32, tag="rec", name="rec")
                nc.vector.reciprocal(rec[:], atts[0:1, 1:T + 1])
                rhs3 = P2.tile([1, 4 * T], F32, tag="rhs3", name="rhs3")
                nc.vector.scalar_tensor_tensor(
                    rhs3[0:1, 0:T], atts[0:1, 0:T], ALPHA, rec[:],
                    op0=OP.mult, op1=OP.mult)
                nc.vector.tensor_scalar_mul(rhs3[0:1, T:2 * T], rec[:], VTH)
                nc.vector.tensor_scalar_mul(rhs3[0:1, 2 * T:3 * T], rec[:],
                                            -VTH)
                nc.vector.tensor_scalar_mul(rhs3[0:1, 3 * T:4 * T], rec[:],
                                            -VTH * 1e8)
                ps_bc = PP.tile([128, 4 * T], F32, tag="pa1", name="ps_bc")
                nc.tensor.matmul(ps_bc[:], ones_t[:], rhs3[:], start=True,
                                 stop=True)
                nc.vector.tensor_scalar_add(bc[s][:], ps_bc[:], 0.0)

            def scan_step(s, t, vg, sp):
                f = s * T + t
                g = gs[s]
                if t == 0:
                    nc.vector.memset(g[:], 0.0)
                y = ys[f % NY]
                yv = y[:, OUT0:OUT0 + 32 * PW].rearrange(
                    "p (r c) -> p r c", c=PW)
                v = P2.tile([128, HW], F32, tag="v", name="v")
                m = (P2.tile([128, HW], F32, tag="m", name="m")
                     if any(e == "p" for e, _, _ in vg) else None)
                so = P3.tile([128, HW], FP8, tag="so", name="so")
                vv = v.rearrange("p (r c) -> p r c", c=W)
                gv = g.rearrange("p (r c) -> p r c", c=W)
                cb = bc[s][:, t:t + 1]
                tn = min(t + 1, T - 1)
                cbn = bc[s][:, tn:tn + 1]
                thr = bc[s][:, T + t:T + t + 1]
                nthr = bc[s][:, 2 * T + t:2 * T + t + 1]
                nthr8 = bc[s][:, 3 * T + t:3 * T + t + 1]
                for eng, r0, r1 in vg:
                    R = slice(r0 // W, r1 // W)
                    if eng == "v":
                        nc.vector.scalar_tensor_tensor(
                            vv[:, R, :], gv[:, R, :], cb, yv[:, R, 0:W],
                            op0=OP.mult, op1=OP.add)
                        nc.vector.scalar_tensor_tensor(
                            g[:, r0:r1], v[:, r0:r1], thr, v[:, r0:r1],
                            op0=OP.is_lt, op1=OP.mult)
                    else:
                        # Pool rows keep g pre-multiplied by c_{t+1}:
                        # v = g + y; m = (v<thr)*c_next; g = m*v
                        nc.gpsimd.tensor_tensor(
                            vv[:, R, :], gv[:, R, :], yv[:, R, 0:W],
                            op=OP.add)
                        nc.gpsimd.tensor_scalar(
                            m[:, r0:r1], v[:, r0:r1], thr, cbn,
                            op0=OP.is_lt, op1=OP.mult)
                        nc.gpsimd.tensor_tensor(
                            g[:, r0:r1], m[:, r0:r1], v[:, r0:r1],
                            op=OP.mult)
                for eng, r0, r1 in sp:
                    if eng == "sig":
                        # saturated sigmoid: 1e8*(v - thr) is past the f32
                        # sigmoid saturation point except ~1e-7 from thr
                        nc.scalar.activation(so[:, r0:r1], v[:, r0:r1],
                                             AF.Sigmoid, bias=nthr8,
                                             scale=1e8)
                    elif eng == "pm":
                        # spike from m (= (v<thr)*c_next): exactly 0 iff spike
                        nc.gpsimd.tensor_scalar(
                            so[:, r0:r1], m[:, r0:r1], 0.0, None,
                            op0=OP.is_equal)
                    else:
                        nc.vector.tensor_scalar(
                            so[:, r0:r1], v[:, r0:r1], thr, None,
                            op0=OP.is_ge)
                nc.sync.dma_start(spk[s, t], so[:])

            OVERLAP_VG = [("v", 0, 896), ("p", 896, HW)]
            OVERLAP_SP = [("sig", 0, HW)]
            TAIL_VG = [("v", 0, 384), ("v", 384, 768), ("p", 768, HW)]
            TAIL_SP = [("sig", 0, HW)]

            conv_frame(0, 0, skip_dma=True)
            for t in range(1, T):
                conv_frame(0, t)
            for t in range(4):
                conv_frame(1, t)
            # att(0) after 4 conv(1) frames: its PE ops sit behind queued
            # conv matmuls while the DVE/ACT chain resolves
            attention(0)
            # input prefetch 2 frames ahead: the spk DMA inside scan_step
            # waits on the scan result and blocks the SP queue behind it
            x_dma(1, 4)
            x_dma(1, 5)
            for t in range(T - 4):
                scan_step(0, t, OVERLAP_VG, OVERLAP_SP)
                conv_frame(1, t + 4, skip_dma=True)
                if t + 6 < T:
                    x_dma(1, t + 6)
            attention(1)
            for t in range(T - 4, T):
                scan_step(0, t, OVERLAP_VG, OVERLAP_SP)
            for t in range(T):
                scan_step(1, t, TAIL_VG, TAIL_SP)

    nc.compile()
    return nc


def _trunc13(a):
    # f32r = round-to-nearest, 11 explicit mantissa bits (HW-verified via
    # DMA roundtrip). Split values must be 11-bit so the hardware re-round
    # is a no-op and x_hi + x_lo == x exactly.
    u = np.ascontiguousarray(a, np.float32).view(np.uint32)
    r = (u + np.uint32(0x800)) & np.uint32(0xFFFFF000)
    return r.view(np.float32)


def _pad_frames(x):
    """[.., 64, 32, 32] -> [.., 64, XCOL] host-packed shared-halo frames."""
    lead = x.shape[:-2]
    padded = np.zeros(lead + (34, PW), np.float32)
    padded[..., 1:33, 1:33] = x
    out = np.zeros(lead + (XCOL,), np.float32)
    out[..., :34 * PW] = padded.reshape(lead + (34 * PW,))
    return out


E4M3 = ml_dtypes.float8_e4m3fn


def _fp8(a):
    return np.asarray(a, np.float32).astype(E4M3)


def _prep_host_inputs(conv_w, conv_b, mlp_w1, mlp_w2):
    wT = np.ascontiguousarray(np.transpose(conv_w, (1, 0, 2, 3)))  # [64,128,3,3]
    hi = {}
    c8 = {}
    for dy, dx in TAPS:
        blk = np.ascontiguousarray(wT[:, :, dy + 1, dx + 1])
        h = _trunc13(blk)
        lo = (blk - h).astype(np.float32)
        hi[(dy, dx)] = h
        # fp8 corr weights: [w_lo*2^16 ; w_hi*2^4] (psum scale 2^16 with
        # x_lo prescaled by 2^12 on the data side)
        c8[(dy, dx)] = np.concatenate(
            [_fp8(lo * 65536.0), _fp8(h * 16.0)], axis=0)          # [128,128]
    # T1 weights prescaled by 2^16 (exact) to share the corr psum scale
    wpair = np.concatenate(
        [np.concatenate([hi[(-1, dx)], hi[(1, dx)]], axis=0)
         for dx in (-1, 0, 1)], axis=1) * 65536.0                  # [128, 384]
    wsing = np.concatenate(
        [hi[(0, dx)] for dx in (-1, 0, 1)], axis=1) * 65536.0
    wc66 = np.stack(
        [np.concatenate([c8[(-1, dx)] for dx in (-1, 0, 1)], axis=1),
         np.concatenate([c8[(1, dx)] for dx in (-1, 0, 1)], axis=1)],
        axis=1)                                                    # [128,2,384]
    wc2 = np.stack([c8[(0, -1)], c8[(0, 1)]], axis=1)              # [128,2,128]
    return {
        "wpair": np.ascontiguousarray(wpair, np.float32),
        "wsing": np.ascontiguousarray(wsing, np.float32),
        "wc66": np.ascontiguousarray(wc66),
        "wc2": np.ascontiguousarray(wc2),
        "wc0": np.ascontiguousarray(
            np.stack([c8[(0, 0)], np.zeros_like(c8[(0, 0)])], axis=1)),
        "bias": np.ascontiguousarray(conv_b.reshape(128, 1), np.float32),
        "w1t": np.ascontiguousarray(mlp_w1.T).astype(np.float32),
        "w2t": np.ascontiguousarray(mlp_w2.T).astype(np.float32),
        "ident": np.eye(128, dtype=np.float32),
    }


_CACHED = {}


def make_in_maps(data, conv_w, conv_b, mlp_w1, mlp_w2):
    data = np.ascontiguousarray(data, np.float32)
    common = _prep_host_inputs(np.asarray(conv_w, np.float32),
                               np.asarray(conv_b, np.float32),
                               np.asarray(mlp_w1, np.float32),
                               np.asarray(mlp_w2, np.float32))
    in_maps = []
    for c in range(N_CORES):
        m = dict(common)
        shard = _pad_frames(data[c * BPC:(c + 1) * BPC])
        h = _trunc13(shard)
        m["xhi"] = h
        # fp8 corr data: [fp8(x_hi) ; fp8(x_lo*2^12)] in two shifted planes
        c8 = np.concatenate(
            [_fp8(h), _fp8((shard - h) * 4096.0)], axis=2)  # [BPC,T,128,XCOL]
        x66 = np.zeros((BPC, T, 128, 2, MAR + XCOL), E4M3)
        x66[:, :, :, 0, MAR:MAR + XCOL] = c8
        x66[:, :, :, 1, 0:XCOL] = c8
        m["xc66"] = x66
        x2 = np.zeros((BPC, T, 128, 2, 2 + XCOL), E4M3)
        x2[:, :, :, 0, 2:2 + XCOL] = c8
        x2[:, :, :, 1, 0:XCOL] = c8
        m["xc2"] = x2
        in_maps.append(m)
    return in_maps


def kernel(data, conv_w, conv_b, mlp_w1, mlp_w2):
    if "prog" not in _CACHED:
        _CACHED["prog"] = _build_program()
    nc = _CACHED["prog"]
    in_maps = make_in_maps(data, conv_w, conv_b, mlp_w1, mlp_w2)
    res = run_bass_kernel_spmd(nc, in_maps, list(range(N_CORES)))
    out = np.concatenate(
        [np.asarray(res.results[c]["spk"]).astype(np.float32)
         for c in range(N_CORES)], axis=0)
    return out.reshape(B, T, CH, H, W)


# revision 16
# speedup vs baseline: 2.6590x; 1.0067x over previous
"""Trainium2 Bass kernel for nn_ConvAttLIF (conv3x3 + temporal attention + LIF scan).

Sharding: data-parallel over batch B=16 across 8 NeuronCores (2 samples/core).

Layout: frames host-packed with shared row halos (33-wide rows: the right
halo of row r is the left halo of row r+1, both zero), so a frame is 1124
contiguous cols and the conv output span is 1056 cols = 3 psum chunks of 352.

Conv: per chunk, 15 f32r matmuls accumulate one psum bank:
  - 3 "pair" units (K=128): taps (-1,dx) and (+1,dx) fused by storing a
    second frame copy shifted 2 rows (66 cols) in partitions 64-127.
  - 3 "single" units (K=64): taps (0,dx) on partitions 0-63.
  - 9 "corr" units (K=128): [x_hi; x_lo] . [w_lo; w_hi] per tap, restoring
    ~fp32 accuracy from the 12-bit f32r operands (x_hi = trunc13(x)).
Chunks are processed in rotating order (frame f starts at chunk f%3) so each
frame's first psum bank was drained one chunk-stream earlier - no PE stall.

LIF scan: attention folded in via v_t = u_t/att_t, so each step is
v = g*c_t + y (STT), g = (v < thr_t)*v (STT, same engine - no cross-engine
hop in the serial chain), spike = (v >= thr_t) off-chain. The sample-1 tail
(no conv left to overlap) splits rows across DVE/Pool/ACT.

kernel(**inputs) takes the FULL unsharded inputs, returns the FULL output.
"""
import sys

sys.path.insert(0, "/opt/trn_rl_repo")

import numpy as np
import ml_dtypes
import concourse.bass as bass
import concourse.bacc as bacc
import concourse.tile as tile
import concourse.mybir as mybir
from concourse.bass_utils import run_bass_kernel_spmd

F32 = mybir.dt.float32
F32R = mybir.dt.float32r
FP8 = mybir.dt.float8e4
BF16 = mybir.dt.bfloat16
DR = mybir.MatmulPerfMode.DoubleRow
AF = mybir.ActivationFunctionType
OP = mybir.AluOpType
AX = mybir.AxisListType

B, T, CIN, H, W = 16, 20, 64, 32, 32
CH = 128
N_CORES = 8
BPC = B // N_CORES
ALPHA, VTH = 0.3, 0.6
HW = H * W                     # 1024
PW = W + 1                     # 33: row stride (shared halo col)
XCOL = 34 * PW + 2             # 1124 packed frame cols (+2 guard)
MAR = 2 * PW                   # 66: left margin in XA for the shifted copy
CN = 352                       # psum chunk cols (3 x 352 = 1056 out span)
OUT0 = PW + 1                  # 34: first out position in frame coords
NY = 25                        # y-tile ring size
TAPS = [(dy, dx) for dy in (-1, 0, 1) for dx in (-1, 0, 1)]


def _build_program():
    nc = bacc.Bacc("TRN2", target_bir_lowering=False, debug=False,
                   num_devices=N_CORES)

    xhi_d = nc.dram_tensor("xhi", [BPC, T, CIN, XCOL], F32,
                           kind="ExternalInput").ap()
    xc66_d = nc.dram_tensor("xc66", [BPC, T, 128, 2, MAR + XCOL], FP8,
                            kind="ExternalInput").ap()
    xc2_d = nc.dram_tensor("xc2", [BPC, T, 128, 2, 2 + XCOL], FP8,
                           kind="ExternalInput").ap()
    wpair_d = nc.dram_tensor("wpair", [128, 3 * 128], F32,
                             kind="ExternalInput").ap()
    wsing_d = nc.dram_tensor("wsing", [64, 3 * 128], F32,
                             kind="ExternalInput").ap()
    wc66_d = nc.dram_tensor("wc66", [128, 2, 3 * 128], FP8,
                            kind="ExternalInput").ap()
    wc2_d = nc.dram_tensor("wc2", [128, 2, 128], FP8,
                           kind="ExternalInput").ap()
    wc0_d = nc.dram_tensor("wc0", [128, 2, 128], FP8,
                           kind="ExternalInput").ap()
    bias_d = nc.dram_tensor("bias", [128, 1], F32, kind="ExternalInput").ap()
    w1t_d = nc.dram_tensor("w1t", [T, 5], F32, kind="ExternalInput").ap()
    w2t_d = nc.dram_tensor("w2t", [5, T], F32, kind="ExternalInput").ap()
    ident_d = nc.dram_tensor("ident", [128, 128], F32, kind="ExternalInput").ap()
    spk = nc.dram_tensor("spk", [BPC, T, CH, HW], FP8,
                         kind="ExternalOutput").ap()

    with tile.TileContext(nc) as tc:
        with tc.tile_pool(name="sb", bufs=1) as P1, \
             tc.tile_pool(name="scr", bufs=2) as P2, \
             tc.tile_pool(name="so", bufs=3) as P3, \
             tc.tile_pool(name="ps", bufs=1, space="PSUM") as PP:

            # ---- persistent tiles ----
            xas = [P1.tile([128, MAR + XCOL], F32R, tag=f"xa{i}", name=f"xa{i}")
                   for i in range(4)]
            xc66s = [P1.tile([128, 2, MAR + XCOL], FP8, tag=f"x6{i}",
                             name=f"x6{i}") for i in range(4)]
            xc2s = [P1.tile([128, 2, 2 + XCOL], FP8, tag=f"x2{i}",
                            name=f"x2{i}") for i in range(4)]

            def x_dma(s, t):
                f = s * T + t
                src = xhi_d[s, t].bitcast(F32R)
                nc.sync.dma_start(xas[f % 4][0:64, MAR:MAR + XCOL], src)
                nc.sync.dma_start(xas[f % 4][64:128, 0:XCOL], src)
                nc.sync.dma_start(xc66s[f % 4][:], xc66_d[s, t])
                nc.sync.dma_start(xc2s[f % 4][:], xc2_d[s, t])

            # startup order: frame-0 XA halves, pair/single weights (first
            # units of the first chunk), then the corr inputs
            f0src = xhi_d[0, 0].bitcast(F32R)
            nc.sync.dma_start(xas[0][0:64, MAR:MAR + XCOL], f0src)
            nc.sync.dma_start(xas[0][64:128, 0:XCOL], f0src)
            wpair = P1.tile([128, 3 * 128], F32R, tag="wpair", name="wpair")
            nc.sync.dma_start(wpair[:], wpair_d[:].bitcast(F32R))
            wsing = P1.tile([64, 3 * 128], F32R, tag="wsing", name="wsing")
            nc.sync.dma_start(wsing[:], wsing_d[:].bitcast(F32R))
            nc.sync.dma_start(xc66s[0][:], xc66_d[0, 0])
            nc.sync.dma_start(xc2s[0][:], xc2_d[0, 0])
            bias_t = P1.tile([128, 1], F32, tag="bias", name="bias")
            nc.sync.dma_start(bias_t[:], bias_d[:])

            wc66_s = P1.tile([128, 2, 3 * 128], FP8, tag="wc66", name="wc66")
            nc.sync.dma_start(wc66_s[:], wc66_d[:])
            wc2_s = P1.tile([128, 2, 128], FP8, tag="wc2", name="wc2")
            nc.sync.dma_start(wc2_s[:], wc2_d[:])
            wc0_s = P1.tile([128, 2, 128], FP8, tag="wc0", name="wc0")
            nc.sync.dma_start(wc0_s[:], wc0_d[:])
            w1t_s = P1.tile([T, 5], F32, tag="w1t", name="w1t")
            nc.sync.dma_start(w1t_s[:], w1t_d[:])
            w2t_s = P1.tile([5, T], F32, tag="w2t", name="w2t")
            nc.sync.dma_start(w2t_s[:], w2t_d[:])
            ident = P1.tile([128, 128], F32, tag="ident", name="ident")
            nc.sync.dma_start(ident[:], ident_d[:])
            ones_t = P1.tile([1, 128], F32, tag="ones", name="ones")
            nc.vector.memset(ones_t[:], 1.0)

            ys = [P1.tile([128, XCOL], F32, tag=f"y{i}", name=f"y{i}")
                  for i in range(NY)]
            gs = [P1.tile([128, HW], F32, tag=f"g{s}", name=f"g{s}")
                  for s in range(BPC)]
            # stats rows: 0-2 chunk sums, 3 -junk, 4 total, 5 max
            s_st = [P1.tile([128, 6 * T], F32, tag=f"S{s}", name=f"S{s}")
                    for s in range(BPC)]
            bc = [P1.tile([128, 4 * T], F32, tag=f"bc{s}", name=f"bc{s}")
                  for s in range(BPC)]

            engines = {"v": nc.vector, "p": nc.gpsimd}

            def conv_frame(s, t, skip_dma=False):
                f = s * T + t
                if not skip_dma:
                    x_dma(s, t)
                xa, x6, x2 = xas[f % 4], xc66s[f % 4], xc2s[f % 4]
                y = ys[f % NY]
                for ci in range(3):
                    c = (f + ci) % 3
                    o = OUT0 + CN * c
                    ps = PP.tile([128, CN], F32, tag=f"p{c}{f % 2}",
                                 name=f"p{c}{f % 2}")
                    units = []
                    for i, dx in enumerate((-1, 0, 1)):
                        units.append((wpair[:, i * 128:(i + 1) * 128],
                                      xa[0:128, MAR + o - PW + dx:
                                         MAR + o - PW + dx + CN], None))
                    for i, dx in enumerate((-1, 0, 1)):
                        units.append((wsing[:, i * 128:(i + 1) * 128],
                                      xa[0:64, MAR + o + dx:MAR + o + dx + CN],
                                      None))
                    # fp8 DoubleRow corr: plane0/plane1 pair taps (-1,dx)
                    # with (+1,dx) (delta 2*PW) and (0,-1) with (0,+1)
                    for i, dx in enumerate((-1, 0, 1)):
                        b0 = MAR + o - PW + dx
                        units.append((wc66_s[:, 0:2, i * 128:(i + 1) * 128],
                                      x6[:, 0:2, b0:b0 + CN], DR))
                    units.append((wc2_s[:, 0:2, :],
                                  x2[:, 0:2, 2 + o - 1:2 + o - 1 + CN], DR))
                    # tap (0,0) as DoubleRow with a zeroed second plane
                    units.append((wc0_s[:, 0:2, :],
                                  x6[:, 0:2, MAR + o:MAR + o + CN], DR))
                    for k, (w_ap, x_ap, pm) in enumerate(units):
                        nc.tensor.matmul(ps[:], w_ap, x_ap,
                                         start=(k == 0),
                                         stop=(k == len(units) - 1),
                                         perf_mode=pm)
                    nc.scalar.activation(
                        y[:, o:o + CN], ps[:], AF.Identity,
                        bias=bias_t[:, 0:1], scale=1.0 / 65536.0,
                        accum_out=s_st[s][:, c * T + t:c * T + t + 1])
                # stats: -junk sum, max over real cols, total
                yj = y[:, MAR:MAR + 32 * PW].rearrange(
                    "p (r c) -> p r c", c=PW)
                nc.vector.reduce_sum(s_st[s][:, 3 * T + t:3 * T + t + 1],
                                     yj[:, :, 0:1], axis=AX.XY, negate=True)
                ym = y[:, OUT0:OUT0 + 32 * PW].rearrange(
                    "p (r c) -> p r c", c=PW)
                nc.vector.reduce_max(s_st[s][:, 5 * T + t:5 * T + t + 1],
                                     ym[:, :, 0:W], axis=AX.XY)
                sv = s_st[s].rearrange("p (k t) -> p k t", t=T)
                nc.vector.reduce_sum(sv[:, 4:5, t:t + 1], sv[:, 0:4, t:t + 1],
                                     axis=AX.XY)

            def attention(s):
                S = s_st[s]
                psT1 = PP.tile([T, 128], F32, tag="pa0", name="psT1")
                nc.tensor.transpose(psT1[:], S[:, 4 * T:5 * T], ident[:])
                psT2 = PP.tile([T, 128], F32, tag="pa1", name="psT2")
                nc.tensor.transpose(psT2[:], S[:, 5 * T:6 * T], ident[:])
                tmp = P2.tile([T, 1], F32, tag="att_tmp", name="att_tmp")
                nc.vector.reduce_sum(tmp[:], psT1[:], axis=AX.X)
                att_in = P2.tile([T, 2], F32, tag="att_in", name="att_in")
                nc.vector.tensor_scalar_mul(att_in[:, 0:1], tmp[:],
                                            1.0 / (CH * HW))
                nc.vector.reduce_max(att_in[:, 1:2], psT2[:], axis=AX.X)
                ps5 = PP.tile([5, 2], F32, tag="pa0", name="ps5")
                nc.tensor.matmul(ps5[:], w1t_s[:], att_in[:], start=True,
                                 stop=True)
                h5 = P2.tile([5, 2], F32, tag="h5", name="h5")
                nc.vector.tensor_scalar_max(h5[:], ps5[:], 0.0)
                ps20 = PP.tile([T, 2], F32, tag="pa1", name="ps20")
                nc.tensor.matmul(ps20[:], w2t_s[:], h5[:], start=True, stop=True)
                a20 = P2.tile([T, 2], F32, tag="a20", name="a20")
                nc.vector.tensor_scalar_add(a20[:], ps20[:], 0.0)
                attp = P2.tile([T, 1], F32, tag="attp", name="attp")
                nc.vector.tensor_tensor(attp[:], a20[:, 0:1], a20[:, 1:2],
                                        op=OP.add)
                # sigmoid via exp + reciprocal (tighter than the Sigmoid table)
                expz = P2.tile([T, 1], F32, tag="expz", name="expz")
                nc.scalar.activation(expz[:], attp[:], AF.Exp, scale=-1.0)
                att1 = P2.tile([T, 1], F32, tag="att1", name="att1")
                nc.vector.tensor_scalar_add(att1[:], expz[:], 1.0)
                att = P2.tile([T, 1], F32, tag="att", name="att")
                nc.vector.reciprocal(att[:], att1[:])
                psT3 = PP.tile([1, T], F32, tag="pa0", name="psT3")
                nc.tensor.transpose(psT3[:], att[:, 0:1], ident[0:T, 0:T])
                atts = P2.tile([1, T + 1], F32, tag="atts", name="atts")
                nc.vector.tensor_scalar_add(atts[0:1, 1:T + 1], psT3[:], 0.0)
                nc.vector.tensor_scalar_add(atts[0:1, 0:1], psT3[0:1, 0:1],
                                            0.0)
                rec = P2.tile([1, T], F32, tag="rec", name="rec")
                nc.vector.reciprocal(rec[:], atts[0:1, 1:T + 1])
                rhs3 = P2.tile([1, 4 * T], F32, tag="rhs3", name="rhs3")
                nc.vector.scalar_tensor_tensor(
                    rhs3[0:1, 0:T], atts[0:1, 0:T], ALPHA, rec[:],
                    op0=OP.mult, op1=OP.mult)
                nc.vector.tensor_scalar_mul(rhs3[0:1, T:2 * T], rec[:], VTH)
                nc.vector.tensor_scalar_mul(rhs3[0:1, 2 * T:3 * T], rec[:],
                                            -VTH)
                nc.vector.tensor_scalar_mul(rhs3[0:1, 3 * T:4 * T], rec[:],
                                            -VTH * 1e8)
                ps_bc = PP.tile([128, 4 * T], F32, tag="pa1", name="ps_bc")
                nc.tensor.matmul(ps_bc[:], ones_t[:], rhs3[:], start=True,
                                 stop=True)
                nc.vector.tensor_scalar_add(bc[s][:], ps_bc[:], 0.0)

            def scan_step(s, t, vg, sp):
                f = s * T + t
                g = gs[s]
                if t == 0:
                    nc.vector.memset(g[:], 0.0)
                y = ys[f % NY]
                yv = y[:, OUT0:OUT0 + 32 * PW].rearrange(
                    "p (r c) -> p r c", c=PW)
                v = P2.tile([128, HW], F32, tag="v", name="v")
                m = (P2.tile([128, HW], F32, tag="m", name="m")
                     if any(e == "p" for e, _, _ in vg) else None)
                so = P3.tile([128, HW], FP8, tag="so", name="so")
                vv = v.rearrange("p (r c) -> p r c", c=W)
                gv = g.rearrange("p (r c) -> p r c", c=W)
                cb = bc[s][:, t:t + 1]
                tn = min(t + 1, T - 1)
                cbn = bc[s][:, tn:tn + 1]
                thr = bc[s][:, T + t:T + t + 1]
                nthr = bc[s][:, 2 * T + t:2 * T + t + 1]
                nthr8 = bc[s][:, 3 * T + t:3 * T + t + 1]
                for eng, r0, r1 in vg:
                    R = slice(r0 // W, r1 // W)
                    if eng == "v":
                        nc.vector.scalar_tensor_tensor(
                            vv[:, R, :], gv[:, R, :], cb, yv[:, R, 0:W],
                            op0=OP.mult, op1=OP.add)
                        nc.vector.scalar_tensor_tensor(
                            g[:, r0:r1], v[:, r0:r1], thr, v[:, r0:r1],
                            op0=OP.is_lt, op1=OP.mult)
                    else:
                        # Pool rows keep g pre-multiplied by c_{t+1}:
                        # v = g + y; m = (v<thr)*c_next; g = m*v
                        nc.gpsimd.tensor_tensor(
                            vv[:, R, :], gv[:, R, :], yv[:, R, 0:W],
                            op=OP.add)
                        nc.gpsimd.tensor_scalar(
                            m[:, r0:r1], v[:, r0:r1], thr, cbn,
                            op0=OP.is_lt, op1=OP.mult)
                        nc.gpsimd.tensor_tensor(
                            g[:, r0:r1], m[:, r0:r1], v[:, r0:r1],
                            op=OP.mult)
                for eng, r0, r1 in sp:
                    if eng == "sig":
                        # saturated sigmoid: 1e8*(v - thr) is past the f32
                        # sigmoid saturation point except ~1e-7 from thr
                        nc.scalar.activation(so[:, r0:r1], v[:, r0:r1],
                                             AF.Sigmoid, bias=nthr8,
                                             scale=1e8)
                    elif eng == "pm":
                        # spike from m (= (v<thr)*c_next): exactly 0 iff spike
                        nc.gpsimd.tensor_scalar(
                            so[:, r0:r1], m[:, r0:r1], 0.0, None,
                            op0=OP.is_equal)
                    else:
                        nc.vector.tensor_scalar(
                            so[:, r0:r1], v[:, r0:r1], thr, None,
                            op0=OP.is_ge)
                nc.sync.dma_start(spk[s, t], so[:])

            OVERLAP_VG = [("v", 0, 896), ("p", 896, HW)]
            OVERLAP_SP = [("sig", 0, HW)]
            TAIL_VG = [("v", 0, 384), ("v", 384, 768), ("p", 768, HW)]
            TAIL_SP = [("sig", 0, HW)]

            conv_frame(0, 0, skip_dma=True)
            for t in range(1, T):
                conv_frame(0, t)
            for t in range(4):
                conv_frame(1, t)
            # att(0) after 4 conv(1) frames: its PE ops sit behind queued
            # conv matmuls while the DVE/ACT chain resolves
            attention(0)
            # input prefetch 2 frames ahead: the spk DMA inside scan_step
            # waits on the scan result and blocks the SP queue behind it
            x_dma(1, 4)
            x_dma(1, 5)
            for t in range(T - 4):
                scan_step(0, t, OVERLAP_VG, OVERLAP_SP)
                conv_frame(1, t + 4, skip_dma=True)
                if t + 6 < T:
                    x_dma(1, t + 6)
            attention(1)
            for t in range(T - 4, T):
                scan_step(0, t, OVERLAP_VG, OVERLAP_SP)
            for t in range(T):
                scan_step(1, t, TAIL_VG, TAIL_SP)

    nc.compile()
    return nc


def _trunc13(a):
    # f32r = round-to-nearest, 11 explicit mantissa bits (HW-verified via
    # DMA roundtrip). Split values must be 11-bit so the hardware re-round
    # is a no-op and x_hi + x_lo == x exactly.
    u = np.ascontiguousarray(a, np.float32).view(np.uint32)
    r = (u + np.uint32(0x800)) & np.uint32(0xFFFFF000)
    return r.view(np.float32)


def _pad_frames(x):
    """[.., 64, 32, 32] -> [.., 64, XCOL] host-packed shared-halo frames."""
    lead = x.shape[:-2]
    padded = np.zeros(lead + (34, PW), np.float32)
    padded[..., 1:33, 1:33] = x
    out = np.zeros(lead + (XCOL,), np.float32)
    out[..., :34 * PW] = padded.reshape(lead + (34 * PW,))
    return out


E4M3 = ml_dtypes.float8_e4m3fn


def _fp8(a):
    return np.asarray(a, np.float32).astype(E4M3)


def _prep_host_inputs(conv_w, conv_b, mlp_w1, mlp_w2):
    wT = np.ascontiguousarray(np.transpose(conv_w, (1, 0, 2, 3)))  # [64,128,3,3]
    hi = {}
    c8 = {}
    for dy, dx in TAPS:
        blk = np.ascontiguousarray(wT[:, :, dy + 1, dx + 1])
        h = _trunc13(blk)
        lo = (blk - h).astype(np.float32)
        hi[(dy, dx)] = h
        # fp8 corr weights: [w_lo*2^16 ; w_hi*2^4] (psum scale 2^16 with
        # x_lo prescaled by 2^12 on the data side)
        c8[(dy, dx)] = np.concatenate(
            [_fp8(lo * 65536.0), _fp8(h * 16.0)], axis=0)          # [128,128]
    # T1 weights prescaled by 2^16 (exact) to share the corr psum scale
    wpair = np.concatenate(
        [np.concatenate([hi[(-1, dx)], hi[(1, dx)]], axis=0)
         for dx in (-1, 0, 1)], axis=1) * 65536.0                  # [128, 384]
    wsing = np.concatenate(
        [hi[(0, dx)] for dx in (-1, 0, 1)], axis=1) * 65536.0
    wc66 = np.stack(
        [np.concatenate([c8[(-1, dx)] for dx in (-1, 0, 1)], axis=1),
         np.concatenate([c8[(1, dx)] for dx in (-1, 0, 1)], axis=1)],
        axis=1)                                                    # [128,2,384]
    wc2 = np.stack([c8[(0, -1)], c8[(0, 1)]], axis=1)              # [128,2,128]
    return {
        "wpair": np.ascontiguousarray(wpair, np.float32),
        "wsing": np.ascontiguousarray(wsing, np.float32),
        "wc66": np.ascontiguousarray(wc66),
        "wc2": np.ascontiguousarray(wc2),
        "wc0": np.ascontiguousarray(
            np.stack([c8[(0, 0)], np.zeros_like(c8[(0, 0)])], axis=1)),
        "bias": np.ascontiguousarray(conv_b.reshape(128, 1), np.float32),
        "w1t": np.ascontiguousarray(mlp_w1.T).astype(np.float32),
        "w2t": np.ascontiguousarray(mlp_w2.T).astype(np.float32),
        "ident": np.eye(128, dtype=np.float32),
    }


_CACHED = {}


def make_in_maps(data, conv_w, conv_b, mlp_w1, mlp_w2):
    data = np.ascontiguousarray(data, np.float32)
    common = _prep_host_inputs(np.asarray(conv_w, np.float32),
                               np.asarray(conv_b, np.float32),
                               np.asarray(mlp_w1, np.float32),
                               np.asarray(mlp_w2, np.float32))
    in_maps = []
    for c in range(N_CORES):
        m = dict(common)
        shard = _pad_frames(data[c * BPC:(c + 1) * BPC])
        h = _trunc13(shard)
        m["xhi"] = h
        # fp8 corr data: [fp8(x_hi) ; fp8(x_lo*2^12)] in two shifted planes
        c8 = np.concatenate(
            [_fp8(h), _fp8((shard - h) * 4096.0)], axis=2)  # [BPC,T,128,XCOL]
        x66 = np.zeros((BPC, T, 128, 2, MAR + XCOL), E4M3)
        x66[:, :, :, 0, MAR:MAR + XCOL] = c8
        x66[:, :, :, 1, 0:XCOL] = c8
        m["xc66"] = x66
        x2 = np.zeros((BPC, T, 128, 2, 2 + XCOL), E4M3)
        x2[:, :, :, 0, 2:2 + XCOL] = c8
        x2[:, :, :, 1, 0:XCOL] = c8
        m["xc2"] = x2
        in_maps.append(m)
    return in_maps


def kernel(data, conv_w, conv_b, mlp_w1, mlp_w2):
    if "prog" not in _CACHED:
        _CACHED["prog"] = _build_program()
    nc = _CACHED["prog"]
    in_maps = make_in_maps(data, conv_w, conv_b, mlp_w1, mlp_w2)
    res = run_bass_kernel_spmd(nc, in_maps, list(range(N_CORES)))
    out = np.concatenate(
        [np.asarray(res.results[c]["spk"]).astype(np.float32)
         for c in range(N_CORES)], axis=0)
    return out.reshape(B, T, CH, H, W)
